# revision 1
# baseline (speedup 1.0000x reference)
"""Trainium2 Bass kernel for a dense transformer block.

Reference computation (B=2, T=2048, D=2048, H=16, Dk=128, FF=8192, fp32):
    h   = rmsnorm(x, g1)
    qkv = h @ w_attn.T ; q,k = rope(q,k) ; y = causal_softmax(q k^T / sqrt(Dk)) v
    x1  = x + y @ w_proj.T
    h2  = rmsnorm(x1, g2)
    out = x1 + (silu(h2 @ w_gate.T) * (h2 @ w_up.T)) @ w_down.T

Distribution: data-parallel over tokens. Each of the 8 NeuronCores owns 512
contiguous tokens (cores 0-3: batch 0, cores 4-7: batch 1). Every core
computes K,V for its own tokens, the K/V are AllGather'd inside each 4-core
batch group, and attention/MLP for the core's rows is fully local. Weights
are replicated (streamed from HBM once per core, bf16).

Matmuls run in bf16 with fp32 PSUM accumulation. Residuals/normalization in
fp32. RoPE is applied in the transposed [dk, t] layout via a host-side
permutation of the head dimension (pairs land 16 lanes apart within each
32-partition quadrant) + DVE stream_shuffle.
"""

import os
import sys
import threading
import time

import numpy as np

for _p in ("/opt/trn_rl_repo", os.path.expanduser("~/.axon_site/_ro/trn_rl_repo")):
    if _p not in sys.path and os.path.isdir(_p):
        sys.path.append(_p)

import ml_dtypes  # noqa: E402

import concourse.bass as bass  # noqa: E402
import concourse.mybir as mybir  # noqa: E402
import concourse.tile as tile  # noqa: E402
from concourse import bacc  # noqa: E402
from concourse.bass_utils import run_bass_kernel_spmd  # noqa: E402
from concourse.masks import make_identity  # noqa: E402
from contextlib import ExitStack  # noqa: E402

F32 = mybir.dt.float32
BF16 = mybir.dt.bfloat16
AF = mybir.ActivationFunctionType
ALU = mybir.AluOpType

B, T, D = 2, 2048, 2048
H, DK, FF = 16, 128, 8192
EPS = 1e-6
N_CORES = 8
TLOC = T * B // N_CORES          # 512 tokens per core
CORES_PER_B = N_CORES // B       # 4
KT = D // 128                    # 16 d-tiles
NT = TLOC // 128                 # 4 t-tiles per core
NKT = T // 128                   # 16 key subtiles (full sequence)
FT_FF = FF // 128                # 64 ff tiles
SCALE = 1.0 / float(np.sqrt(DK))
SHUF_MASK = [(j + 16) % 32 for j in range(32)]


def _rope_perm():
    """Within-head row permutation: pair i=(16*qd + j) real part -> partition
    32*qd + j, imag part -> partition 32*qd + 16 + j."""
    perm = np.zeros(DK, dtype=np.int64)
    for p in range(DK):
        qd, j = p // 32, p % 32
        i = 16 * qd + (j if j < 16 else j - 16)
        perm[p] = 2 * i + (0 if j < 16 else 1)
    return perm


def build_program(sim=False, repeat=1):
    nc = bacc.Bacc("TRN2", target_bir_lowering=False, debug=False,
                   num_devices=1 if sim else N_CORES)

    x_d = nc.declare_dram_parameter("x", [TLOC, D], F32, isOutput=False)
    qkw_d = nc.declare_dram_parameter("qk_w", [2 * H, 128, D], BF16, isOutput=False)
    vw_d = nc.declare_dram_parameter("v_w", [KT, 128, D], BF16, isOutput=False)
    pw_d = nc.declare_dram_parameter("proj_w", [H, 128, D], BF16, isOutput=False)
    gw_d = nc.declare_dram_parameter("gate_w", [FT_FF, 128, D], BF16, isOutput=False)
    uw_d = nc.declare_dram_parameter("up_w", [FT_FF, 128, D], BF16, isOutput=False)
    dw_d = nc.declare_dram_parameter("down_w", [FT_FF, 128, D], BF16, isOutput=False)
    cs1_d = nc.declare_dram_parameter("cs1", [128, TLOC], F32, isOutput=False)
    cs2_d = nc.declare_dram_parameter("cs2", [128, TLOC], F32, isOutput=False)
    msk_d = nc.declare_dram_parameter("masks", [NKT, 128, TLOC], BF16, isOutput=False)
    out_d = nc.declare_dram_parameter("out", [TLOC, D], F32, isOutput=True)

    with ExitStack() as ctx:
        tc = ctx.enter_context(tile.TileContext(nc))
        for _rep in range(repeat):
            _emit_block(nc, tc, sim, x_d, qkw_d, vw_d, pw_d, gw_d, uw_d, dw_d,
                        cs1_d, cs2_d, msk_d, out_d)

    nc.compile()
    return nc


def _emit_block(nc, tc, sim, x_d, qkw_d, vw_d, pw_d, gw_d, uw_d, dw_d,
                cs1_d, cs2_d, msk_d, out_d):
    with ExitStack() as ctx:
        const = ctx.enter_context(tc.tile_pool(name="const", bufs=1))
        ident = const.tile([128, 128], BF16)
        make_identity(nc, ident)
        ones_col = const.tile([128, 1], BF16)
        nc.vector.memset(ones_col, 1.0)
        ones_row = const.tile([1, 128], F32)
        nc.vector.memset(ones_row, 1.0)
        cs1_sb = const.tile([128, TLOC], F32)
        nc.sync.dma_start(out=cs1_sb[:], in_=cs1_d[:, :])
        cs2_sb = const.tile([128, TLOC], F32)
        nc.sync.dma_start(out=cs2_sb[:], in_=cs2_d[:, :])

        # DRAM scratch: K/V allgather buffers + x1 spill
        dram = ctx.enter_context(tc.tile_pool(name="dram", bufs=1, space="DRAM"))
        k_local = dram.tile([H, 128, TLOC], BF16)
        v_local = dram.tile([NT, D // 512, 128, 512], BF16)
        k_full = dram.tile([CORES_PER_B, H, 128, TLOC], BF16)
        v_full = dram.tile([CORES_PER_B, NT, D // 512, 128, 512], BF16)

        def rmsnorm_transpose(loader, dst_sb, pool, psum_pool):
            """loader(it) -> [128, D] fp32 AP; writes dst_sb [128, KT, TLOC]
            bf16 = (rms-normalized rows) transposed. Gains folded in weights.
            Square runs per 512-col chunk so partial sums overlap producers."""
            hrows = []
            ssqs = []
            for it in range(NT):
                sq_scr = pool.tile([128, D], BF16, name="sq_scr")
                ssq = pool.tile([128, 1], F32, name=f"ssq{it}", tag=f"ssq{it}")
                nc.scalar.activation(sq_scr[:], loader(it), AF.Square,
                                     accum_out=ssq[:])
                ssqs.append(ssq)
            for it in range(NT):
                src = loader(it)
                mean = pool.tile([128, 1], F32, name="mean")
                nc.vector.tensor_scalar(mean[:], ssqs[it][:], 1.0 / D, EPS,
                                        ALU.mult, ALU.add)
                rec = pool.tile([128, 1], F32, name="rec")
                nc.vector.reciprocal(rec[:], mean[:])
                rstd = pool.tile([128, 1], F32, name="rstd")
                nc.scalar.activation(rstd[:], rec[:], AF.Sqrt)
                hrow = pool.tile([128, D], BF16, name=f"hrow{it}",
                                 tag=f"hrow{it}")
                nc.vector.tensor_scalar(hrow[:], src, rstd[:], None, ALU.mult)
                hrows.append(hrow)
            # k-outer so dst_sb[:, k, :] completes early for the consumers
            for k in range(KT):
                for it in range(NT):
                    tp = psum_pool.tile([128, 128], BF16, name="tp")
                    nc.tensor.transpose(tp[:], hrows[it][:, k * 128:(k + 1) * 128],
                                        ident[:])
                    nc.vector.tensor_copy(dst_sb[:, k, it * 128:(it + 1) * 128],
                                          tp[:])

        def rope_evict(ps, dst, pool):
            """ps: [128, TLOC] psum q/k head tile (permuted lanes) -> rotated"""
            sh = pool.tile([128, TLOC], F32, name="rp_sh")
            nc.vector.stream_shuffle(sh[:], ps[:], mask=SHUF_MASK)
            t1 = pool.tile([128, TLOC], F32, name="rp_t1")
            nc.vector.tensor_tensor(t1[:], ps[:], cs1_sb[:], ALU.mult)
            t2 = pool.tile([128, TLOC], F32, name="rp_t2")
            nc.vector.tensor_tensor(t2[:], sh[:], cs2_sb[:], ALU.mult)
            nc.vector.tensor_tensor(dst[:], t1[:], t2[:], ALU.add)

        # persistent pools, strict LIFO: x1 | hT | x(ph1) | qrot | y
        x1_cm = tc.tile_pool(name="x1pool", bufs=1)
        x1pool = x1_cm.__enter__()
        x1_sb = x1pool.tile([128, NT, D], F32)
        hT_cm = tc.tile_pool(name="hT_pool", bufs=1)
        hT_pool = hT_cm.__enter__()
        hT_sb = hT_pool.tile([128, KT, TLOC], BF16)

        # ---------------- phase 1: norm1 + h^T ----------------
        x_cm = tc.tile_pool(name="xpool", bufs=1)
        xpool = x_cm.__enter__()
        x_sb = xpool.tile([128, NT, D], F32)
        for it in range(NT):
            nc.sync.dma_start(out=x_sb[:, it, :],
                              in_=x_d[it * 128:(it + 1) * 128, :])
        with ExitStack() as ph:
            pool = ph.enter_context(tc.tile_pool(name="n1_pool", bufs=2))
            psum_pool = ph.enter_context(
                tc.tile_pool(name="n1_psum", bufs=4, space="PSUM"))
            rmsnorm_transpose(lambda it: x_sb[:, it, :], hT_sb, pool, psum_pool)
        x_cm.__exit__(None, None, None)

        qrot_cm = tc.tile_pool(name="qrot_pool", bufs=1)
        qrot_pool = qrot_cm.__enter__()
        qrot_sb = qrot_pool.tile([128, H, TLOC], BF16)
        # ---------------- phase 2: QKV + rope + allgathers ----------------
        with ExitStack() as ph:
            wpool = ph.enter_context(tc.tile_pool(name="qkv_w", bufs=4))
            spool = ph.enter_context(tc.tile_pool(name="qkv_s", bufs=3))
            pspool = ph.enter_context(
                tc.tile_pool(name="qkv_ps", bufs=3, space="PSUM"))
            vpspool = ph.enter_context(
                tc.tile_pool(name="v_psp", bufs=1, space="PSUM"))
            vwpool = ph.enter_context(tc.tile_pool(name="vw_pool", bufs=10))

            # K heads first; allgather K while V/Q compute
            for h in range(H):
                wt = wpool.tile([128, KT, 128], BF16, name="qk_wt")
                nc.sync.dma_start(out=wt[:], in_=qkw_d[H + h].rearrange(
                    "p (k c) -> p k c", k=KT))
                ps = pspool.tile([128, TLOC], F32, name="qk_ps")
                for k in range(KT):
                    nc.tensor.matmul(ps[:], wt[:, k, :], hT_sb[:, k, :],
                                     start=(k == 0), stop=(k == KT - 1))
                krot = spool.tile([128, TLOC], BF16, name="krot")
                rope_evict(ps, krot[:], spool)
                nc.sync.dma_start(out=k_local[h], in_=krot[:])
            if sim:
                for r in range(CORES_PER_B):
                    nc.gpsimd.dma_start(out=k_full[r], in_=k_local[:])
            else:
                nc.gpsimd.collective_compute(
                    "AllGather", ALU.bypass,
                    replica_groups=[[0, 1, 2, 3], [4, 5, 6, 7]],
                    ins=[k_local.opt()], outs=[k_full.opt()],
                )

            # V: out[t, f] accumulated per (it, fb)
            for fb in range(D // 512):
                vps = [vpspool.tile([128, 512], F32, name=f"v_ps{it}",
                                    tag=f"v_ps{it}") for it in range(NT)]
                for k in range(KT):
                    vwt = vwpool.tile([128, 512], BF16, name="vwt")
                    nc.sync.dma_start(out=vwt[:],
                                      in_=vw_d[k][:, fb * 512:(fb + 1) * 512])
                    for it in range(NT):
                        nc.tensor.matmul(vps[it][:],
                                         hT_sb[:, k, it * 128:(it + 1) * 128],
                                         vwt[:], start=(k == 0), stop=(k == KT - 1))
                for it in range(NT):
                    vsb = spool.tile([128, 512], BF16, name="vsb")
                    nc.scalar.copy(vsb[:], vps[it][:])
                    nc.sync.dma_start(out=v_local[it, fb], in_=vsb[:])
            if sim:
                for r in range(CORES_PER_B):
                    nc.gpsimd.dma_start(out=v_full[r], in_=v_local[:])
            else:
                nc.gpsimd.collective_compute(
                    "AllGather", ALU.bypass,
                    replica_groups=[[0, 1, 2, 3], [4, 5, 6, 7]],
                    ins=[v_local.opt()], outs=[v_full.opt()],
                )

            # Q weights prefetched in phase-2 scope end
        # (phase-2 weight/psum pools closed here)

        # y + v_all persist through attention; v_all DMAs overlap Q compute
        y_cm = tc.tile_pool(name="y_pool", bufs=1)
        y_pool = y_cm.__enter__()
        y_sb = y_pool.tile([128, H, TLOC], BF16)
        vall_cm = tc.tile_pool(name="vall_pool", bufs=1)
        vap = vall_cm.__enter__()
        v_all = vap.tile([128, NKT, D], BF16)
        for r in range(CORES_PER_B):
            for it in range(NT):
                for fb in range(D // 512):
                    nc.gpsimd.dma_start(
                        out=v_all[:, r * NT + it, fb * 512:(fb + 1) * 512],
                        in_=v_full[r, it, fb])

        # Q heads + rope (stay in SBUF); overlaps the V allgather
        with ExitStack() as ph:
            wpool = ph.enter_context(tc.tile_pool(name="q_w", bufs=3))
            spool = ph.enter_context(tc.tile_pool(name="q_s", bufs=3))
            pspool = ph.enter_context(
                tc.tile_pool(name="q_ps", bufs=3, space="PSUM"))
            for h in range(H):
                wt = wpool.tile([128, KT, 128], BF16, name="qk_wt")
                nc.sync.dma_start(out=wt[:], in_=qkw_d[h].rearrange(
                    "p (k c) -> p k c", k=KT))
                ps = pspool.tile([128, TLOC], F32, name="qk_ps")
                for k in range(KT):
                    nc.tensor.matmul(ps[:], wt[:, k, :], hT_sb[:, k, :],
                                     start=(k == 0), stop=(k == KT - 1))
                rope_evict(ps, qrot_sb[:, h, :], spool)

        # ---------------- phase 3: attention ----------------
        with ExitStack() as ph:
            mpool = ph.enter_context(tc.tile_pool(name="msk_pool", bufs=1))
            masks_sb = mpool.tile([128, NKT, TLOC], BF16)
            for n in range(NKT):
                nc.gpsimd.dma_start(out=masks_sb[:, n, :], in_=msk_d[n])
            apool = ph.enter_context(tc.tile_pool(name="att_pool", bufs=3))
            epool = ph.enter_context(tc.tile_pool(name="exp_pool", bufs=8))
            aps = ph.enter_context(tc.tile_pool(name="att_ps", bufs=2, space="PSUM"))
            sps_pool = ph.enter_context(
                tc.tile_pool(name="sps_pool", bufs=3, space="PSUM"))
            bps_pool = ph.enter_context(
                tc.tile_pool(name="bps_pool", bufs=1, space="PSUM"))

            for h in range(H):
                kT_sb = apool.tile([128, T], BF16, name="kT_sb")
                for r in range(CORES_PER_B):
                    nc.gpsimd.dma_start(out=kT_sb[:, r * TLOC:(r + 1) * TLOC],
                                      in_=k_full[r, h])
                yps = aps.tile([128, TLOC], F32, name="y_ps", tag="y_ps")
                sums = aps.tile([1, TLOC], F32, name="sums_ps", tag="sums_ps")
                for kt in range(NKT):
                    sps = sps_pool.tile([128, TLOC], F32, name="s_ps", tag="s_ps")
                    nc.tensor.matmul(sps[:], kT_sb[:, kt * 128:(kt + 1) * 128],
                                     qrot_sb[:, h, :], start=True, stop=True)
                    et = epool.tile([128, TLOC], BF16, name="et")
                    nc.scalar.activation(et[:], sps[:], AF.Exp, scale=SCALE)
                    em = epool.tile([128, TLOC], BF16, name="em")
                    nc.vector.tensor_tensor(em[:], et[:], masks_sb[:, kt, :],
                                            ALU.mult)
                    nc.tensor.matmul(yps[:],
                                     v_all[:, kt, h * 128:(h + 1) * 128], em[:],
                                     start=(kt == 0), stop=(kt == NKT - 1))
                    nc.tensor.matmul(sums[:], ones_col[:], em[:],
                                     start=(kt == 0), stop=(kt == NKT - 1))
                rec = apool.tile([1, TLOC], F32, name="rec_att")
                nc.vector.reciprocal(rec[:], sums[:])
                bps = bps_pool.tile([128, TLOC], F32, name="b_ps", tag="b_ps")
                nc.tensor.matmul(bps[:], ones_row[:], rec[:], start=True,
                                 stop=True)
                bsb = apool.tile([128, TLOC], F32, name="bsb")
                nc.vector.tensor_copy(bsb[:], bps[:])
                nc.vector.tensor_tensor(y_sb[:, h, :], yps[:], bsb[:], ALU.mult)
        vall_cm.__exit__(None, None, None)

        # ---------------- phase 4: proj + residual -> x1 (SBUF) ----------
        with ExitStack() as ph:
            spool = ph.enter_context(tc.tile_pool(name="pj_s", bufs=3))
            pwpool = ph.enter_context(tc.tile_pool(name="pw_pool", bufs=8))
            pps = ph.enter_context(tc.tile_pool(name="pj_ps", bufs=2, space="PSUM"))
            for fb in range(D // 512):
                pps_t = [pps.tile([128, 512], F32, name=f"p_ps{it}",
                                  tag=f"p_ps{it}") for it in range(NT)]
                for hd in range(H):
                    pwt = pwpool.tile([128, 512], BF16, name="pwt")
                    nc.sync.dma_start(out=pwt[:],
                                      in_=pw_d[hd][:, fb * 512:(fb + 1) * 512])
                    for it in range(NT):
                        nc.tensor.matmul(pps_t[it][:],
                                         y_sb[:, hd, it * 128:(it + 1) * 128],
                                         pwt[:], start=(hd == 0),
                                         stop=(hd == H - 1))
                for it in range(NT):
                    xr = spool.tile([128, 512], F32, name="xr_p")
                    nc.sync.dma_start(
                        out=xr[:],
                        in_=x_d[it * 128:(it + 1) * 128,
                                fb * 512:(fb + 1) * 512])
                    nc.vector.tensor_tensor(
                        x1_sb[:, it, fb * 512:(fb + 1) * 512], pps_t[it][:],
                        xr[:], ALU.add)

        y_cm.__exit__(None, None, None)
        qrot_cm.__exit__(None, None, None)
        hT_cm.__exit__(None, None, None)

        # ---------------- phase 5: norm2 + h2^T ----------------
        h2T_cm = tc.tile_pool(name="h2T_pool", bufs=1)
        h2T_pool = h2T_cm.__enter__()
        h2T_sb = h2T_pool.tile([128, KT, TLOC], BF16)
        with ExitStack() as ph:
            pool = ph.enter_context(tc.tile_pool(name="n2_pool", bufs=2))
            psum_pool = ph.enter_context(
                tc.tile_pool(name="n2_psum", bufs=4, space="PSUM"))
            rmsnorm_transpose(lambda it: x1_sb[:, it, :], h2T_sb, pool, psum_pool)

        # ---------------- phase 6: gate/up ----------------
        gu_cm = tc.tile_pool(name="gu_pool", bufs=1)
        gu_pool = gu_cm.__enter__()
        gu_sb = gu_pool.tile([128, FT_FF, TLOC], BF16)
        with ExitStack() as ph:
            wpool = ph.enter_context(tc.tile_pool(name="mlp_w", bufs=3))
            spool = ph.enter_context(tc.tile_pool(name="mlp_s", bufs=3))
            mps = ph.enter_context(tc.tile_pool(name="mlp_ps", bufs=4, space="PSUM"))
            for f in range(FT_FF):
                gwt = wpool.tile([128, KT, 128], BF16, name="gwt")
                nc.sync.dma_start(out=gwt[:], in_=gw_d[f].rearrange(
                    "p (k c) -> p k c", k=KT))
                gps = mps.tile([128, TLOC], F32, name="g_ps", tag="g_ps")
                for k in range(KT):
                    nc.tensor.matmul(gps[:], gwt[:, k, :], h2T_sb[:, k, :],
                                     start=(k == 0), stop=(k == KT - 1))
                gsil = spool.tile([128, TLOC], BF16, name="gsil")
                nc.scalar.activation(gsil[:], gps[:], AF.Silu)
                uwt = wpool.tile([128, KT, 128], BF16, name="uwt")
                nc.sync.dma_start(out=uwt[:], in_=uw_d[f].rearrange(
                    "p (k c) -> p k c", k=KT))
                ups = mps.tile([128, TLOC], F32, name="u_ps", tag="u_ps")
                for k in range(KT):
                    nc.tensor.matmul(ups[:], uwt[:, k, :], h2T_sb[:, k, :],
                                     start=(k == 0), stop=(k == KT - 1))
                nc.vector.tensor_tensor(gu_sb[:, f, :], ups[:], gsil[:],
                                        ALU.mult)

        # ---------------- phase 7: down + residual -> out ----------------
        with ExitStack() as ph:
            spool = ph.enter_context(tc.tile_pool(name="dn_s", bufs=8))
            dps = ph.enter_context(tc.tile_pool(name="dn_ps", bufs=1, space="PSUM"))
            for fbp in range(2):
                dps_t = [[dps.tile([128, 512], F32, name=f"d_ps{it}_{fbi}",
                                   tag=f"d_ps{it}_{fbi}") for fbi in range(2)]
                         for it in range(NT)]
                for k in range(FT_FF):
                    dwt = spool.tile([128, 1024], BF16, name="dwt")
                    nc.sync.dma_start(
                        out=dwt[:],
                        in_=dw_d[k][:, fbp * 1024:(fbp + 1) * 1024])
                    for it in range(NT):
                        for fbi in range(2):
                            nc.tensor.matmul(
                                dps_t[it][fbi][:],
                                gu_sb[:, k, it * 128:(it + 1) * 128],
                                dwt[:, fbi * 512:(fbi + 1) * 512],
                                start=(k == 0), stop=(k == FT_FF - 1))
                for it in range(NT):
                    for fbi in range(2):
                        fb = fbp * 2 + fbi
                        osb = spool.tile([128, 512], F32, name="osb_d")
                        nc.vector.tensor_tensor(
                            osb[:], dps_t[it][fbi][:],
                            x1_sb[:, it, fb * 512:(fb + 1) * 512], ALU.add)
                        nc.sync.dma_start(
                            out=out_d[it * 128:(it + 1) * 128,
                                      fb * 512:(fb + 1) * 512],
                            in_=osb[:])

        gu_cm.__exit__(None, None, None)
        h2T_cm.__exit__(None, None, None)
        x1_cm.__exit__(None, None, None)


def prepare_inputs(x, f_cos, f_sin, w_attn, w_proj, w_gate, w_up, w_down, g1, g2):
    """Host-side sharding + weight re-layout. Returns list of 8 input dicts."""
    x = np.asarray(x, dtype=np.float32)
    f_cos = np.asarray(f_cos, dtype=np.float32)
    f_sin = np.asarray(f_sin, dtype=np.float32)
    w_attn = np.asarray(w_attn, dtype=np.float32)
    g1 = np.asarray(g1, dtype=np.float32)
    g2 = np.asarray(g2, dtype=np.float32)

    perm = _rope_perm()
    wq = w_attn[0:D] * g1[None, :]
    wk = w_attn[D:2 * D] * g1[None, :]
    wv = w_attn[2 * D:3 * D] * g1[None, :]
    # permute rows within each head for q and k
    wq_p = wq.reshape(H, DK, D)[:, perm, :].reshape(H * DK, D)
    wk_p = wk.reshape(H, DK, D)[:, perm, :].reshape(H * DK, D)

    def lhsT_layout(w):  # w: [F, D] -> [F/128, 128(d within k-tile), D(k*128+c)]
        f = w.shape[0]
        # out[ft, p, k*128+c] = w[ft*128+c, k*128+p]
        a = w.reshape(f // 128, 128, KT, 128)       # [ft, c, k, p]
        a = a.transpose(0, 3, 2, 1).reshape(f // 128, 128, D)  # [ft, p, (k c)]
        return np.ascontiguousarray(a).astype(ml_dtypes.bfloat16)

    def rhsT_layout(w):  # w: [F, D_in] -> [D_in/128, 128(p), F] = w.T tiled
        d_in = w.shape[1]
        a = w.T.reshape(d_in // 128, 128, w.shape[0])  # [k, p, c]
        return np.ascontiguousarray(a).astype(ml_dtypes.bfloat16)

    qk_w = np.concatenate([lhsT_layout(wq_p), lhsT_layout(wk_p)], axis=0)
    v_w = rhsT_layout(wv)
    proj_w = rhsT_layout(np.asarray(w_proj, dtype=np.float32))
    gate_w = lhsT_layout(np.asarray(w_gate, dtype=np.float32) * g2[None, :])
    up_w = lhsT_layout(np.asarray(w_up, dtype=np.float32) * g2[None, :])
    down_w = rhsT_layout(np.asarray(w_down, dtype=np.float32))

    # cs1/cs2 in permuted-lane layout: [128, T]
    pair = np.zeros(DK, dtype=np.int64)
    sign = np.zeros(DK, dtype=np.float32)
    for p in range(DK):
        qd, j = p // 32, p % 32
        pair[p] = 16 * qd + (j if j < 16 else j - 16)
        sign[p] = -1.0 if j < 16 else 1.0
    cs1_full = f_cos.T[pair, :]                       # [128, T]
    cs2_full = f_sin.T[pair, :] * sign[:, None]       # [128, T]

    tok = np.arange(T)
    in_maps = []
    for core in range(N_CORES):
        b, c = core // CORES_PER_B, core % CORES_PER_B
        t0 = c * TLOC
        masks = (np.arange(NKT * 128)[None, :] <= (t0 + np.arange(TLOC))[:, None])
        masks = np.ascontiguousarray(
            masks.T.reshape(NKT, 128, TLOC)).astype(ml_dtypes.bfloat16)
        in_maps.append({
            "x": np.ascontiguousarray(x[b, t0:t0 + TLOC, :]),
            "qk_w": qk_w, "v_w": v_w, "proj_w": proj_w,
            "gate_w": gate_w, "up_w": up_w, "down_w": down_w,
            "cs1": np.ascontiguousarray(cs1_full[:, t0:t0 + TLOC]),
            "cs2": np.ascontiguousarray(cs2_full[:, t0:t0 + TLOC]),
            "masks": masks,
        })
    return in_maps


def assemble_output(results):
    out = np.zeros((B, T, D), dtype=np.float32)
    for core in range(N_CORES):
        b, c = core // CORES_PER_B, core % CORES_PER_B
        t0 = c * TLOC
        out[b, t0:t0 + TLOC, :] = results[core]["out"]
    return out


_CACHE = {}
_LOCK = threading.Lock()


def get_program():
    with _LOCK:
        if "nc" not in _CACHE:
            _CACHE["nc"] = build_program()
        return _CACHE["nc"]


def kernel(**inputs):
    nc = get_program()
    in_maps = prepare_inputs(**inputs)
    res = run_bass_kernel_spmd(nc, in_maps, list(range(N_CORES)))
    return assemble_output(res.results)


def bench(inputs, iters=10):
    """Wall-clock the sharded executable with device-resident inputs.

    Returns the min per-call time in ns (upper bound on HW exec time: it
    includes one dispatch round-trip)."""
    import jax
    from jax.sharding import Mesh, PartitionSpec, NamedSharding
    from jax.experimental.shard_map import shard_map
    from concourse import bass2jax, mybir as mb

    nc = get_program()
    in_maps = prepare_inputs(**inputs)
    bass2jax.install_neuronx_cc_hook()

    partition_name = (nc.partition_id_tensor.name
                      if nc.partition_id_tensor else None)
    in_names, out_names, out_avals, zero_outs = [], [], [], []
    for alloc in nc.m.functions[0].allocations:
        if not isinstance(alloc, mb.MemoryLocationSet):
            continue
        name = alloc.memorylocations[0].name
        if alloc.kind == "ExternalInput":
            if name != partition_name:
                in_names.append(name)
        elif alloc.kind == "ExternalOutput":
            shape = tuple(alloc.tensor_shape)
            dtype = mb.dt.np(alloc.dtype)
            out_names.append(name)
            out_avals.append(jax.core.ShapedArray(shape, dtype))
            zero_outs.append(np.zeros(shape, dtype))
    n_params = len(in_names)
    all_in_names = list(in_names) + list(out_names)
    if partition_name is not None:
        all_in_names.append(partition_name)
    donate = tuple(range(n_params, n_params + len(out_names)))

    def _body(*args):
        operands = list(args)
        if partition_name is not None:
            operands.append(bass2jax.partition_id_tensor())
        return tuple(bass2jax._bass_exec_p.bind(
            *operands,
            out_avals=tuple(out_avals),
            in_names=tuple(all_in_names),
            out_names=tuple(out_names),
            lowering_input_output_aliases=(),
            sim_require_finite=True,
            sim_require_nnan=True,
            nc=nc,
        ))

    devices = jax.devices()[:N_CORES]
    mesh = Mesh(np.asarray(devices), ("core",))
    in_specs = (PartitionSpec("core"),) * (n_params + len(out_names))
    out_specs = (PartitionSpec("core"),) * len(out_names)
    sharded = jax.jit(
        shard_map(_body, mesh=mesh, in_specs=in_specs, out_specs=out_specs,
                  check_rep=False),
        donate_argnums=donate, keep_unused=True)

    sh = NamedSharding(mesh, PartitionSpec("core"))
    concat_in = [
        jax.device_put(
            np.concatenate([np.asarray(in_maps[c][nm]) for c in range(N_CORES)],
                           axis=0), sh)
        for nm in in_names]
    jax.block_until_ready(concat_in)

    def make_zeros():
        return [jax.device_put(
            np.zeros((N_CORES * z.shape[0], *z.shape[1:]), z.dtype), sh)
            for z in zero_outs]

    # warmup (compile)
    outs = sharded(*concat_in, *make_zeros())
    jax.block_until_ready(outs)

    zs = [make_zeros() for _ in range(iters)]
    for z in zs:
        jax.block_until_ready(z)
    # async pipelined dispatch amortizes the ~50ms axon round-trip
    t0 = time.perf_counter()
    outs = [sharded(*concat_in, *zs[i]) for i in range(iters)]
    jax.block_until_ready(outs)
    dt = (time.perf_counter() - t0) / iters
    return dt * 1e9



# revision 13
# speedup vs baseline: 1.4122x; 1.4122x over previous
"""Trainium2 Bass kernel for a dense transformer block.

Reference computation (B=2, T=2048, D=2048, H=16, Dk=128, FF=8192, fp32):
    h   = rmsnorm(x, g1)
    qkv = h @ w_attn.T ; q,k = rope(q,k) ; y = causal_softmax(q k^T / sqrt(Dk)) v
    x1  = x + y @ w_proj.T
    h2  = rmsnorm(x1, g2)
    out = x1 + (silu(h2 @ w_gate.T) * (h2 @ w_up.T)) @ w_down.T

Distribution: data-parallel over tokens, 512 per core (cores 0-3: batch 0,
cores 4-7: batch 1). Token tiles are "snake"-folded across the 4-core group:
core c owns global 128-token tiles {c, 7-c, 8+c, 15-c}, so every core's
causal key footprint is identical (tiles 0..3 attend 4 key tiles, 4..7
attend 8, 8..11 attend 12, 12..15 attend 16 -> 62.5% of the dense score/AV
work, perfectly balanced). Causal masking within the padded footprint is
data-driven (per-core 0/1 mask tiles multiply the exp'd scores), which keeps
the SPMD program identical on all cores. K,V are computed locally and
AllGather'd inside each 4-core group (V gathers split per 512-col block so
attention can start while late blocks are still in flight).

All weight matrices stream through one shared SBUF pool, so the DMA queue
naturally prefetches the next phase's weights while the current phase
computes. Matmuls run in bf16 with fp32 PSUM accumulation. Residuals and
normalization in fp32. RoPE is applied in the transposed [dk, t] layout via
a host-side permutation of the head dimension + DVE stream_shuffle.
"""

import os
import sys
import threading
import time

import numpy as np

for _p in ("/opt/trn_rl_repo", os.path.expanduser("~/.axon_site/_ro/trn_rl_repo")):
    if _p not in sys.path and os.path.isdir(_p):
        sys.path.append(_p)

import ml_dtypes  # noqa: E402

import concourse.bass as bass  # noqa: E402
import concourse.mybir as mybir  # noqa: E402
import concourse.tile as tile  # noqa: E402
from concourse import bacc  # noqa: E402
from concourse.bass_utils import run_bass_kernel_spmd  # noqa: E402
from concourse.masks import make_identity  # noqa: E402
from contextlib import ExitStack  # noqa: E402

F32 = mybir.dt.float32
BF16 = mybir.dt.bfloat16
AF = mybir.ActivationFunctionType
ALU = mybir.AluOpType

B, T, D = 2, 2048, 2048
H, DK, FF = 16, 128, 8192
EPS = 1e-6
N_CORES = 8
TLOC = T * B // N_CORES          # 512 tokens per core
CORES_PER_B = N_CORES // B       # 4
KT = D // 128                    # 16 d-tiles
NT = TLOC // 128                 # 4 t-tiles per core
NKT = T // 128                   # 16 key subtiles (full sequence)
FT_FF = FF // 128                # 64 ff tiles
NFB = D // 512                   # 4 v/proj 512-col blocks
SCALE = 1.0 / float(np.sqrt(DK))
SHUF_MASK = [(j + 16) % 32 for j in range(32)]


def snake_tiles(c):
    """Global 128-token tile indices owned by group-core c, local order."""
    return [c, 7 - c, 8 + c, 15 - c]


def _gmaps():
    """global tile g -> (owning group-core, local tile index)."""
    rmap, lmap = [0] * NKT, [0] * NKT
    for g in range(NKT):
        for r in range(CORES_PER_B):
            if g in snake_tiles(r):
                rmap[g], lmap[g] = r, snake_tiles(r).index(g)
    return rmap, lmap


RMAP, LMAP = _gmaps()
# core-major position of global tile g inside gathered K/V SBUF tiles
POS = [RMAP[g] * NT + LMAP[g] for g in range(NKT)]


def _rope_perm():
    """Within-head row permutation: pair i=(16*qd + j) real part -> partition
    32*qd + j, imag part -> partition 32*qd + 16 + j."""
    perm = np.zeros(DK, dtype=np.int64)
    for p in range(DK):
        qd, j = p // 32, p % 32
        i = 16 * qd + (j if j < 16 else j - 16)
        perm[p] = 2 * i + (0 if j < 16 else 1)
    return perm


def build_program(sim=False, repeat=1):
    nc = bacc.Bacc("TRN2", target_bir_lowering=False, debug=False,
                   num_devices=1 if sim else N_CORES)

    x_d = nc.declare_dram_parameter("x", [TLOC, D], F32, isOutput=False)
    qkw_d = nc.declare_dram_parameter("qk_w", [2 * H, 128, D], BF16, isOutput=False)
    vw_d = nc.declare_dram_parameter("v_w", [KT, 128, D], BF16, isOutput=False)
    pw_d = nc.declare_dram_parameter("proj_w", [H, 128, D], BF16, isOutput=False)
    gw_d = nc.declare_dram_parameter("gate_w", [FT_FF, 128, D], BF16, isOutput=False)
    uw_d = nc.declare_dram_parameter("up_w", [FT_FF, 128, D], BF16, isOutput=False)
    dw_d = nc.declare_dram_parameter("down_w", [FT_FF, 128, D], BF16, isOutput=False)
    cs1_d = nc.declare_dram_parameter("cs1", [128, TLOC], F32, isOutput=False)
    cs2_d = nc.declare_dram_parameter("cs2", [128, TLOC], F32, isOutput=False)
    tri_d = nc.declare_dram_parameter("tri", [128, NKT * 128], BF16,
                                      isOutput=False)
    out_d = nc.declare_dram_parameter("out", [TLOC, D], F32, isOutput=True)

    with ExitStack() as ctx:
        tc = ctx.enter_context(tile.TileContext(nc))
        for _rep in range(repeat):
            _emit_block(nc, tc, sim, x_d, qkw_d, vw_d, pw_d, gw_d, uw_d, dw_d,
                        cs1_d, cs2_d, tri_d, out_d)

    nc.compile()
    return nc


def _emit_block(nc, tc, sim, x_d, qkw_d, vw_d, pw_d, gw_d, uw_d, dw_d,
                cs1_d, cs2_d, tri_d, out_d):
    with ExitStack() as ctx:
        const = ctx.enter_context(tc.tile_pool(name="const", bufs=1))
        ident = const.tile([128, 128], BF16)
        make_identity(nc, ident)
        ones_col = const.tile([128, 1], BF16)
        nc.vector.memset(ones_col, 1.0)
        ones_row = const.tile([1, 128], F32)
        nc.vector.memset(ones_row, 1.0)
        trib_sb = const.tile([128, NKT, 128], BF16)
        nc.gpsimd.dma_start(out=trib_sb[:], in_=tri_d.rearrange(
            "p (n q) -> p n q", n=NKT))

        # shared streaming pool for ALL weight tiles: one rotation across
        # phases lets the DMA queue prefetch phase N+1's weights during
        # phase N's compute (slot = 4KB/partition).
        wflow_cm = tc.tile_pool(name="wflow", bufs=3)
        wflow = wflow_cm.__enter__()

        # DRAM scratch: K/V allgather buffers
        dram = ctx.enter_context(tc.tile_pool(name="dram", bufs=1, space="DRAM"))
        k_local = dram.tile([H, 128, TLOC], BF16)
        k_full = dram.tile([CORES_PER_B, H, 128, TLOC], BF16)
        v_locals = [dram.tile([NT, 128, 512], BF16, name=f"v_loc{fb}")
                    for fb in range(NFB)]
        v_fulls = [dram.tile([CORES_PER_B, NT, 128, 512], BF16,
                             name=f"v_full{fb}") for fb in range(NFB)]

        def rmsnorm_transpose(loader, dst_sb, pool, psum_pool):
            """loader(it) -> [128, D] fp32 AP; writes dst_sb [128, KT, TLOC]
            bf16 = (rms-normalized rows) transposed. Gains folded in weights."""
            rstds = []
            for it in range(NT):
                sq_scr = pool.tile([128, D], BF16, name="sq_scr")
                ssq = pool.tile([128, 1], F32, name=f"ssq{it}", tag=f"ssq{it}",
                                bufs=1)
                nc.scalar.activation(sq_scr[:], loader(it), AF.Square,
                                     accum_out=ssq[:])
                mean = pool.tile([128, 1], F32, name="mean")
                nc.vector.tensor_scalar(mean[:], ssq[:], 1.0 / D, EPS,
                                        ALU.mult, ALU.add)
                rec = pool.tile([128, 1], F32, name="rec")
                nc.vector.reciprocal(rec[:], mean[:])
                rstd = pool.tile([128, 1], F32, name=f"rstd{it}",
                                 tag=f"rstd{it}", bufs=1)
                nc.scalar.activation(rstd[:], rec[:], AF.Sqrt)
                rstds.append(rstd)
            hrows = []
            for it in range(NT):
                hrow = pool.tile([128, D], BF16, name=f"hrow{it}",
                                 tag=f"hrow{it}", bufs=1)
                nc.vector.tensor_scalar(hrow[:], loader(it), rstds[it][:],
                                        None, ALU.mult)
                hrows.append(hrow)
            # k-outer so dst_sb[:, k, :] completes early for the consumers
            for k in range(KT):
                for it in range(NT):
                    tp = psum_pool.tile([128, 128], BF16, name="tp")
                    nc.tensor.transpose(tp[:], hrows[it][:, k * 128:(k + 1) * 128],
                                        ident[:])
                    nc.vector.tensor_copy(dst_sb[:, k, it * 128:(it + 1) * 128],
                                          tp[:])

        def rope_evict(ps, dst, pool):
            """ps: [128, TLOC] psum q/k head tile (permuted lanes) -> rotated"""
            sh = pool.tile([128, TLOC], F32, name="rp_sh")
            nc.vector.stream_shuffle(sh[:], ps[:], mask=SHUF_MASK)
            t1 = pool.tile([128, TLOC], F32, name="rp_t1")
            nc.vector.tensor_tensor(t1[:], ps[:], cs1_sb[:], ALU.mult)
            t2 = pool.tile([128, TLOC], F32, name="rp_t2")
            nc.vector.tensor_tensor(t2[:], sh[:], cs2_sb[:], ALU.mult)
            nc.vector.tensor_tensor(dst[:], t1[:], t2[:], ALU.add)

        # persistent pools, strict LIFO
        x1_cm = tc.tile_pool(name="x1pool", bufs=1)
        x1pool = x1_cm.__enter__()
        x1_sb = x1pool.tile([128, NT, D], F32)
        qrot_cm = tc.tile_pool(name="qrot_pool", bufs=1)
        qrot_pool = qrot_cm.__enter__()
        qrot_sb = qrot_pool.tile([128, H, TLOC], BF16)
        hT_cm = tc.tile_pool(name="hT_pool", bufs=1)
        hT_pool = hT_cm.__enter__()
        hT_sb = hT_pool.tile([128, KT, TLOC], BF16)
        cs_cm = tc.tile_pool(name="cs_pool", bufs=1)
        cs_pool = cs_cm.__enter__()
        cs1_sb = cs_pool.tile([128, TLOC], F32)
        nc.sync.dma_start(out=cs1_sb[:], in_=cs1_d[:, :])
        cs2_sb = cs_pool.tile([128, TLOC], F32)
        nc.sync.dma_start(out=cs2_sb[:], in_=cs2_d[:, :])

        # ---------------- phase 1: norm1 + h^T ----------------
        x_cm = tc.tile_pool(name="xpool", bufs=1)
        xpool = x_cm.__enter__()
        x_sb = xpool.tile([128, NT, D], F32)
        for it in range(NT):
            eng = nc.sync if it % 2 == 0 else nc.gpsimd
            eng.dma_start(out=x_sb[:, it, :],
                          in_=x_d[it * 128:(it + 1) * 128, :])
        # prefetch the first K-head weight tiles behind the x loads
        qk_wts = {}
        for h in range(2):
            wt = wflow.tile([128, KT, 128], BF16, name="qk_wt")
            nc.sync.dma_start(out=wt[:], in_=qkw_d[H + h].rearrange(
                "p (k c) -> p k c", k=KT))
            qk_wts[h] = wt
        with ExitStack() as ph:
            pool = ph.enter_context(tc.tile_pool(name="n1_pool", bufs=2))
            psum_pool = ph.enter_context(
                tc.tile_pool(name="n1_psum", bufs=4, space="PSUM"))
            rmsnorm_transpose(lambda it: x_sb[:, it, :], hT_sb, pool, psum_pool)
        x_cm.__exit__(None, None, None)

        # ---------------- phase 2a: K heads + allgather ----------------
        with ExitStack() as ph:
            spool = ph.enter_context(tc.tile_pool(name="qkv_s", bufs=3))
            pspool = ph.enter_context(
                tc.tile_pool(name="qkv_ps", bufs=3, space="PSUM"))
            for h in range(H):
                if h in qk_wts:
                    wt = qk_wts.pop(h)
                else:
                    wt = wflow.tile([128, KT, 128], BF16, name="qk_wt")
                    nc.sync.dma_start(out=wt[:], in_=qkw_d[H + h].rearrange(
                        "p (k c) -> p k c", k=KT))
                ps = pspool.tile([128, TLOC], F32, name="qk_ps")
                for k in range(KT):
                    nc.tensor.matmul(ps[:], wt[:, k, :], hT_sb[:, k, :],
                                     start=(k == 0), stop=(k == KT - 1))
                krot = spool.tile([128, TLOC], BF16, name="krot")
                rope_evict(ps, krot[:], spool)
                nc.sync.dma_start(out=k_local[h], in_=krot[:])
            if sim:
                for r in range(CORES_PER_B):
                    nc.gpsimd.dma_start(out=k_full[r], in_=k_local[:])
            else:
                nc.gpsimd.collective_compute(
                    "AllGather", ALU.bypass,
                    replica_groups=[[0, 1, 2, 3], [4, 5, 6, 7]],
                    ins=[k_local.opt()], outs=[k_full.opt()],
                )

            # ---------------- phase 2b: Q heads + rope ----------------
            for h in range(H):
                wt = wflow.tile([128, KT, 128], BF16, name="qk_wt")
                nc.sync.dma_start(out=wt[:], in_=qkw_d[h].rearrange(
                    "p (k c) -> p k c", k=KT))
                ps = pspool.tile([128, TLOC], F32, name="qk_ps")
                for k in range(KT):
                    nc.tensor.matmul(ps[:], wt[:, k, :], hT_sb[:, k, :],
                                     start=(k == 0), stop=(k == KT - 1))
                rope_evict(ps, qrot_sb[:, h, :], spool)

        cs_cm.__exit__(None, None, None)

        # stage the first heads' K columns early so attention isn't stuck
        # behind the V traffic on the gpsimd DMA queue
        kt_cm = tc.tile_pool(name="ktpool", bufs=2)
        ktpool = kt_cm.__enter__()
        kT_tiles = {}
        for h in range(2):
            kT_sb = ktpool.tile([128, T], BF16, name="kT_sb")
            for r in range(CORES_PER_B):
                nc.gpsimd.dma_start(
                    out=kT_sb[:, r * TLOC:(r + 1) * TLOC],
                    in_=k_full[r, h])
            kT_tiles[h] = kT_sb

        # V columns for attention rotate per 512-col block (2 resident:
        # heads 4fb..4fb+3 consume block fb while fb+1 streams in)
        vall_cm = tc.tile_pool(name="vall_pool", bufs=2)
        vap = vall_cm.__enter__()
        v_fbs = []

        # ---------------- phase 2c: V (per 512-col block + gather) -------
        with ExitStack() as ph:
            spool = ph.enter_context(tc.tile_pool(name="v_s", bufs=3))
            vpspool = ph.enter_context(
                tc.tile_pool(name="v_psp", bufs=1, space="PSUM"))
            for fb in range(NFB):
                vps = [vpspool.tile([128, 512], F32, name=f"v_ps{it}",
                                    tag=f"v_ps{it}") for it in range(NT)]
                for k in range(KT):
                    vwt = wflow.tile([128, 512], BF16, name="vwt")
                    nc.sync.dma_start(out=vwt[:],
                                      in_=vw_d[k][:, fb * 512:(fb + 1) * 512])
                    for it in range(NT):
                        nc.tensor.matmul(vps[it][:],
                                         hT_sb[:, k, it * 128:(it + 1) * 128],
                                         vwt[:], start=(k == 0), stop=(k == KT - 1))
                for it in range(NT):
                    vsb = spool.tile([128, 512], BF16, name="vsb")
                    nc.scalar.copy(vsb[:], vps[it][:])
                    nc.sync.dma_start(out=v_locals[fb][it], in_=vsb[:])
                if sim:
                    for r in range(CORES_PER_B):
                        nc.gpsimd.dma_start(out=v_fulls[fb][r],
                                            in_=v_locals[fb][:])
                else:
                    nc.gpsimd.collective_compute(
                        "AllGather", ALU.bypass,
                        replica_groups=[[0, 1, 2, 3], [4, 5, 6, 7]],
                        ins=[v_locals[fb].opt()], outs=[v_fulls[fb].opt()],
                    )
                v_fb = vap.tile([128, NKT, 512], BF16, name="v_fb")
                for r in range(CORES_PER_B):
                    nc.gpsimd.dma_start(
                        out=v_fb[:, r * NT:(r + 1) * NT, :],
                        in_=v_fulls[fb][r].rearrange("l p c -> p l c"))
                v_fbs.append(v_fb)

        y_cm = tc.tile_pool(name="y_pool", bufs=1)
        y_pool = y_cm.__enter__()
        y_sb = y_pool.tile([128, H, TLOC], BF16)

        # ---------------- phase 3: attention (snake-folded causal) -------
        # kt block l=kt//4 covers local query cols [l*128:512); the first
        # 128 cols get the data-driven causal mask, the rest are always
        # fully allowed by construction of the snake fold.
        with ExitStack() as ph:
            apool = ph.enter_context(tc.tile_pool(name="att_pool", bufs=2))
            epool = ph.enter_context(tc.tile_pool(name="exp_pool", bufs=4))
            aps = ph.enter_context(tc.tile_pool(name="att_ps", bufs=2, space="PSUM"))
            sps_pool = ph.enter_context(
                tc.tile_pool(name="sps_pool", bufs=3, space="PSUM"))
            bps_pool = ph.enter_context(
                tc.tile_pool(name="bps_pool", bufs=1, space="PSUM"))

            for h in range(H):
                if h in kT_tiles:
                    kT_sb = kT_tiles.pop(h)
                else:
                    kT_sb = ktpool.tile([128, T], BF16, name="kT_sb")
                    for r in range(CORES_PER_B):
                        nc.gpsimd.dma_start(
                            out=kT_sb[:, r * TLOC:(r + 1) * TLOC],
                            in_=k_full[r, h])
                yps = aps.tile([128, TLOC], F32, name="y_ps", tag="y_ps")
                sums = aps.tile([1, TLOC], F32, name="sums_ps", tag="sums_ps")
                for kt in range(NKT):
                    c0 = (kt // 4) * 128
                    w = TLOC - c0
                    sps = sps_pool.tile([128, TLOC], F32, name="s_ps",
                                        tag="s_ps")
                    kp = POS[kt]
                    nc.tensor.matmul(sps[:, :w], kT_sb[:, kp * 128:(kp + 1) * 128],
                                     qrot_sb[:, h, c0:TLOC], start=True,
                                     stop=True)
                    em = epool.tile([128, TLOC], BF16, name="em")
                    nc.scalar.activation(em[:, :w], sps[:, :w], AF.Exp,
                                         scale=SCALE)
                    nc.vector.tensor_tensor(em[:, 0:128], em[:, 0:128],
                                            trib_sb[:, kt, :], ALU.mult)
                    nc.tensor.matmul(yps[:, c0:TLOC],
                                     v_fbs[h // 4][:, kp,
                                                   (h % 4) * 128:
                                                   (h % 4 + 1) * 128],
                                     em[:, :w], start=(kt == 0),
                                     stop=(kt == NKT - 1),
                                     skip_group_check=True)
                    nc.tensor.matmul(sums[:, c0:TLOC], ones_col[:], em[:, :w],
                                     start=(kt == 0), stop=(kt == NKT - 1),
                                     skip_group_check=True)
                rec = apool.tile([1, TLOC], F32, name="rec_att")
                nc.vector.reciprocal(rec[:], sums[:])
                bps = bps_pool.tile([128, TLOC], F32, name="b_ps", tag="b_ps")
                nc.tensor.matmul(bps[:], ones_row[:], rec[:], start=True,
                                 stop=True)
                bsb = apool.tile([128, TLOC], F32, name="bsb")
                nc.vector.tensor_copy(bsb[:], bps[:])
                nc.vector.tensor_tensor(y_sb[:, h, :], yps[:], bsb[:], ALU.mult)
        # ---------------- phase 4: proj + residual -> x1 (SBUF) ----------
        with ExitStack() as ph:
            spool = ph.enter_context(tc.tile_pool(name="pj_s", bufs=3))
            pps = ph.enter_context(tc.tile_pool(name="pj_ps", bufs=2, space="PSUM"))
            for fb in range(NFB):
                pps_t = [pps.tile([128, 512], F32, name=f"p_ps{it}",
                                  tag=f"p_ps{it}") for it in range(NT)]
                for hd in range(H):
                    pwt = wflow.tile([128, 512], BF16, name="pwt")
                    nc.sync.dma_start(out=pwt[:],
                                      in_=pw_d[hd][:, fb * 512:(fb + 1) * 512])
                    for it in range(NT):
                        nc.tensor.matmul(pps_t[it][:],
                                         y_sb[:, hd, it * 128:(it + 1) * 128],
                                         pwt[:], start=(hd == 0),
                                         stop=(hd == H - 1))
                for it in range(NT):
                    xr = spool.tile([128, 512], F32, name="xr_p")
                    nc.sync.dma_start(
                        out=xr[:],
                        in_=x_d[it * 128:(it + 1) * 128,
                                fb * 512:(fb + 1) * 512])
                    nc.vector.tensor_tensor(
                        x1_sb[:, it, fb * 512:(fb + 1) * 512], pps_t[it][:],
                        xr[:], ALU.add)

        y_cm.__exit__(None, None, None)
        vall_cm.__exit__(None, None, None)
        kt_cm.__exit__(None, None, None)
        cs_noop = None  # cs pool already closed after phase 2b
        hT_cm.__exit__(None, None, None)
        qrot_cm.__exit__(None, None, None)

        # ---------------- phase 5: norm2 + h2^T ----------------
        h2T_cm = tc.tile_pool(name="h2T_pool", bufs=1)
        h2T_pool = h2T_cm.__enter__()
        h2T_sb = h2T_pool.tile([128, KT, TLOC], BF16)
        with ExitStack() as ph:
            pool = ph.enter_context(tc.tile_pool(name="n2_pool", bufs=2))
            psum_pool = ph.enter_context(
                tc.tile_pool(name="n2_psum", bufs=4, space="PSUM"))
            rmsnorm_transpose(lambda it: x1_sb[:, it, :], h2T_sb, pool, psum_pool)

        # ---------------- phase 6: gate/up ----------------
        gu_cm = tc.tile_pool(name="gu_pool", bufs=1)
        gu_pool = gu_cm.__enter__()
        gu_sb = gu_pool.tile([128, FT_FF, TLOC], BF16)
        with ExitStack() as ph:
            spool = ph.enter_context(tc.tile_pool(name="mlp_s", bufs=3))
            mps = ph.enter_context(tc.tile_pool(name="mlp_ps", bufs=4, space="PSUM"))
            for f in range(FT_FF):
                gwt = wflow.tile([128, KT, 128], BF16, name="gwt")
                nc.sync.dma_start(out=gwt[:], in_=gw_d[f].rearrange(
                    "p (k c) -> p k c", k=KT))
                gps = mps.tile([128, TLOC], F32, name="g_ps", tag="g_ps")
                for k in range(KT):
                    nc.tensor.matmul(gps[:], gwt[:, k, :], h2T_sb[:, k, :],
                                     start=(k == 0), stop=(k == KT - 1))
                gsil = spool.tile([128, TLOC], BF16, name="gsil")
                nc.scalar.activation(gsil[:], gps[:], AF.Silu)
                uwt = wflow.tile([128, KT, 128], BF16, name="uwt")
                nc.sync.dma_start(out=uwt[:], in_=uw_d[f].rearrange(
                    "p (k c) -> p k c", k=KT))
                ups = mps.tile([128, TLOC], F32, name="u_ps", tag="u_ps")
                for k in range(KT):
                    nc.tensor.matmul(ups[:], uwt[:, k, :], h2T_sb[:, k, :],
                                     start=(k == 0), stop=(k == KT - 1))
                nc.vector.tensor_tensor(gu_sb[:, f, :], ups[:], gsil[:],
                                        ALU.mult)

        # ---------------- phase 7: down + residual -> out ----------------
        with ExitStack() as ph:
            spool = ph.enter_context(tc.tile_pool(name="dn_s", bufs=6))
            dps = ph.enter_context(tc.tile_pool(name="dn_ps", bufs=1, space="PSUM"))
            for fbp in range(2):
                dps_t = [[dps.tile([128, 512], F32, name=f"d_ps{it}_{fbi}",
                                   tag=f"d_ps{it}_{fbi}") for fbi in range(2)]
                         for it in range(NT)]
                for k in range(FT_FF):
                    dwt = wflow.tile([128, 1024], BF16, name="dwt")
                    nc.sync.dma_start(
                        out=dwt[:],
                        in_=dw_d[k][:, fbp * 1024:(fbp + 1) * 1024])
                    for it in range(NT):
                        for fbi in range(2):
                            nc.tensor.matmul(
                                dps_t[it][fbi][:],
                                gu_sb[:, k, it * 128:(it + 1) * 128],
                                dwt[:, fbi * 512:(fbi + 1) * 512],
                                start=(k == 0), stop=(k == FT_FF - 1))
                for it in range(NT):
                    for fbi in range(2):
                        fb = fbp * 2 + fbi
                        osb = spool.tile([128, 512], F32, name="osb_d")
                        nc.vector.tensor_tensor(
                            osb[:], dps_t[it][fbi][:],
                            x1_sb[:, it, fb * 512:(fb + 1) * 512], ALU.add)
                        nc.sync.dma_start(
                            out=out_d[it * 128:(it + 1) * 128,
                                      fb * 512:(fb + 1) * 512],
                            in_=osb[:])

        gu_cm.__exit__(None, None, None)
        h2T_cm.__exit__(None, None, None)
        x1_cm.__exit__(None, None, None)
        wflow_cm.__exit__(None, None, None)


def core_token_idx(c):
    """Global token indices (within the batch row) owned by group-core c."""
    return np.concatenate([np.arange(g * 128, (g + 1) * 128)
                           for g in snake_tiles(c)])


def prepare_inputs(x, f_cos, f_sin, w_attn, w_proj, w_gate, w_up, w_down, g1, g2):
    """Host-side sharding + weight re-layout. Returns list of 8 input dicts."""
    x = np.asarray(x, dtype=np.float32)
    f_cos = np.asarray(f_cos, dtype=np.float32)
    f_sin = np.asarray(f_sin, dtype=np.float32)
    w_attn = np.asarray(w_attn, dtype=np.float32)
    g1 = np.asarray(g1, dtype=np.float32)
    g2 = np.asarray(g2, dtype=np.float32)

    perm = _rope_perm()
    wq = w_attn[0:D] * g1[None, :]
    wk = w_attn[D:2 * D] * g1[None, :]
    wv = w_attn[2 * D:3 * D] * g1[None, :]
    # permute rows within each head for q and k
    wq_p = wq.reshape(H, DK, D)[:, perm, :].reshape(H * DK, D)
    wk_p = wk.reshape(H, DK, D)[:, perm, :].reshape(H * DK, D)

    def lhsT_layout(w):  # w: [F, D] -> [F/128, 128(d within k-tile), D(k*128+c)]
        f = w.shape[0]
        # out[ft, p, k*128+c] = w[ft*128+c, k*128+p]
        a = w.reshape(f // 128, 128, KT, 128)       # [ft, c, k, p]
        a = a.transpose(0, 3, 2, 1).reshape(f // 128, 128, D)  # [ft, p, (k c)]
        return np.ascontiguousarray(a).astype(ml_dtypes.bfloat16)

    def rhsT_layout(w):  # w: [F, D_in] -> [D_in/128, 128(p), F] = w.T tiled
        d_in = w.shape[1]
        a = w.T.reshape(d_in // 128, 128, w.shape[0])  # [k, p, c]
        return np.ascontiguousarray(a).astype(ml_dtypes.bfloat16)

    qk_w = np.concatenate([lhsT_layout(wq_p), lhsT_layout(wk_p)], axis=0)
    v_w = rhsT_layout(wv)
    proj_w = rhsT_layout(np.asarray(w_proj, dtype=np.float32))
    gate_w = lhsT_layout(np.asarray(w_gate, dtype=np.float32) * g2[None, :])
    up_w = lhsT_layout(np.asarray(w_up, dtype=np.float32) * g2[None, :])
    down_w = rhsT_layout(np.asarray(w_down, dtype=np.float32))

    # cs1/cs2 in permuted-lane layout: [128, T]
    pair = np.zeros(DK, dtype=np.int64)
    sign = np.zeros(DK, dtype=np.float32)
    for p in range(DK):
        qd, j = p // 32, p % 32
        pair[p] = 16 * qd + (j if j < 16 else j - 16)
        sign[p] = -1.0 if j < 16 else 1.0
    cs1_full = f_cos.T[pair, :]                       # [128, T]
    cs2_full = f_sin.T[pair, :] * sign[:, None]       # [128, T]

    in_maps = []
    for core in range(N_CORES):
        b, c = core // CORES_PER_B, core % CORES_PER_B
        tok = core_token_idx(c)
        tiles = snake_tiles(c)
        # causal mask tiles: kt covers query tile l=kt//4 (this core's
        # global tile tiles[l]); allowed iff key_pos <= query_pos
        tri = np.zeros((NKT, 128, 128), dtype=np.float32)
        kk = np.arange(128)[:, None]
        qq = np.arange(128)[None, :]
        for kt in range(NKT):
            g = tiles[kt // 4]
            tri[kt] = (kt * 128 + kk) <= (g * 128 + qq)
        tri = np.ascontiguousarray(
            tri.transpose(1, 0, 2).reshape(128, NKT * 128))
        in_maps.append({
            "x": np.ascontiguousarray(x[b, tok, :]),
            "qk_w": qk_w, "v_w": v_w, "proj_w": proj_w,
            "gate_w": gate_w, "up_w": up_w, "down_w": down_w,
            "cs1": np.ascontiguousarray(cs1_full[:, tok]),
            "cs2": np.ascontiguousarray(cs2_full[:, tok]),
            "tri": tri.astype(ml_dtypes.bfloat16),
        })
    return in_maps


def assemble_output(results):
    out = np.zeros((B, T, D), dtype=np.float32)
    for core in range(N_CORES):
        b, c = core // CORES_PER_B, core % CORES_PER_B
        out[b, core_token_idx(c), :] = results[core]["out"]
    return out


_CACHE = {}
_LOCK = threading.Lock()


def get_program():
    with _LOCK:
        if "nc" not in _CACHE:
            _CACHE["nc"] = build_program()
        return _CACHE["nc"]


def kernel(**inputs):
    nc = get_program()
    in_maps = prepare_inputs(**inputs)
    res = run_bass_kernel_spmd(nc, in_maps, list(range(N_CORES)))
    return assemble_output(res.results)


def bench(inputs, iters=10):
    """Wall-clock the sharded executable with device-resident inputs.

    Returns the mean pipelined per-call time in ns (upper bound on HW exec
    time: it includes 1/iters of the axon dispatch round-trip)."""
    import jax
    from jax.sharding import Mesh, PartitionSpec, NamedSharding
    from jax.experimental.shard_map import shard_map
    from concourse import bass2jax, mybir as mb

    nc = get_program()
    in_maps = prepare_inputs(**inputs)
    bass2jax.install_neuronx_cc_hook()

    partition_name = (nc.partition_id_tensor.name
                      if nc.partition_id_tensor else None)
    in_names, out_names, out_avals, zero_outs = [], [], [], []
    for alloc in nc.m.functions[0].allocations:
        if not isinstance(alloc, mb.MemoryLocationSet):
            continue
        name = alloc.memorylocations[0].name
        if alloc.kind == "ExternalInput":
            if name != partition_name:
                in_names.append(name)
        elif alloc.kind == "ExternalOutput":
            shape = tuple(alloc.tensor_shape)
            dtype = mb.dt.np(alloc.dtype)
            out_names.append(name)
            out_avals.append(jax.core.ShapedArray(shape, dtype))
            zero_outs.append(np.zeros(shape, dtype))
    n_params = len(in_names)
    all_in_names = list(in_names) + list(out_names)
    if partition_name is not None:
        all_in_names.append(partition_name)
    donate = tuple(range(n_params, n_params + len(out_names)))

    def _body(*args):
        operands = list(args)
        if partition_name is not None:
            operands.append(bass2jax.partition_id_tensor())
        return tuple(bass2jax._bass_exec_p.bind(
            *operands,
            out_avals=tuple(out_avals),
            in_names=tuple(all_in_names),
            out_names=tuple(out_names),
            lowering_input_output_aliases=(),
            sim_require_finite=True,
            sim_require_nnan=True,
            nc=nc,
        ))

    devices = jax.devices()[:N_CORES]
    mesh = Mesh(np.asarray(devices), ("core",))
    in_specs = (PartitionSpec("core"),) * (n_params + len(out_names))
    out_specs = (PartitionSpec("core"),) * len(out_names)
    sharded = jax.jit(
        shard_map(_body, mesh=mesh, in_specs=in_specs, out_specs=out_specs,
                  check_rep=False),
        donate_argnums=donate, keep_unused=True)

    sh = NamedSharding(mesh, PartitionSpec("core"))
    concat_in = [
        jax.device_put(
            np.concatenate([np.asarray(in_maps[c][nm]) for c in range(N_CORES)],
                           axis=0), sh)
        for nm in in_names]
    jax.block_until_ready(concat_in)

    def make_zeros():
        return [jax.device_put(
            np.zeros((N_CORES * z.shape[0], *z.shape[1:]), z.dtype), sh)
            for z in zero_outs]

    # warmup (compile)
    outs = sharded(*concat_in, *make_zeros())
    jax.block_until_ready(outs)

    zs = [make_zeros() for _ in range(iters)]
    for z in zs:
        jax.block_until_ready(z)
    # async pipelined dispatch amortizes the ~100ms axon round-trip
    t0 = time.perf_counter()
    outs = [sharded(*concat_in, *zs[i]) for i in range(iters)]
    jax.block_until_ready(outs)
    dt = (time.perf_counter() - t0) / iters
    return dt * 1e9


# revision 15
# speedup vs baseline: 1.6742x; 1.1855x over previous
"""Trainium2 Bass kernel for a dense transformer block.

Reference computation (B=2, T=2048, D=2048, H=16, Dk=128, FF=8192, fp32):
    h   = rmsnorm(x, g1)
    qkv = h @ w_attn.T ; q,k = rope(q,k) ; y = causal_softmax(q k^T / sqrt(Dk)) v
    x1  = x + y @ w_proj.T
    h2  = rmsnorm(x1, g2)
    out = x1 + (silu(h2 @ w_gate.T) * (h2 @ w_up.T)) @ w_down.T

Distribution: data-parallel over tokens, 512 per core (cores 0-3: batch 0,
cores 4-7: batch 1). Token tiles are "snake"-folded across the 4-core group:
core c owns global 128-token tiles {c, 7-c, 8+c, 15-c}, so every core's
causal key footprint is identical (tiles 0..3 attend 4 key tiles, 4..7
attend 8, 8..11 attend 12, 12..15 attend 16 -> 62.5% of the dense score/AV
work, perfectly balanced). Causal masking within the padded footprint is
data-driven (per-core 0/1 mask tiles multiply the exp'd scores), which keeps
the SPMD program identical on all cores. K,V are computed locally and
AllGather'd inside each 4-core group (V gathers split per 512-col block so
attention can start while late blocks are still in flight).

All weight matrices stream through one shared SBUF pool, so the DMA queue
naturally prefetches the next phase's weights while the current phase
computes. Matmuls run in bf16 with fp32 PSUM accumulation. Residuals and
normalization in fp32. RoPE is applied in the transposed [dk, t] layout via
a host-side permutation of the head dimension + DVE stream_shuffle.
"""

import os
import sys
import threading
import time

import numpy as np

for _p in ("/opt/trn_rl_repo", os.path.expanduser("~/.axon_site/_ro/trn_rl_repo")):
    if _p not in sys.path and os.path.isdir(_p):
        sys.path.append(_p)

import ml_dtypes  # noqa: E402

import concourse.bass as bass  # noqa: E402
import concourse.mybir as mybir  # noqa: E402
import concourse.tile as tile  # noqa: E402
from concourse import bacc  # noqa: E402
from concourse.bass_utils import run_bass_kernel_spmd  # noqa: E402
from concourse.masks import make_identity  # noqa: E402
from contextlib import ExitStack  # noqa: E402

F32 = mybir.dt.float32
BF16 = mybir.dt.bfloat16
AF = mybir.ActivationFunctionType
ALU = mybir.AluOpType

B, T, D = 2, 2048, 2048
H, DK, FF = 16, 128, 8192
EPS = 1e-6
N_CORES = 8
TLOC = T * B // N_CORES          # 512 tokens per core
CORES_PER_B = N_CORES // B       # 4
KT = D // 128                    # 16 d-tiles
NT = TLOC // 128                 # 4 t-tiles per core
NKT = T // 128                   # 16 key subtiles (full sequence)
FT_FF = FF // 128                # 64 ff tiles
NFB = D // 512                   # 4 v/proj 512-col blocks
SCALE = 1.0 / float(np.sqrt(DK))
SHUF_MASK = [(j + 16) % 32 for j in range(32)]


def snake_tiles(c):
    """Global 128-token tile indices owned by group-core c, local order."""
    return [c, 7 - c, 8 + c, 15 - c]


def _gmaps():
    """global tile g -> (owning group-core, local tile index)."""
    rmap, lmap = [0] * NKT, [0] * NKT
    for g in range(NKT):
        for r in range(CORES_PER_B):
            if g in snake_tiles(r):
                rmap[g], lmap[g] = r, snake_tiles(r).index(g)
    return rmap, lmap


RMAP, LMAP = _gmaps()
# core-major position of global tile g inside gathered K/V SBUF tiles
POS = [RMAP[g] * NT + LMAP[g] for g in range(NKT)]


def _rope_perm():
    """Within-head row permutation: pair i=(16*qd + j) real part -> partition
    32*qd + j, imag part -> partition 32*qd + 16 + j."""
    perm = np.zeros(DK, dtype=np.int64)
    for p in range(DK):
        qd, j = p // 32, p % 32
        i = 16 * qd + (j if j < 16 else j - 16)
        perm[p] = 2 * i + (0 if j < 16 else 1)
    return perm


def build_program(sim=False, repeat=1):
    nc = bacc.Bacc("TRN2", target_bir_lowering=False, debug=False,
                   num_devices=1 if sim else N_CORES)

    x_d = nc.declare_dram_parameter("x", [TLOC, D], F32, isOutput=False)
    qkw_d = nc.declare_dram_parameter("qk_w", [2 * H, 128, D], BF16, isOutput=False)
    vw_d = nc.declare_dram_parameter("v_w", [KT, 128, D], BF16, isOutput=False)
    pw_d = nc.declare_dram_parameter("proj_w", [H, 128, D], BF16, isOutput=False)
    gw_d = nc.declare_dram_parameter("gate_w", [FT_FF, 128, D], BF16, isOutput=False)
    uw_d = nc.declare_dram_parameter("up_w", [FT_FF, 128, D], BF16, isOutput=False)
    dw_d = nc.declare_dram_parameter("down_w", [FT_FF, 128, D], BF16, isOutput=False)
    cs1_d = nc.declare_dram_parameter("cs1", [128, TLOC], F32, isOutput=False)
    cs2_d = nc.declare_dram_parameter("cs2", [128, TLOC], F32, isOutput=False)
    tri_d = nc.declare_dram_parameter("tri", [128, NKT * 128], BF16,
                                      isOutput=False)
    out_d = nc.declare_dram_parameter("out", [TLOC, D], F32, isOutput=True)

    with ExitStack() as ctx:
        tc = ctx.enter_context(tile.TileContext(nc))
        for _rep in range(repeat):
            _emit_block(nc, tc, sim, x_d, qkw_d, vw_d, pw_d, gw_d, uw_d, dw_d,
                        cs1_d, cs2_d, tri_d, out_d)

    nc.compile()
    return nc


def _emit_block(nc, tc, sim, x_d, qkw_d, vw_d, pw_d, gw_d, uw_d, dw_d,
                cs1_d, cs2_d, tri_d, out_d):
    with ExitStack() as ctx:
        const = ctx.enter_context(tc.tile_pool(name="const", bufs=1))
        ident = const.tile([128, 128], BF16)
        make_identity(nc, ident)
        ones_col = const.tile([128, 1], BF16)
        nc.vector.memset(ones_col, 1.0)
        ones_row = const.tile([1, 128], F32)
        nc.vector.memset(ones_row, 1.0)
        cs1_sb = const.tile([128, TLOC], F32)
        nc.sync.dma_start(out=cs1_sb[:], in_=cs1_d[:, :])
        cs2_sb = const.tile([128, TLOC], F32)
        nc.sync.dma_start(out=cs2_sb[:], in_=cs2_d[:, :])
        trib_sb = const.tile([128, NKT, 128], BF16)
        nc.gpsimd.dma_start(out=trib_sb[:], in_=tri_d.rearrange(
            "p (n q) -> p n q", n=NKT))

        # shared streaming pool for ALL weight tiles: one rotation across
        # phases lets the DMA queue prefetch phase N+1's weights during
        # phase N's compute (slot = 4KB/partition).
        wflow_cm = tc.tile_pool(name="wflow", bufs=4)
        wflow = wflow_cm.__enter__()

        # DRAM scratch: K/V allgather buffers + x1 spill
        dram = ctx.enter_context(tc.tile_pool(name="dram", bufs=1, space="DRAM"))
        k_local = dram.tile([H, 128, TLOC], BF16)
        k_full = dram.tile([CORES_PER_B, H, 128, TLOC], BF16)
        v_local = dram.tile([NFB, NT, 128, 512], BF16)
        v_full = dram.tile([CORES_PER_B, NFB, NT, 128, 512], BF16)
        x1_d = dram.tile([NT, 128, D], F32)

        def rmsnorm_transpose(loader, dst_sb, pool, psum_pool):
            """loader(it) -> [128, D] fp32 AP; writes dst_sb [128, KT, TLOC]
            bf16 = (rms-normalized rows) transposed. Gains folded in weights."""
            rstds = []
            for it in range(NT):
                sq_scr = pool.tile([128, D], BF16, name="sq_scr")
                ssq = pool.tile([128, 1], F32, name=f"ssq{it}", tag=f"ssq{it}",
                                bufs=1)
                nc.scalar.activation(sq_scr[:], loader(it), AF.Square,
                                     accum_out=ssq[:])
                mean = pool.tile([128, 1], F32, name="mean")
                nc.vector.tensor_scalar(mean[:], ssq[:], 1.0 / D, EPS,
                                        ALU.mult, ALU.add)
                rec = pool.tile([128, 1], F32, name="rec")
                nc.vector.reciprocal(rec[:], mean[:])
                rstd = pool.tile([128, 1], F32, name=f"rstd{it}",
                                 tag=f"rstd{it}", bufs=1)
                nc.scalar.activation(rstd[:], rec[:], AF.Sqrt)
                rstds.append(rstd)
            hrows = []
            for it in range(NT):
                hrow = pool.tile([128, D], BF16, name=f"hrow{it}",
                                 tag=f"hrow{it}", bufs=1)
                nc.vector.tensor_scalar(hrow[:], loader(it), rstds[it][:],
                                        None, ALU.mult)
                hrows.append(hrow)
            # k-outer so dst_sb[:, k, :] completes early for the consumers
            for k in range(KT):
                for it in range(NT):
                    tp = psum_pool.tile([128, 128], BF16, name="tp")
                    nc.tensor.transpose(tp[:], hrows[it][:, k * 128:(k + 1) * 128],
                                        ident[:])
                    nc.vector.tensor_copy(dst_sb[:, k, it * 128:(it + 1) * 128],
                                          tp[:])

        def rope_evict(ps, dst, pool):
            """ps: [128, TLOC] psum q/k head tile (permuted lanes) -> rotated"""
            sh = pool.tile([128, TLOC], F32, name="rp_sh")
            nc.vector.stream_shuffle(sh[:], ps[:], mask=SHUF_MASK)
            t1 = pool.tile([128, TLOC], F32, name="rp_t1")
            nc.vector.tensor_tensor(t1[:], ps[:], cs1_sb[:], ALU.mult)
            t2 = pool.tile([128, TLOC], F32, name="rp_t2")
            nc.vector.tensor_tensor(t2[:], sh[:], cs2_sb[:], ALU.mult)
            nc.vector.tensor_tensor(dst[:], t1[:], t2[:], ALU.add)

        # persistent pools, strict LIFO
        qrot_cm = tc.tile_pool(name="qrot_pool", bufs=1)
        qrot_pool = qrot_cm.__enter__()
        qrot_sb = qrot_pool.tile([128, H, TLOC], BF16)
        hT_cm = tc.tile_pool(name="hT_pool", bufs=1)
        hT_pool = hT_cm.__enter__()
        hT_sb = hT_pool.tile([128, KT, TLOC], BF16)

        # ---------------- phase 1: norm1 + h^T ----------------
        x_cm = tc.tile_pool(name="xpool", bufs=1)
        xpool = x_cm.__enter__()
        x_sb = xpool.tile([128, NT, D], F32)
        for it in range(NT):
            eng = nc.sync if it % 2 == 0 else nc.gpsimd
            eng.dma_start(out=x_sb[:, it, :],
                          in_=x_d[it * 128:(it + 1) * 128, :])
        # prefetch the first K-head weight tiles behind the x loads
        qk_wts = {}
        for h in range(2):
            wt = wflow.tile([128, KT, 128], BF16, name="qk_wt")
            nc.sync.dma_start(out=wt[:], in_=qkw_d[H + h].rearrange(
                "p (k c) -> p k c", k=KT))
            qk_wts[h] = wt
        with ExitStack() as ph:
            pool = ph.enter_context(tc.tile_pool(name="n1_pool", bufs=2))
            psum_pool = ph.enter_context(
                tc.tile_pool(name="n1_psum", bufs=4, space="PSUM"))
            rmsnorm_transpose(lambda it: x_sb[:, it, :], hT_sb, pool, psum_pool)
        x_cm.__exit__(None, None, None)

        # ---------------- phase 2a: K heads + allgather ----------------
        with ExitStack() as ph:
            spool = ph.enter_context(tc.tile_pool(name="k_s", bufs=3))
            pspool = ph.enter_context(
                tc.tile_pool(name="k_ps", bufs=3, space="PSUM"))
            for h in range(H):
                if h in qk_wts:
                    wt = qk_wts.pop(h)
                else:
                    wt = wflow.tile([128, KT, 128], BF16, name="qk_wt")
                    nc.sync.dma_start(out=wt[:], in_=qkw_d[H + h].rearrange(
                        "p (k c) -> p k c", k=KT))
                ps = pspool.tile([128, TLOC], F32, name="qk_ps")
                for k in range(KT):
                    nc.tensor.matmul(ps[:], wt[:, k, :], hT_sb[:, k, :],
                                     start=(k == 0), stop=(k == KT - 1))
                krot = spool.tile([128, TLOC], BF16, name="krot")
                rope_evict(ps, krot[:], spool)
                nc.sync.dma_start(out=k_local[h], in_=krot[:])
            if sim:
                for r in range(CORES_PER_B):
                    nc.gpsimd.dma_start(out=k_full[r], in_=k_local[:])
            else:
                nc.gpsimd.collective_compute(
                    "AllGather", ALU.bypass,
                    replica_groups=[[0, 1, 2, 3], [4, 5, 6, 7]],
                    ins=[k_local.opt()], outs=[k_full.opt()],
                )

        # stage the first heads' K columns early (gpsimd queue is idle here)
        kt_cm = tc.tile_pool(name="ktpool", bufs=2)
        ktpool = kt_cm.__enter__()
        kT_tiles = {}
        for h in range(2):
            kT_sb = ktpool.tile([128, T], BF16, name="kT_sb")
            for r in range(CORES_PER_B):
                nc.gpsimd.dma_start(
                    out=kT_sb[:, r * TLOC:(r + 1) * TLOC],
                    in_=k_full[r, h])
            kT_tiles[h] = kT_sb

        # V columns for attention rotate per 512-col block (2 resident:
        # heads 4fb..4fb+3 consume block fb while fb+1 streams in)
        vall_cm = tc.tile_pool(name="vall_pool", bufs=2)
        vap = vall_cm.__enter__()
        v_fbs = []

        # ---------------- phase 2b: V + allgather ----------------
        with ExitStack() as ph:
            spool = ph.enter_context(tc.tile_pool(name="v_s", bufs=3))
            vpspool = ph.enter_context(
                tc.tile_pool(name="v_psp", bufs=1, space="PSUM"))
            for fb in range(NFB):
                vps = [vpspool.tile([128, 512], F32, name=f"v_ps{it}",
                                    tag=f"v_ps{it}") for it in range(NT)]
                for k in range(KT):
                    vwt = wflow.tile([128, 512], BF16, name="vwt")
                    nc.sync.dma_start(out=vwt[:],
                                      in_=vw_d[k][:, fb * 512:(fb + 1) * 512])
                    for it in range(NT):
                        nc.tensor.matmul(vps[it][:],
                                         hT_sb[:, k, it * 128:(it + 1) * 128],
                                         vwt[:], start=(k == 0), stop=(k == KT - 1))
                for it in range(NT):
                    vsb = spool.tile([128, 512], BF16, name="vsb")
                    nc.scalar.copy(vsb[:], vps[it][:])
                    nc.sync.dma_start(out=v_local[fb, it], in_=vsb[:])
            if sim:
                for r in range(CORES_PER_B):
                    nc.gpsimd.dma_start(out=v_full[r], in_=v_local[:])
            else:
                nc.gpsimd.collective_compute(
                    "AllGather", ALU.bypass,
                    replica_groups=[[0, 1, 2, 3], [4, 5, 6, 7]],
                    ins=[v_local.opt()], outs=[v_full.opt()],
                )
            for fb in range(NFB):
                v_fb = vap.tile([128, NKT, 512], BF16, name="v_fb")
                for r in range(CORES_PER_B):
                    nc.gpsimd.dma_start(
                        out=v_fb[:, r * NT:(r + 1) * NT, :],
                        in_=v_full[r, fb].rearrange("l p c -> p l c"))
                v_fbs.append(v_fb)

        # ---------------- phase 2c: Q heads + rope (V gather overlaps) ---
        with ExitStack() as ph:
            spool = ph.enter_context(tc.tile_pool(name="q_s", bufs=3))
            pspool = ph.enter_context(
                tc.tile_pool(name="q_ps", bufs=3, space="PSUM"))
            for h in range(H):
                wt = wflow.tile([128, KT, 128], BF16, name="qk_wt")
                nc.sync.dma_start(out=wt[:], in_=qkw_d[h].rearrange(
                    "p (k c) -> p k c", k=KT))
                ps = pspool.tile([128, TLOC], F32, name="qk_ps")
                for k in range(KT):
                    nc.tensor.matmul(ps[:], wt[:, k, :], hT_sb[:, k, :],
                                     start=(k == 0), stop=(k == KT - 1))
                rope_evict(ps, qrot_sb[:, h, :], spool)

        y_cm = tc.tile_pool(name="y_pool", bufs=1)
        y_pool = y_cm.__enter__()
        y_sb = y_pool.tile([128, H, TLOC], BF16)

        # ---------------- phase 3: attention (snake-folded causal) -------
        # kt block l=kt//4 covers local query cols [l*128:512); the first
        # 128 cols get the data-driven causal mask, the rest are always
        # fully allowed by construction of the snake fold.
        with ExitStack() as ph:
            apool = ph.enter_context(tc.tile_pool(name="att_pool", bufs=2))
            epool = ph.enter_context(tc.tile_pool(name="exp_pool", bufs=4))
            aps = ph.enter_context(tc.tile_pool(name="att_ps", bufs=2, space="PSUM"))
            sps_pool = ph.enter_context(
                tc.tile_pool(name="sps_pool", bufs=3, space="PSUM"))
            bps_pool = ph.enter_context(
                tc.tile_pool(name="bps_pool", bufs=1, space="PSUM"))

            for h in range(H):
                if h in kT_tiles:
                    kT_sb = kT_tiles.pop(h)
                else:
                    kT_sb = ktpool.tile([128, T], BF16, name="kT_sb")
                    for r in range(CORES_PER_B):
                        nc.gpsimd.dma_start(
                            out=kT_sb[:, r * TLOC:(r + 1) * TLOC],
                            in_=k_full[r, h])
                yps = aps.tile([128, TLOC], F32, name="y_ps", tag="y_ps")
                sums = aps.tile([1, TLOC], F32, name="sums_ps", tag="sums_ps")
                for kt in range(NKT):
                    c0 = (kt // 4) * 128
                    w = TLOC - c0
                    kp = POS[kt]
                    sps = sps_pool.tile([128, TLOC], F32, name="s_ps",
                                        tag="s_ps")
                    nc.tensor.matmul(sps[:, :w], kT_sb[:, kp * 128:(kp + 1) * 128],
                                     qrot_sb[:, h, c0:TLOC], start=True,
                                     stop=True)
                    em = epool.tile([128, TLOC], BF16, name="em")
                    nc.scalar.activation(em[:, :w], sps[:, :w], AF.Exp,
                                         scale=SCALE)
                    nc.vector.tensor_tensor(em[:, 0:128], em[:, 0:128],
                                            trib_sb[:, kt, :], ALU.mult)
                    nc.tensor.matmul(yps[:, c0:TLOC],
                                     v_fbs[h // 4][:, kp,
                                                   (h % 4) * 128:
                                                   (h % 4 + 1) * 128],
                                     em[:, :w], start=(kt == 0),
                                     stop=(kt == NKT - 1),
                                     skip_group_check=True)
                    nc.tensor.matmul(sums[:, c0:TLOC], ones_col[:], em[:, :w],
                                     start=(kt == 0), stop=(kt == NKT - 1),
                                     skip_group_check=True)
                rec = apool.tile([1, TLOC], F32, name="rec_att")
                nc.vector.reciprocal(rec[:], sums[:])
                bps = bps_pool.tile([128, TLOC], F32, name="b_ps", tag="b_ps")
                nc.tensor.matmul(bps[:], ones_row[:], rec[:], start=True,
                                 stop=True)
                bsb = apool.tile([128, TLOC], F32, name="bsb")
                nc.vector.tensor_copy(bsb[:], bps[:])
                nc.vector.tensor_tensor(y_sb[:, h, :], yps[:], bsb[:], ALU.mult)

        # ------- phase 4: proj + residual -> x1 (DRAM spill) -------------
        with ExitStack() as ph:
            spool = ph.enter_context(tc.tile_pool(name="pj_s", bufs=4))
            pps = ph.enter_context(tc.tile_pool(name="pj_ps", bufs=2, space="PSUM"))
            for fb in range(NFB):
                pps_t = [pps.tile([128, 512], F32, name=f"p_ps{it}",
                                  tag=f"p_ps{it}") for it in range(NT)]
                for hd in range(H):
                    pwt = wflow.tile([128, 512], BF16, name="pwt")
                    nc.sync.dma_start(out=pwt[:],
                                      in_=pw_d[hd][:, fb * 512:(fb + 1) * 512])
                    for it in range(NT):
                        nc.tensor.matmul(pps_t[it][:],
                                         y_sb[:, hd, it * 128:(it + 1) * 128],
                                         pwt[:], start=(hd == 0),
                                         stop=(hd == H - 1))
                for it in range(NT):
                    xr = spool.tile([128, 512], F32, name="xr_p")
                    nc.sync.dma_start(
                        out=xr[:],
                        in_=x_d[it * 128:(it + 1) * 128,
                                fb * 512:(fb + 1) * 512])
                    x1t = spool.tile([128, 512], F32, name="x1t")
                    nc.vector.tensor_tensor(x1t[:], pps_t[it][:], xr[:],
                                            ALU.add)
                    nc.sync.dma_start(
                        out=x1_d[it][:, fb * 512:(fb + 1) * 512], in_=x1t[:])

        y_cm.__exit__(None, None, None)
        vall_cm.__exit__(None, None, None)
        kt_cm.__exit__(None, None, None)
        hT_cm.__exit__(None, None, None)
        qrot_cm.__exit__(None, None, None)

        # ---------------- phase 5: norm2 + h2^T ----------------
        h2T_cm = tc.tile_pool(name="h2T_pool", bufs=1)
        h2T_pool = h2T_cm.__enter__()
        h2T_sb = h2T_pool.tile([128, KT, TLOC], BF16)
        with ExitStack() as ph:
            x1p = ph.enter_context(tc.tile_pool(name="x1r_pool", bufs=1))
            x1rows = []
            for it in range(NT):
                x1r = x1p.tile([128, D], F32, name=f"x1r{it}", tag=f"x1r{it}")
                eng = nc.sync if it % 2 == 0 else nc.gpsimd
                eng.dma_start(out=x1r[:], in_=x1_d[it])
                x1rows.append(x1r)
            pool = ph.enter_context(tc.tile_pool(name="n2_pool", bufs=2))
            psum_pool = ph.enter_context(
                tc.tile_pool(name="n2_psum", bufs=4, space="PSUM"))
            rmsnorm_transpose(lambda it: x1rows[it][:], h2T_sb, pool, psum_pool)

        # ---------------- phase 6: gate/up ----------------
        gu_cm = tc.tile_pool(name="gu_pool", bufs=1)
        gu_pool = gu_cm.__enter__()
        gu_sb = gu_pool.tile([128, FT_FF, TLOC], BF16)
        with ExitStack() as ph:
            spool = ph.enter_context(tc.tile_pool(name="mlp_s", bufs=3))
            mps = ph.enter_context(tc.tile_pool(name="mlp_ps", bufs=4, space="PSUM"))
            for f in range(FT_FF):
                gwt = wflow.tile([128, KT, 128], BF16, name="gwt")
                nc.sync.dma_start(out=gwt[:], in_=gw_d[f].rearrange(
                    "p (k c) -> p k c", k=KT))
                gps = mps.tile([128, TLOC], F32, name="g_ps", tag="g_ps")
                for k in range(KT):
                    nc.tensor.matmul(gps[:], gwt[:, k, :], h2T_sb[:, k, :],
                                     start=(k == 0), stop=(k == KT - 1))
                gsil = spool.tile([128, TLOC], BF16, name="gsil")
                nc.scalar.activation(gsil[:], gps[:], AF.Silu)
                uwt = wflow.tile([128, KT, 128], BF16, name="uwt")
                nc.sync.dma_start(out=uwt[:], in_=uw_d[f].rearrange(
                    "p (k c) -> p k c", k=KT))
                ups = mps.tile([128, TLOC], F32, name="u_ps", tag="u_ps")
                for k in range(KT):
                    nc.tensor.matmul(ups[:], uwt[:, k, :], h2T_sb[:, k, :],
                                     start=(k == 0), stop=(k == KT - 1))
                nc.vector.tensor_tensor(gu_sb[:, f, :], ups[:], gsil[:],
                                        ALU.mult)

        # ---------------- phase 7: down + residual -> out ----------------
        with ExitStack() as ph:
            spool = ph.enter_context(tc.tile_pool(name="dn_s", bufs=8))
            dps = ph.enter_context(tc.tile_pool(name="dn_ps", bufs=1, space="PSUM"))
            for fbp in range(2):
                dps_t = [[dps.tile([128, 512], F32, name=f"d_ps{it}_{fbi}",
                                   tag=f"d_ps{it}_{fbi}") for fbi in range(2)]
                         for it in range(NT)]
                for k in range(FT_FF):
                    dwt = wflow.tile([128, 1024], BF16, name="dwt")
                    nc.sync.dma_start(
                        out=dwt[:],
                        in_=dw_d[k][:, fbp * 1024:(fbp + 1) * 1024])
                    for it in range(NT):
                        for fbi in range(2):
                            nc.tensor.matmul(
                                dps_t[it][fbi][:],
                                gu_sb[:, k, it * 128:(it + 1) * 128],
                                dwt[:, fbi * 512:(fbi + 1) * 512],
                                start=(k == 0), stop=(k == FT_FF - 1))
                for it in range(NT):
                    for fbi in range(2):
                        fb = fbp * 2 + fbi
                        xr = spool.tile([128, 512], F32, name="xr_d")
                        nc.gpsimd.dma_start(
                            out=xr[:],
                            in_=x1_d[it][:, fb * 512:(fb + 1) * 512])
                        osb = spool.tile([128, 512], F32, name="osb_d")
                        nc.vector.tensor_tensor(
                            osb[:], dps_t[it][fbi][:], xr[:], ALU.add)
                        nc.sync.dma_start(
                            out=out_d[it * 128:(it + 1) * 128,
                                      fb * 512:(fb + 1) * 512],
                            in_=osb[:])

        gu_cm.__exit__(None, None, None)
        h2T_cm.__exit__(None, None, None)
        wflow_cm.__exit__(None, None, None)


def core_token_idx(c):
    """Global token indices (within the batch row) owned by group-core c."""
    return np.concatenate([np.arange(g * 128, (g + 1) * 128)
                           for g in snake_tiles(c)])


def prepare_inputs(x, f_cos, f_sin, w_attn, w_proj, w_gate, w_up, w_down, g1, g2):
    """Host-side sharding + weight re-layout. Returns list of 8 input dicts."""
    x = np.asarray(x, dtype=np.float32)
    f_cos = np.asarray(f_cos, dtype=np.float32)
    f_sin = np.asarray(f_sin, dtype=np.float32)
    w_attn = np.asarray(w_attn, dtype=np.float32)
    g1 = np.asarray(g1, dtype=np.float32)
    g2 = np.asarray(g2, dtype=np.float32)

    perm = _rope_perm()
    wq = w_attn[0:D] * g1[None, :]
    wk = w_attn[D:2 * D] * g1[None, :]
    wv = w_attn[2 * D:3 * D] * g1[None, :]
    # permute rows within each head for q and k
    wq_p = wq.reshape(H, DK, D)[:, perm, :].reshape(H * DK, D)
    wk_p = wk.reshape(H, DK, D)[:, perm, :].reshape(H * DK, D)

    def lhsT_layout(w):  # w: [F, D] -> [F/128, 128(d within k-tile), D(k*128+c)]
        f = w.shape[0]
        # out[ft, p, k*128+c] = w[ft*128+c, k*128+p]
        a = w.reshape(f // 128, 128, KT, 128)       # [ft, c, k, p]
        a = a.transpose(0, 3, 2, 1).reshape(f // 128, 128, D)  # [ft, p, (k c)]
        return np.ascontiguousarray(a).astype(ml_dtypes.bfloat16)

    def rhsT_layout(w):  # w: [F, D_in] -> [D_in/128, 128(p), F] = w.T tiled
        d_in = w.shape[1]
        a = w.T.reshape(d_in // 128, 128, w.shape[0])  # [k, p, c]
        return np.ascontiguousarray(a).astype(ml_dtypes.bfloat16)

    qk_w = np.concatenate([lhsT_layout(wq_p), lhsT_layout(wk_p)], axis=0)
    v_w = rhsT_layout(wv)
    proj_w = rhsT_layout(np.asarray(w_proj, dtype=np.float32))
    gate_w = lhsT_layout(np.asarray(w_gate, dtype=np.float32) * g2[None, :])
    up_w = lhsT_layout(np.asarray(w_up, dtype=np.float32) * g2[None, :])
    down_w = rhsT_layout(np.asarray(w_down, dtype=np.float32))

    # cs1/cs2 in permuted-lane layout: [128, T]
    pair = np.zeros(DK, dtype=np.int64)
    sign = np.zeros(DK, dtype=np.float32)
    for p in range(DK):
        qd, j = p // 32, p % 32
        pair[p] = 16 * qd + (j if j < 16 else j - 16)
        sign[p] = -1.0 if j < 16 else 1.0
    cs1_full = f_cos.T[pair, :]                       # [128, T]
    cs2_full = f_sin.T[pair, :] * sign[:, None]       # [128, T]

    in_maps = []
    for core in range(N_CORES):
        b, c = core // CORES_PER_B, core % CORES_PER_B
        tok = core_token_idx(c)
        tiles = snake_tiles(c)
        # causal mask tiles: kt covers query tile l=kt//4 (this core's
        # global tile tiles[l]); allowed iff key_pos <= query_pos
        tri = np.zeros((NKT, 128, 128), dtype=np.float32)
        kk = np.arange(128)[:, None]
        qq = np.arange(128)[None, :]
        for kt in range(NKT):
            g = tiles[kt // 4]
            tri[kt] = (kt * 128 + kk) <= (g * 128 + qq)
        tri = np.ascontiguousarray(
            tri.transpose(1, 0, 2).reshape(128, NKT * 128))
        in_maps.append({
            "x": np.ascontiguousarray(x[b, tok, :]),
            "qk_w": qk_w, "v_w": v_w, "proj_w": proj_w,
            "gate_w": gate_w, "up_w": up_w, "down_w": down_w,
            "cs1": np.ascontiguousarray(cs1_full[:, tok]),
            "cs2": np.ascontiguousarray(cs2_full[:, tok]),
            "tri": tri.astype(ml_dtypes.bfloat16),
        })
    return in_maps


def assemble_output(results):
    out = np.zeros((B, T, D), dtype=np.float32)
    for core in range(N_CORES):
        b, c = core // CORES_PER_B, core % CORES_PER_B
        out[b, core_token_idx(c), :] = results[core]["out"]
    return out


_CACHE = {}
_LOCK = threading.Lock()


def get_program():
    with _LOCK:
        if "nc" not in _CACHE:
            _CACHE["nc"] = build_program()
        return _CACHE["nc"]


def kernel(**inputs):
    nc = get_program()
    in_maps = prepare_inputs(**inputs)
    res = run_bass_kernel_spmd(nc, in_maps, list(range(N_CORES)))
    return assemble_output(res.results)


def bench(inputs, iters=10):
    """Wall-clock the sharded executable with device-resident inputs.

    Returns the mean pipelined per-call time in ns (upper bound on HW exec
    time: it includes 1/iters of the axon dispatch round-trip)."""
    import jax
    from jax.sharding import Mesh, PartitionSpec, NamedSharding
    from jax.experimental.shard_map import shard_map
    from concourse import bass2jax, mybir as mb

    nc = get_program()
    in_maps = prepare_inputs(**inputs)
    bass2jax.install_neuronx_cc_hook()

    partition_name = (nc.partition_id_tensor.name
                      if nc.partition_id_tensor else None)
    in_names, out_names, out_avals, zero_outs = [], [], [], []
    for alloc in nc.m.functions[0].allocations:
        if not isinstance(alloc, mb.MemoryLocationSet):
            continue
        name = alloc.memorylocations[0].name
        if alloc.kind == "ExternalInput":
            if name != partition_name:
                in_names.append(name)
        elif alloc.kind == "ExternalOutput":
            shape = tuple(alloc.tensor_shape)
            dtype = mb.dt.np(alloc.dtype)
            out_names.append(name)
            out_avals.append(jax.core.ShapedArray(shape, dtype))
            zero_outs.append(np.zeros(shape, dtype))
    n_params = len(in_names)
    all_in_names = list(in_names) + list(out_names)
    if partition_name is not None:
        all_in_names.append(partition_name)
    donate = tuple(range(n_params, n_params + len(out_names)))

    def _body(*args):
        operands = list(args)
        if partition_name is not None:
            operands.append(bass2jax.partition_id_tensor())
        return tuple(bass2jax._bass_exec_p.bind(
            *operands,
            out_avals=tuple(out_avals),
            in_names=tuple(all_in_names),
            out_names=tuple(out_names),
            lowering_input_output_aliases=(),
            sim_require_finite=True,
            sim_require_nnan=True,
            nc=nc,
        ))

    devices = jax.devices()[:N_CORES]
    mesh = Mesh(np.asarray(devices), ("core",))
    in_specs = (PartitionSpec("core"),) * (n_params + len(out_names))
    out_specs = (PartitionSpec("core"),) * len(out_names)
    sharded = jax.jit(
        shard_map(_body, mesh=mesh, in_specs=in_specs, out_specs=out_specs,
                  check_rep=False),
        donate_argnums=donate, keep_unused=True)

    sh = NamedSharding(mesh, PartitionSpec("core"))
    concat_in = [
        jax.device_put(
            np.concatenate([np.asarray(in_maps[c][nm]) for c in range(N_CORES)],
                           axis=0), sh)
        for nm in in_names]
    jax.block_until_ready(concat_in)

    def make_zeros():
        return [jax.device_put(
            np.zeros((N_CORES * z.shape[0], *z.shape[1:]), z.dtype), sh)
            for z in zero_outs]

    # warmup (compile)
    outs = sharded(*concat_in, *make_zeros())
    jax.block_until_ready(outs)

    zs = [make_zeros() for _ in range(iters)]
    for z in zs:
        jax.block_until_ready(z)
    # async pipelined dispatch amortizes the ~100ms axon round-trip
    t0 = time.perf_counter()
    outs = [sharded(*concat_in, *zs[i]) for i in range(iters)]
    jax.block_until_ready(outs)
    dt = (time.perf_counter() - t0) / iters
    return dt * 1e9


# revision 16
# speedup vs baseline: 1.8678x; 1.1157x over previous
"""Trainium2 Bass kernel for a dense transformer block.

Reference computation (B=2, T=2048, D=2048, H=16, Dk=128, FF=8192, fp32):
    h   = rmsnorm(x, g1)
    qkv = h @ w_attn.T ; q,k = rope(q,k) ; y = causal_softmax(q k^T / sqrt(Dk)) v
    x1  = x + y @ w_proj.T
    h2  = rmsnorm(x1, g2)
    out = x1 + (silu(h2 @ w_gate.T) * (h2 @ w_up.T)) @ w_down.T

Distribution: data-parallel over tokens, 512 per core (cores 0-3: batch 0,
cores 4-7: batch 1). Token tiles are "snake"-folded across the 4-core group:
core c owns global 128-token tiles {c, 7-c, 8+c, 15-c}, so every core's
causal key footprint is identical (tiles 0..3 attend 4 key tiles, 4..7
attend 8, 8..11 attend 12, 12..15 attend 16 -> 62.5% of the dense score/AV
work, perfectly balanced). Causal masking within the padded footprint is
data-driven (per-core 0/1 mask tiles multiply the exp'd scores), which keeps
the SPMD program identical on all cores. K,V are computed locally and
AllGather'd inside each 4-core group (V gathers split per 512-col block so
attention can start while late blocks are still in flight).

All weight matrices stream through one shared SBUF pool, so the DMA queue
naturally prefetches the next phase's weights while the current phase
computes. Matmuls run in bf16 with fp32 PSUM accumulation. Residuals and
normalization in fp32. RoPE is applied in the transposed [dk, t] layout via
a host-side permutation of the head dimension + DVE stream_shuffle.
"""

import os
import sys
import threading
import time

import numpy as np

for _p in ("/opt/trn_rl_repo", os.path.expanduser("~/.axon_site/_ro/trn_rl_repo")):
    if _p not in sys.path and os.path.isdir(_p):
        sys.path.append(_p)

import ml_dtypes  # noqa: E402

import concourse.bass as bass  # noqa: E402
import concourse.mybir as mybir  # noqa: E402
import concourse.tile as tile  # noqa: E402
from concourse import bacc  # noqa: E402
from concourse.bass_utils import run_bass_kernel_spmd  # noqa: E402
from concourse.masks import make_identity  # noqa: E402
from contextlib import ExitStack  # noqa: E402

F32 = mybir.dt.float32
BF16 = mybir.dt.bfloat16
AF = mybir.ActivationFunctionType
ALU = mybir.AluOpType

B, T, D = 2, 2048, 2048
H, DK, FF = 16, 128, 8192
EPS = 1e-6
N_CORES = 8
TLOC = T * B // N_CORES          # 512 tokens per core
CORES_PER_B = N_CORES // B       # 4
KT = D // 128                    # 16 d-tiles
NT = TLOC // 128                 # 4 t-tiles per core
NKT = T // 128                   # 16 key subtiles (full sequence)
FT_FF = FF // 128                # 64 ff tiles
NFB = D // 512                   # 4 v/proj 512-col blocks
SCALE = 1.0 / float(np.sqrt(DK))
SHUF_MASK = [(j + 16) % 32 for j in range(32)]


def snake_tiles(c):
    """Global 128-token tile indices owned by group-core c, local order."""
    return [c, 7 - c, 8 + c, 15 - c]


def _gmaps():
    """global tile g -> (owning group-core, local tile index)."""
    rmap, lmap = [0] * NKT, [0] * NKT
    for g in range(NKT):
        for r in range(CORES_PER_B):
            if g in snake_tiles(r):
                rmap[g], lmap[g] = r, snake_tiles(r).index(g)
    return rmap, lmap


RMAP, LMAP = _gmaps()
# core-major position of global tile g inside gathered K/V SBUF tiles
POS = [RMAP[g] * NT + LMAP[g] for g in range(NKT)]


def _rope_perm():
    """Within-head row permutation: pair i=(16*qd + j) real part -> partition
    32*qd + j, imag part -> partition 32*qd + 16 + j."""
    perm = np.zeros(DK, dtype=np.int64)
    for p in range(DK):
        qd, j = p // 32, p % 32
        i = 16 * qd + (j if j < 16 else j - 16)
        perm[p] = 2 * i + (0 if j < 16 else 1)
    return perm


def build_program(sim=False, repeat=1):
    nc = bacc.Bacc("TRN2", target_bir_lowering=False, debug=False,
                   num_devices=1 if sim else N_CORES)

    x_d = nc.declare_dram_parameter("x", [TLOC, D], F32, isOutput=False)
    qkw_d = nc.declare_dram_parameter("qk_w", [2 * H, 128, D], BF16, isOutput=False)
    vw_d = nc.declare_dram_parameter("v_w", [KT, 128, D], BF16, isOutput=False)
    pw_d = nc.declare_dram_parameter("proj_w", [H, 128, D], BF16, isOutput=False)
    gw_d = nc.declare_dram_parameter("gate_w", [FT_FF, 128, D], BF16, isOutput=False)
    uw_d = nc.declare_dram_parameter("up_w", [FT_FF, 128, D], BF16, isOutput=False)
    dw_d = nc.declare_dram_parameter("down_w", [FT_FF, 128, D], BF16, isOutput=False)
    cs1_d = nc.declare_dram_parameter("cs1", [128, TLOC], F32, isOutput=False)
    cs2_d = nc.declare_dram_parameter("cs2", [128, TLOC], F32, isOutput=False)
    tri_d = nc.declare_dram_parameter("tri", [128, NKT * 128], BF16,
                                      isOutput=False)
    out_d = nc.declare_dram_parameter("out", [TLOC, D], F32, isOutput=True)

    with ExitStack() as ctx:
        tc = ctx.enter_context(tile.TileContext(nc))
        for _rep in range(repeat):
            _emit_block(nc, tc, sim, x_d, qkw_d, vw_d, pw_d, gw_d, uw_d, dw_d,
                        cs1_d, cs2_d, tri_d, out_d)

    nc.compile()
    return nc


def _emit_block(nc, tc, sim, x_d, qkw_d, vw_d, pw_d, gw_d, uw_d, dw_d,
                cs1_d, cs2_d, tri_d, out_d):
    with ExitStack() as ctx:
        const = ctx.enter_context(tc.tile_pool(name="const", bufs=1))
        ident = const.tile([128, 128], BF16)
        make_identity(nc, ident)
        ones_col = const.tile([128, 1], BF16)
        nc.vector.memset(ones_col, 1.0)
        ones_row = const.tile([1, 128], F32)
        nc.vector.memset(ones_row, 1.0)
        cs1_sb = const.tile([128, TLOC], F32)
        nc.sync.dma_start(out=cs1_sb[:], in_=cs1_d[:, :])
        cs2_sb = const.tile([128, TLOC], F32)
        nc.sync.dma_start(out=cs2_sb[:], in_=cs2_d[:, :])
        trib_sb = const.tile([128, NKT, 128], BF16)
        nc.gpsimd.dma_start(out=trib_sb[:], in_=tri_d.rearrange(
            "p (n q) -> p n q", n=NKT))

        # shared streaming pool for ALL weight tiles: one rotation across
        # phases lets the DMA queue prefetch phase N+1's weights during
        # phase N's compute (slot = 4KB/partition).
        wflow_cm = tc.tile_pool(name="wflow", bufs=4)
        wflow = wflow_cm.__enter__()

        # DRAM scratch: K/V allgather buffers + x1 spill
        dram = ctx.enter_context(tc.tile_pool(name="dram", bufs=1, space="DRAM"))
        # merged K+V allgather payload: entries 0..H-1 = K heads,
        # H + fb*NT + l = V block (fb, local tile l). One collective per
        # call (collective launches carry ~1ms/call fixed runtime cost).
        kv_local = dram.tile([2 * H, 128, TLOC], BF16)
        kv_full = dram.tile([CORES_PER_B, 2 * H, 128, TLOC], BF16)
        x1_d = dram.tile([NT, 128, D], F32)

        def rmsnorm_transpose(loader, dst_sb, pool, psum_pool):
            """loader(it) -> [128, D] fp32 AP; writes dst_sb [128, KT, TLOC]
            bf16 = (rms-normalized rows) transposed. Gains folded in weights."""
            rstds = []
            for it in range(NT):
                sq_scr = pool.tile([128, D], BF16, name="sq_scr")
                ssq = pool.tile([128, 1], F32, name=f"ssq{it}", tag=f"ssq{it}",
                                bufs=1)
                nc.scalar.activation(sq_scr[:], loader(it), AF.Square,
                                     accum_out=ssq[:])
                mean = pool.tile([128, 1], F32, name="mean")
                nc.vector.tensor_scalar(mean[:], ssq[:], 1.0 / D, EPS,
                                        ALU.mult, ALU.add)
                rec = pool.tile([128, 1], F32, name="rec")
                nc.vector.reciprocal(rec[:], mean[:])
                rstd = pool.tile([128, 1], F32, name=f"rstd{it}",
                                 tag=f"rstd{it}", bufs=1)
                nc.scalar.activation(rstd[:], rec[:], AF.Sqrt)
                rstds.append(rstd)
            hrows = []
            for it in range(NT):
                hrow = pool.tile([128, D], BF16, name=f"hrow{it}",
                                 tag=f"hrow{it}", bufs=1)
                nc.vector.tensor_scalar(hrow[:], loader(it), rstds[it][:],
                                        None, ALU.mult)
                hrows.append(hrow)
            # k-outer so dst_sb[:, k, :] completes early for the consumers
            for k in range(KT):
                for it in range(NT):
                    tp = psum_pool.tile([128, 128], BF16, name="tp")
                    nc.tensor.transpose(tp[:], hrows[it][:, k * 128:(k + 1) * 128],
                                        ident[:])
                    nc.vector.tensor_copy(dst_sb[:, k, it * 128:(it + 1) * 128],
                                          tp[:])

        def rope_evict(ps, dst, pool):
            """ps: [128, TLOC] psum q/k head tile (permuted lanes) -> rotated"""
            sh = pool.tile([128, TLOC], F32, name="rp_sh")
            nc.vector.stream_shuffle(sh[:], ps[:], mask=SHUF_MASK)
            t1 = pool.tile([128, TLOC], F32, name="rp_t1")
            nc.vector.tensor_tensor(t1[:], ps[:], cs1_sb[:], ALU.mult)
            t2 = pool.tile([128, TLOC], F32, name="rp_t2")
            nc.vector.tensor_tensor(t2[:], sh[:], cs2_sb[:], ALU.mult)
            nc.vector.tensor_tensor(dst[:], t1[:], t2[:], ALU.add)

        # persistent pools, strict LIFO
        qrot_cm = tc.tile_pool(name="qrot_pool", bufs=1)
        qrot_pool = qrot_cm.__enter__()
        qrot_sb = qrot_pool.tile([128, H, TLOC], BF16)
        hT_cm = tc.tile_pool(name="hT_pool", bufs=1)
        hT_pool = hT_cm.__enter__()
        hT_sb = hT_pool.tile([128, KT, TLOC], BF16)

        # ---------------- phase 1: norm1 + h^T ----------------
        x_cm = tc.tile_pool(name="xpool", bufs=1)
        xpool = x_cm.__enter__()
        x_sb = xpool.tile([128, NT, D], F32)
        for it in range(NT):
            eng = nc.sync if it % 2 == 0 else nc.gpsimd
            eng.dma_start(out=x_sb[:, it, :],
                          in_=x_d[it * 128:(it + 1) * 128, :])
        # prefetch the first K-head weight tiles behind the x loads
        qk_wts = {}
        for h in range(2):
            wt = wflow.tile([128, KT, 128], BF16, name="qk_wt")
            nc.sync.dma_start(out=wt[:], in_=qkw_d[H + h].rearrange(
                "p (k c) -> p k c", k=KT))
            qk_wts[h] = wt
        with ExitStack() as ph:
            pool = ph.enter_context(tc.tile_pool(name="n1_pool", bufs=2))
            psum_pool = ph.enter_context(
                tc.tile_pool(name="n1_psum", bufs=4, space="PSUM"))
            rmsnorm_transpose(lambda it: x_sb[:, it, :], hT_sb, pool, psum_pool)
        x_cm.__exit__(None, None, None)

        # ---------------- phase 2a: K heads + allgather ----------------
        with ExitStack() as ph:
            spool = ph.enter_context(tc.tile_pool(name="k_s", bufs=3))
            pspool = ph.enter_context(
                tc.tile_pool(name="k_ps", bufs=3, space="PSUM"))
            for h in range(H):
                if h in qk_wts:
                    wt = qk_wts.pop(h)
                else:
                    wt = wflow.tile([128, KT, 128], BF16, name="qk_wt")
                    nc.sync.dma_start(out=wt[:], in_=qkw_d[H + h].rearrange(
                        "p (k c) -> p k c", k=KT))
                ps = pspool.tile([128, TLOC], F32, name="qk_ps")
                for k in range(KT):
                    nc.tensor.matmul(ps[:], wt[:, k, :], hT_sb[:, k, :],
                                     start=(k == 0), stop=(k == KT - 1))
                krot = spool.tile([128, TLOC], BF16, name="krot")
                rope_evict(ps, krot[:], spool)
                nc.sync.dma_start(out=kv_local[h], in_=krot[:])

        kt_cm = tc.tile_pool(name="ktpool", bufs=2)
        ktpool = kt_cm.__enter__()

        # V columns for attention rotate per 512-col block (2 resident:
        # heads 4fb..4fb+3 consume block fb while fb+1 streams in)
        vall_cm = tc.tile_pool(name="vall_pool", bufs=2)
        vap = vall_cm.__enter__()
        v_fbs = []

        # ---------------- phase 2b: V + allgather ----------------
        with ExitStack() as ph:
            spool = ph.enter_context(tc.tile_pool(name="v_s", bufs=3))
            vpspool = ph.enter_context(
                tc.tile_pool(name="v_psp", bufs=1, space="PSUM"))
            for fb in range(NFB):
                vps = [vpspool.tile([128, 512], F32, name=f"v_ps{it}",
                                    tag=f"v_ps{it}") for it in range(NT)]
                for k in range(KT):
                    vwt = wflow.tile([128, 512], BF16, name="vwt")
                    nc.sync.dma_start(out=vwt[:],
                                      in_=vw_d[k][:, fb * 512:(fb + 1) * 512])
                    for it in range(NT):
                        nc.tensor.matmul(vps[it][:],
                                         hT_sb[:, k, it * 128:(it + 1) * 128],
                                         vwt[:], start=(k == 0), stop=(k == KT - 1))
                for it in range(NT):
                    vsb = spool.tile([128, 512], BF16, name="vsb")
                    nc.scalar.copy(vsb[:], vps[it][:])
                    nc.sync.dma_start(out=kv_local[H + fb * NT + it],
                                      in_=vsb[:])
            if sim:
                for r in range(CORES_PER_B):
                    nc.gpsimd.dma_start(out=kv_full[r], in_=kv_local[:])
            else:
                nc.gpsimd.collective_compute(
                    "AllGather", ALU.bypass,
                    replica_groups=[[0, 1, 2, 3], [4, 5, 6, 7]],
                    ins=[kv_local.opt()], outs=[kv_full.opt()],
                )
            for fb in range(NFB):
                v_fb = vap.tile([128, NKT, 512], BF16, name="v_fb")
                for r in range(CORES_PER_B):
                    nc.gpsimd.dma_start(
                        out=v_fb[:, r * NT:(r + 1) * NT, :],
                        in_=kv_full[r, H + fb * NT:H + fb * NT + NT].rearrange(
                            "l p c -> p l c"))
                v_fbs.append(v_fb)

        # stage the first heads' K columns (gather overlaps the Q pass)
        kT_tiles = {}
        for h in range(2):
            kT_sb = ktpool.tile([128, T], BF16, name="kT_sb")
            for r in range(CORES_PER_B):
                nc.gpsimd.dma_start(
                    out=kT_sb[:, r * TLOC:(r + 1) * TLOC],
                    in_=kv_full[r, h])
            kT_tiles[h] = kT_sb

        # ---------------- phase 2c: Q heads + rope (KV gather overlaps) --
        with ExitStack() as ph:
            spool = ph.enter_context(tc.tile_pool(name="q_s", bufs=3))
            pspool = ph.enter_context(
                tc.tile_pool(name="q_ps", bufs=3, space="PSUM"))
            for h in range(H):
                wt = wflow.tile([128, KT, 128], BF16, name="qk_wt")
                nc.sync.dma_start(out=wt[:], in_=qkw_d[h].rearrange(
                    "p (k c) -> p k c", k=KT))
                ps = pspool.tile([128, TLOC], F32, name="qk_ps")
                for k in range(KT):
                    nc.tensor.matmul(ps[:], wt[:, k, :], hT_sb[:, k, :],
                                     start=(k == 0), stop=(k == KT - 1))
                rope_evict(ps, qrot_sb[:, h, :], spool)

        y_cm = tc.tile_pool(name="y_pool", bufs=1)
        y_pool = y_cm.__enter__()
        y_sb = y_pool.tile([128, H, TLOC], BF16)

        # ---------------- phase 3: attention (snake-folded causal) -------
        # kt block l=kt//4 covers local query cols [l*128:512); the first
        # 128 cols get the data-driven causal mask, the rest are always
        # fully allowed by construction of the snake fold.
        with ExitStack() as ph:
            apool = ph.enter_context(tc.tile_pool(name="att_pool", bufs=2))
            epool = ph.enter_context(tc.tile_pool(name="exp_pool", bufs=4))
            aps = ph.enter_context(tc.tile_pool(name="att_ps", bufs=2, space="PSUM"))
            sps_pool = ph.enter_context(
                tc.tile_pool(name="sps_pool", bufs=3, space="PSUM"))
            bps_pool = ph.enter_context(
                tc.tile_pool(name="bps_pool", bufs=1, space="PSUM"))

            for h in range(H):
                if h in kT_tiles:
                    kT_sb = kT_tiles.pop(h)
                else:
                    kT_sb = ktpool.tile([128, T], BF16, name="kT_sb")
                    for r in range(CORES_PER_B):
                        nc.gpsimd.dma_start(
                            out=kT_sb[:, r * TLOC:(r + 1) * TLOC],
                            in_=kv_full[r, h])
                yps = aps.tile([128, TLOC], F32, name="y_ps", tag="y_ps")
                sums = aps.tile([1, TLOC], F32, name="sums_ps", tag="sums_ps")
                for kt in range(NKT):
                    c0 = (kt // 4) * 128
                    w = TLOC - c0
                    kp = POS[kt]
                    sps = sps_pool.tile([128, TLOC], F32, name="s_ps",
                                        tag="s_ps")
                    nc.tensor.matmul(sps[:, :w], kT_sb[:, kp * 128:(kp + 1) * 128],
                                     qrot_sb[:, h, c0:TLOC], start=True,
                                     stop=True)
                    em = epool.tile([128, TLOC], BF16, name="em")
                    nc.scalar.activation(em[:, :w], sps[:, :w], AF.Exp,
                                         scale=SCALE)
                    nc.vector.tensor_tensor(em[:, 0:128], em[:, 0:128],
                                            trib_sb[:, kt, :], ALU.mult)
                    nc.tensor.matmul(yps[:, c0:TLOC],
                                     v_fbs[h // 4][:, kp,
                                                   (h % 4) * 128:
                                                   (h % 4 + 1) * 128],
                                     em[:, :w], start=(kt == 0),
                                     stop=(kt == NKT - 1),
                                     skip_group_check=True)
                    nc.tensor.matmul(sums[:, c0:TLOC], ones_col[:], em[:, :w],
                                     start=(kt == 0), stop=(kt == NKT - 1),
                                     skip_group_check=True)
                rec = apool.tile([1, TLOC], F32, name="rec_att")
                nc.vector.reciprocal(rec[:], sums[:])
                bps = bps_pool.tile([128, TLOC], F32, name="b_ps", tag="b_ps")
                nc.tensor.matmul(bps[:], ones_row[:], rec[:], start=True,
                                 stop=True)
                bsb = apool.tile([128, TLOC], F32, name="bsb")
                nc.vector.tensor_copy(bsb[:], bps[:])
                nc.vector.tensor_tensor(y_sb[:, h, :], yps[:], bsb[:], ALU.mult)

        # ------- phase 4: proj + residual -> x1 (DRAM spill) -------------
        with ExitStack() as ph:
            spool = ph.enter_context(tc.tile_pool(name="pj_s", bufs=4))
            pps = ph.enter_context(tc.tile_pool(name="pj_ps", bufs=2, space="PSUM"))
            for fb in range(NFB):
                pps_t = [pps.tile([128, 512], F32, name=f"p_ps{it}",
                                  tag=f"p_ps{it}") for it in range(NT)]
                for hd in range(H):
                    pwt = wflow.tile([128, 512], BF16, name="pwt")
                    nc.sync.dma_start(out=pwt[:],
                                      in_=pw_d[hd][:, fb * 512:(fb + 1) * 512])
                    for it in range(NT):
                        nc.tensor.matmul(pps_t[it][:],
                                         y_sb[:, hd, it * 128:(it + 1) * 128],
                                         pwt[:], start=(hd == 0),
                                         stop=(hd == H - 1))
                for it in range(NT):
                    xr = spool.tile([128, 512], F32, name="xr_p")
                    nc.sync.dma_start(
                        out=xr[:],
                        in_=x_d[it * 128:(it + 1) * 128,
                                fb * 512:(fb + 1) * 512])
                    x1t = spool.tile([128, 512], F32, name="x1t")
                    nc.vector.tensor_tensor(x1t[:], pps_t[it][:], xr[:],
                                            ALU.add)
                    nc.sync.dma_start(
                        out=x1_d[it][:, fb * 512:(fb + 1) * 512], in_=x1t[:])

        y_cm.__exit__(None, None, None)
        vall_cm.__exit__(None, None, None)
        kt_cm.__exit__(None, None, None)
        hT_cm.__exit__(None, None, None)
        qrot_cm.__exit__(None, None, None)

        # ---------------- phase 5: norm2 + h2^T ----------------
        h2T_cm = tc.tile_pool(name="h2T_pool", bufs=1)
        h2T_pool = h2T_cm.__enter__()
        h2T_sb = h2T_pool.tile([128, KT, TLOC], BF16)
        with ExitStack() as ph:
            x1p = ph.enter_context(tc.tile_pool(name="x1r_pool", bufs=1))
            x1rows = []
            for it in range(NT):
                x1r = x1p.tile([128, D], F32, name=f"x1r{it}", tag=f"x1r{it}")
                eng = nc.sync if it % 2 == 0 else nc.gpsimd
                eng.dma_start(out=x1r[:], in_=x1_d[it])
                x1rows.append(x1r)
            pool = ph.enter_context(tc.tile_pool(name="n2_pool", bufs=2))
            psum_pool = ph.enter_context(
                tc.tile_pool(name="n2_psum", bufs=4, space="PSUM"))
            rmsnorm_transpose(lambda it: x1rows[it][:], h2T_sb, pool, psum_pool)

        # ---------------- phase 6: gate/up ----------------
        gu_cm = tc.tile_pool(name="gu_pool", bufs=1)
        gu_pool = gu_cm.__enter__()
        gu_sb = gu_pool.tile([128, FT_FF, TLOC], BF16)
        with ExitStack() as ph:
            spool = ph.enter_context(tc.tile_pool(name="mlp_s", bufs=3))
            mps = ph.enter_context(tc.tile_pool(name="mlp_ps", bufs=4, space="PSUM"))
            for f in range(FT_FF):
                gwt = wflow.tile([128, KT, 128], BF16, name="gwt")
                nc.sync.dma_start(out=gwt[:], in_=gw_d[f].rearrange(
                    "p (k c) -> p k c", k=KT))
                gps = mps.tile([128, TLOC], F32, name="g_ps", tag="g_ps")
                for k in range(KT):
                    nc.tensor.matmul(gps[:], gwt[:, k, :], h2T_sb[:, k, :],
                                     start=(k == 0), stop=(k == KT - 1))
                gsil = spool.tile([128, TLOC], BF16, name="gsil")
                nc.scalar.activation(gsil[:], gps[:], AF.Silu)
                uwt = wflow.tile([128, KT, 128], BF16, name="uwt")
                nc.sync.dma_start(out=uwt[:], in_=uw_d[f].rearrange(
                    "p (k c) -> p k c", k=KT))
                ups = mps.tile([128, TLOC], F32, name="u_ps", tag="u_ps")
                for k in range(KT):
                    nc.tensor.matmul(ups[:], uwt[:, k, :], h2T_sb[:, k, :],
                                     start=(k == 0), stop=(k == KT - 1))
                nc.vector.tensor_tensor(gu_sb[:, f, :], ups[:], gsil[:],
                                        ALU.mult)

        # ---------------- phase 7: down + residual -> out ----------------
        with ExitStack() as ph:
            spool = ph.enter_context(tc.tile_pool(name="dn_s", bufs=8))
            dps = ph.enter_context(tc.tile_pool(name="dn_ps", bufs=1, space="PSUM"))
            for fbp in range(2):
                dps_t = [[dps.tile([128, 512], F32, name=f"d_ps{it}_{fbi}",
                                   tag=f"d_ps{it}_{fbi}") for fbi in range(2)]
                         for it in range(NT)]
                for k in range(FT_FF):
                    dwt = wflow.tile([128, 1024], BF16, name="dwt")
                    nc.sync.dma_start(
                        out=dwt[:],
                        in_=dw_d[k][:, fbp * 1024:(fbp + 1) * 1024])
                    for it in range(NT):
                        for fbi in range(2):
                            nc.tensor.matmul(
                                dps_t[it][fbi][:],
                                gu_sb[:, k, it * 128:(it + 1) * 128],
                                dwt[:, fbi * 512:(fbi + 1) * 512],
                                start=(k == 0), stop=(k == FT_FF - 1))
                for it in range(NT):
                    for fbi in range(2):
                        fb = fbp * 2 + fbi
                        xr = spool.tile([128, 512], F32, name="xr_d")
                        nc.gpsimd.dma_start(
                            out=xr[:],
                            in_=x1_d[it][:, fb * 512:(fb + 1) * 512])
                        osb = spool.tile([128, 512], F32, name="osb_d")
                        nc.vector.tensor_tensor(
                            osb[:], dps_t[it][fbi][:], xr[:], ALU.add)
                        nc.sync.dma_start(
                            out=out_d[it * 128:(it + 1) * 128,
                                      fb * 512:(fb + 1) * 512],
                            in_=osb[:])

        gu_cm.__exit__(None, None, None)
        h2T_cm.__exit__(None, None, None)
        wflow_cm.__exit__(None, None, None)


def core_token_idx(c):
    """Global token indices (within the batch row) owned by group-core c."""
    return np.concatenate([np.arange(g * 128, (g + 1) * 128)
                           for g in snake_tiles(c)])


def prepare_inputs(x, f_cos, f_sin, w_attn, w_proj, w_gate, w_up, w_down, g1, g2):
    """Host-side sharding + weight re-layout. Returns list of 8 input dicts."""
    x = np.asarray(x, dtype=np.float32)
    f_cos = np.asarray(f_cos, dtype=np.float32)
    f_sin = np.asarray(f_sin, dtype=np.float32)
    w_attn = np.asarray(w_attn, dtype=np.float32)
    g1 = np.asarray(g1, dtype=np.float32)
    g2 = np.asarray(g2, dtype=np.float32)

    perm = _rope_perm()
    wq = w_attn[0:D] * g1[None, :]
    wk = w_attn[D:2 * D] * g1[None, :]
    wv = w_attn[2 * D:3 * D] * g1[None, :]
    # permute rows within each head for q and k
    wq_p = wq.reshape(H, DK, D)[:, perm, :].reshape(H * DK, D)
    wk_p = wk.reshape(H, DK, D)[:, perm, :].reshape(H * DK, D)

    def lhsT_layout(w):  # w: [F, D] -> [F/128, 128(d within k-tile), D(k*128+c)]
        f = w.shape[0]
        # out[ft, p, k*128+c] = w[ft*128+c, k*128+p]
        a = w.reshape(f // 128, 128, KT, 128)       # [ft, c, k, p]
        a = a.transpose(0, 3, 2, 1).reshape(f // 128, 128, D)  # [ft, p, (k c)]
        return np.ascontiguousarray(a).astype(ml_dtypes.bfloat16)

    def rhsT_layout(w):  # w: [F, D_in] -> [D_in/128, 128(p), F] = w.T tiled
        d_in = w.shape[1]
        a = w.T.reshape(d_in // 128, 128, w.shape[0])  # [k, p, c]
        return np.ascontiguousarray(a).astype(ml_dtypes.bfloat16)

    qk_w = np.concatenate([lhsT_layout(wq_p), lhsT_layout(wk_p)], axis=0)
    v_w = rhsT_layout(wv)
    proj_w = rhsT_layout(np.asarray(w_proj, dtype=np.float32))
    gate_w = lhsT_layout(np.asarray(w_gate, dtype=np.float32) * g2[None, :])
    up_w = lhsT_layout(np.asarray(w_up, dtype=np.float32) * g2[None, :])
    down_w = rhsT_layout(np.asarray(w_down, dtype=np.float32))

    # cs1/cs2 in permuted-lane layout: [128, T]
    pair = np.zeros(DK, dtype=np.int64)
    sign = np.zeros(DK, dtype=np.float32)
    for p in range(DK):
        qd, j = p // 32, p % 32
        pair[p] = 16 * qd + (j if j < 16 else j - 16)
        sign[p] = -1.0 if j < 16 else 1.0
    cs1_full = f_cos.T[pair, :]                       # [128, T]
    cs2_full = f_sin.T[pair, :] * sign[:, None]       # [128, T]

    in_maps = []
    for core in range(N_CORES):
        b, c = core // CORES_PER_B, core % CORES_PER_B
        tok = core_token_idx(c)
        tiles = snake_tiles(c)
        # causal mask tiles: kt covers query tile l=kt//4 (this core's
        # global tile tiles[l]); allowed iff key_pos <= query_pos
        tri = np.zeros((NKT, 128, 128), dtype=np.float32)
        kk = np.arange(128)[:, None]
        qq = np.arange(128)[None, :]
        for kt in range(NKT):
            g = tiles[kt // 4]
            tri[kt] = (kt * 128 + kk) <= (g * 128 + qq)
        tri = np.ascontiguousarray(
            tri.transpose(1, 0, 2).reshape(128, NKT * 128))
        in_maps.append({
            "x": np.ascontiguousarray(x[b, tok, :]),
            "qk_w": qk_w, "v_w": v_w, "proj_w": proj_w,
            "gate_w": gate_w, "up_w": up_w, "down_w": down_w,
            "cs1": np.ascontiguousarray(cs1_full[:, tok]),
            "cs2": np.ascontiguousarray(cs2_full[:, tok]),
            "tri": tri.astype(ml_dtypes.bfloat16),
        })
    return in_maps


def assemble_output(results):
    out = np.zeros((B, T, D), dtype=np.float32)
    for core in range(N_CORES):
        b, c = core // CORES_PER_B, core % CORES_PER_B
        out[b, core_token_idx(c), :] = results[core]["out"]
    return out


_CACHE = {}
_LOCK = threading.Lock()


def get_program():
    with _LOCK:
        if "nc" not in _CACHE:
            _CACHE["nc"] = build_program()
        return _CACHE["nc"]


def kernel(**inputs):
    nc = get_program()
    in_maps = prepare_inputs(**inputs)
    res = run_bass_kernel_spmd(nc, in_maps, list(range(N_CORES)))
    return assemble_output(res.results)


def bench(inputs, iters=10):
    """Wall-clock the sharded executable with device-resident inputs.

    Returns the mean pipelined per-call time in ns (upper bound on HW exec
    time: it includes 1/iters of the axon dispatch round-trip)."""
    import jax
    from jax.sharding import Mesh, PartitionSpec, NamedSharding
    from jax.experimental.shard_map import shard_map
    from concourse import bass2jax, mybir as mb

    nc = get_program()
    in_maps = prepare_inputs(**inputs)
    bass2jax.install_neuronx_cc_hook()

    partition_name = (nc.partition_id_tensor.name
                      if nc.partition_id_tensor else None)
    in_names, out_names, out_avals, zero_outs = [], [], [], []
    for alloc in nc.m.functions[0].allocations:
        if not isinstance(alloc, mb.MemoryLocationSet):
            continue
        name = alloc.memorylocations[0].name
        if alloc.kind == "ExternalInput":
            if name != partition_name:
                in_names.append(name)
        elif alloc.kind == "ExternalOutput":
            shape = tuple(alloc.tensor_shape)
            dtype = mb.dt.np(alloc.dtype)
            out_names.append(name)
            out_avals.append(jax.core.ShapedArray(shape, dtype))
            zero_outs.append(np.zeros(shape, dtype))
    n_params = len(in_names)
    all_in_names = list(in_names) + list(out_names)
    if partition_name is not None:
        all_in_names.append(partition_name)
    donate = tuple(range(n_params, n_params + len(out_names)))

    def _body(*args):
        operands = list(args)
        if partition_name is not None:
            operands.append(bass2jax.partition_id_tensor())
        return tuple(bass2jax._bass_exec_p.bind(
            *operands,
            out_avals=tuple(out_avals),
            in_names=tuple(all_in_names),
            out_names=tuple(out_names),
            lowering_input_output_aliases=(),
            sim_require_finite=True,
            sim_require_nnan=True,
            nc=nc,
        ))

    devices = jax.devices()[:N_CORES]
    mesh = Mesh(np.asarray(devices), ("core",))
    in_specs = (PartitionSpec("core"),) * (n_params + len(out_names))
    out_specs = (PartitionSpec("core"),) * len(out_names)
    sharded = jax.jit(
        shard_map(_body, mesh=mesh, in_specs=in_specs, out_specs=out_specs,
                  check_rep=False),
        donate_argnums=donate, keep_unused=True)

    sh = NamedSharding(mesh, PartitionSpec("core"))
    concat_in = [
        jax.device_put(
            np.concatenate([np.asarray(in_maps[c][nm]) for c in range(N_CORES)],
                           axis=0), sh)
        for nm in in_names]
    jax.block_until_ready(concat_in)

    def make_zeros():
        return [jax.device_put(
            np.zeros((N_CORES * z.shape[0], *z.shape[1:]), z.dtype), sh)
            for z in zero_outs]

    # warmup (compile)
    outs = sharded(*concat_in, *make_zeros())
    jax.block_until_ready(outs)

    zs = [make_zeros() for _ in range(iters)]
    for z in zs:
        jax.block_until_ready(z)
    # async pipelined dispatch amortizes the ~100ms axon round-trip
    t0 = time.perf_counter()
    outs = [sharded(*concat_in, *zs[i]) for i in range(iters)]
    jax.block_until_ready(outs)
    dt = (time.perf_counter() - t0) / iters
    return dt * 1e9


# revision 17
# speedup vs baseline: 1.8693x; 1.0008x over previous
"""Trainium2 Bass kernel for a dense transformer block.

Reference computation (B=2, T=2048, D=2048, H=16, Dk=128, FF=8192, fp32):
    h   = rmsnorm(x, g1)
    qkv = h @ w_attn.T ; q,k = rope(q,k) ; y = causal_softmax(q k^T / sqrt(Dk)) v
    x1  = x + y @ w_proj.T
    h2  = rmsnorm(x1, g2)
    out = x1 + (silu(h2 @ w_gate.T) * (h2 @ w_up.T)) @ w_down.T

Distribution: data-parallel over tokens, 512 per core (cores 0-3: batch 0,
cores 4-7: batch 1). Token tiles are "snake"-folded across the 4-core group:
core c owns global 128-token tiles {c, 7-c, 8+c, 15-c}, so every core's
causal key footprint is identical (tiles 0..3 attend 4 key tiles, 4..7
attend 8, 8..11 attend 12, 12..15 attend 16 -> 62.5% of the dense score/AV
work, perfectly balanced). Causal masking within the padded footprint is
data-driven (per-core 0/1 mask tiles multiply the exp'd scores), which keeps
the SPMD program identical on all cores. K,V are computed locally and
AllGather'd inside each 4-core group (V gathers split per 512-col block so
attention can start while late blocks are still in flight).

All weight matrices stream through one shared SBUF pool, so the DMA queue
naturally prefetches the next phase's weights while the current phase
computes. Matmuls run in bf16 with fp32 PSUM accumulation. Residuals and
normalization in fp32. RoPE is applied in the transposed [dk, t] layout via
a host-side permutation of the head dimension + DVE stream_shuffle.
"""

import os
import sys
import threading
import time

import numpy as np

for _p in ("/opt/trn_rl_repo", os.path.expanduser("~/.axon_site/_ro/trn_rl_repo")):
    if _p not in sys.path and os.path.isdir(_p):
        sys.path.append(_p)

import ml_dtypes  # noqa: E402

import concourse.bass as bass  # noqa: E402
import concourse.mybir as mybir  # noqa: E402
import concourse.tile as tile  # noqa: E402
from concourse import bacc  # noqa: E402
from concourse.bass_utils import run_bass_kernel_spmd  # noqa: E402
from concourse.masks import make_identity  # noqa: E402
from contextlib import ExitStack  # noqa: E402

F32 = mybir.dt.float32
BF16 = mybir.dt.bfloat16
AF = mybir.ActivationFunctionType
ALU = mybir.AluOpType

B, T, D = 2, 2048, 2048
H, DK, FF = 16, 128, 8192
EPS = 1e-6
N_CORES = 8
TLOC = T * B // N_CORES          # 512 tokens per core
CORES_PER_B = N_CORES // B       # 4
KT = D // 128                    # 16 d-tiles
NT = TLOC // 128                 # 4 t-tiles per core
NKT = T // 128                   # 16 key subtiles (full sequence)
FT_FF = FF // 128                # 64 ff tiles
NFB = D // 512                   # 4 v/proj 512-col blocks
SCALE = 1.0 / float(np.sqrt(DK))
SHUF_MASK = [(j + 16) % 32 for j in range(32)]


def snake_tiles(c):
    """Global 128-token tile indices owned by group-core c, local order."""
    return [c, 7 - c, 8 + c, 15 - c]


def _gmaps():
    """global tile g -> (owning group-core, local tile index)."""
    rmap, lmap = [0] * NKT, [0] * NKT
    for g in range(NKT):
        for r in range(CORES_PER_B):
            if g in snake_tiles(r):
                rmap[g], lmap[g] = r, snake_tiles(r).index(g)
    return rmap, lmap


RMAP, LMAP = _gmaps()
# core-major position of global tile g inside gathered K/V SBUF tiles
POS = [RMAP[g] * NT + LMAP[g] for g in range(NKT)]


def _rope_perm():
    """Within-head row permutation: pair i=(16*qd + j) real part -> partition
    32*qd + j, imag part -> partition 32*qd + 16 + j."""
    perm = np.zeros(DK, dtype=np.int64)
    for p in range(DK):
        qd, j = p // 32, p % 32
        i = 16 * qd + (j if j < 16 else j - 16)
        perm[p] = 2 * i + (0 if j < 16 else 1)
    return perm


def build_program(sim=False, repeat=1):
    nc = bacc.Bacc("TRN2", target_bir_lowering=False, debug=False,
                   num_devices=1 if sim else N_CORES)

    x_d = nc.declare_dram_parameter("x", [TLOC, D], F32, isOutput=False)
    qkw_d = nc.declare_dram_parameter("qk_w", [2 * H, 128, D], BF16, isOutput=False)
    vw_d = nc.declare_dram_parameter("v_w", [KT, 128, D], BF16, isOutput=False)
    pw_d = nc.declare_dram_parameter("proj_w", [H, 128, D], BF16, isOutput=False)
    gw_d = nc.declare_dram_parameter("gate_w", [FT_FF, 128, D], BF16, isOutput=False)
    uw_d = nc.declare_dram_parameter("up_w", [FT_FF, 128, D], BF16, isOutput=False)
    dw_d = nc.declare_dram_parameter("down_w", [FT_FF, 128, D], BF16, isOutput=False)
    cs1_d = nc.declare_dram_parameter("cs1", [128, TLOC], F32, isOutput=False)
    cs2_d = nc.declare_dram_parameter("cs2", [128, TLOC], F32, isOutput=False)
    tri_d = nc.declare_dram_parameter("tri", [128, NKT * 128], BF16,
                                      isOutput=False)
    out_d = nc.declare_dram_parameter("out", [TLOC, D], F32, isOutput=True)

    with ExitStack() as ctx:
        tc = ctx.enter_context(tile.TileContext(nc))
        for _rep in range(repeat):
            _emit_block(nc, tc, sim, x_d, qkw_d, vw_d, pw_d, gw_d, uw_d, dw_d,
                        cs1_d, cs2_d, tri_d, out_d)

    nc.compile()
    return nc


def _emit_block(nc, tc, sim, x_d, qkw_d, vw_d, pw_d, gw_d, uw_d, dw_d,
                cs1_d, cs2_d, tri_d, out_d):
    with ExitStack() as ctx:
        const = ctx.enter_context(tc.tile_pool(name="const", bufs=1))
        ident = const.tile([128, 128], BF16)
        make_identity(nc, ident)
        ones_col = const.tile([128, 1], BF16)
        nc.vector.memset(ones_col, 1.0)
        ones_row = const.tile([1, 128], F32)
        nc.vector.memset(ones_row, 1.0)
        cs1_sb = const.tile([128, TLOC], F32)
        nc.sync.dma_start(out=cs1_sb[:], in_=cs1_d[:, :])
        cs2_sb = const.tile([128, TLOC], F32)
        nc.sync.dma_start(out=cs2_sb[:], in_=cs2_d[:, :])
        trib_sb = const.tile([128, NKT, 128], BF16)
        nc.gpsimd.dma_start(out=trib_sb[:], in_=tri_d.rearrange(
            "p (n q) -> p n q", n=NKT))

        # shared streaming pool for ALL weight tiles: one rotation across
        # phases lets the DMA queue prefetch phase N+1's weights during
        # phase N's compute (slot = 4KB/partition).
        wflow_cm = tc.tile_pool(name="wflow", bufs=5)
        wflow = wflow_cm.__enter__()

        # DRAM scratch: K/V allgather buffers + x1 spill
        dram = ctx.enter_context(tc.tile_pool(name="dram", bufs=1, space="DRAM"))
        # merged K+V allgather payload: entries 0..H-1 = K heads,
        # H + fb*NT + l = V block (fb, local tile l). One collective per
        # call (collective launches carry ~1ms/call fixed runtime cost).
        kv_local = dram.tile([2 * H, 128, TLOC], BF16)
        kv_full = dram.tile([CORES_PER_B, 2 * H, 128, TLOC], BF16)
        x1_d = dram.tile([NT, 128, D], F32)

        def rmsnorm_transpose(loader, dst_sb, pool, psum_pool):
            """loader(it) -> [128, D] fp32 AP; writes dst_sb [128, KT, TLOC]
            bf16 = (rms-normalized rows) transposed. Gains folded in weights."""
            rstds = []
            for it in range(NT):
                sq_scr = pool.tile([128, D], BF16, name="sq_scr")
                ssq = pool.tile([128, 1], F32, name=f"ssq{it}", tag=f"ssq{it}",
                                bufs=1)
                nc.scalar.activation(sq_scr[:], loader(it), AF.Square,
                                     accum_out=ssq[:])
                mean = pool.tile([128, 1], F32, name="mean")
                nc.vector.tensor_scalar(mean[:], ssq[:], 1.0 / D, EPS,
                                        ALU.mult, ALU.add)
                rec = pool.tile([128, 1], F32, name="rec")
                nc.vector.reciprocal(rec[:], mean[:])
                rstd = pool.tile([128, 1], F32, name=f"rstd{it}",
                                 tag=f"rstd{it}", bufs=1)
                nc.scalar.activation(rstd[:], rec[:], AF.Sqrt)
                rstds.append(rstd)
            hrows = []
            for it in range(NT):
                hrow = pool.tile([128, D], BF16, name=f"hrow{it}",
                                 tag=f"hrow{it}", bufs=1)
                nc.vector.tensor_scalar(hrow[:], loader(it), rstds[it][:],
                                        None, ALU.mult)
                hrows.append(hrow)
            # k-outer so dst_sb[:, k, :] completes early for the consumers
            for k in range(KT):
                for it in range(NT):
                    tp = psum_pool.tile([128, 128], BF16, name="tp")
                    nc.tensor.transpose(tp[:], hrows[it][:, k * 128:(k + 1) * 128],
                                        ident[:])
                    nc.vector.tensor_copy(dst_sb[:, k, it * 128:(it + 1) * 128],
                                          tp[:])

        def rope_evict(ps, dst, pool):
            """ps: [128, TLOC] psum q/k head tile (permuted lanes) -> rotated"""
            sh = pool.tile([128, TLOC], F32, name="rp_sh")
            nc.vector.stream_shuffle(sh[:], ps[:], mask=SHUF_MASK)
            t1 = pool.tile([128, TLOC], F32, name="rp_t1")
            nc.vector.tensor_tensor(t1[:], ps[:], cs1_sb[:], ALU.mult)
            t2 = pool.tile([128, TLOC], F32, name="rp_t2")
            nc.vector.tensor_tensor(t2[:], sh[:], cs2_sb[:], ALU.mult)
            nc.vector.tensor_tensor(dst[:], t1[:], t2[:], ALU.add)

        # persistent pools, strict LIFO
        qrot_cm = tc.tile_pool(name="qrot_pool", bufs=1)
        qrot_pool = qrot_cm.__enter__()
        qrot_sb = qrot_pool.tile([128, H, TLOC], BF16)
        hT_cm = tc.tile_pool(name="hT_pool", bufs=1)
        hT_pool = hT_cm.__enter__()
        hT_sb = hT_pool.tile([128, KT, TLOC], BF16)

        # ---------------- phase 1: norm1 + h^T ----------------
        x_cm = tc.tile_pool(name="xpool", bufs=1)
        xpool = x_cm.__enter__()
        x_sb = xpool.tile([128, NT, D], F32)
        for it in range(NT):
            eng = nc.sync if it % 2 == 0 else nc.gpsimd
            eng.dma_start(out=x_sb[:, it, :],
                          in_=x_d[it * 128:(it + 1) * 128, :])
        # prefetch the first K-head weight tiles behind the x loads
        qk_wts = {}
        for h in range(2):
            wt = wflow.tile([128, KT, 128], BF16, name="qk_wt")
            nc.sync.dma_start(out=wt[:], in_=qkw_d[H + h].rearrange(
                "p (k c) -> p k c", k=KT))
            qk_wts[h] = wt
        with ExitStack() as ph:
            pool = ph.enter_context(tc.tile_pool(name="n1_pool", bufs=2))
            psum_pool = ph.enter_context(
                tc.tile_pool(name="n1_psum", bufs=4, space="PSUM"))
            rmsnorm_transpose(lambda it: x_sb[:, it, :], hT_sb, pool, psum_pool)
        x_cm.__exit__(None, None, None)

        # ---------------- phase 2a: K heads + allgather ----------------
        with ExitStack() as ph:
            spool = ph.enter_context(tc.tile_pool(name="k_s", bufs=3))
            pspool = ph.enter_context(
                tc.tile_pool(name="k_ps", bufs=3, space="PSUM"))
            for h in range(H):
                if h in qk_wts:
                    wt = qk_wts.pop(h)
                else:
                    wt = wflow.tile([128, KT, 128], BF16, name="qk_wt")
                    nc.sync.dma_start(out=wt[:], in_=qkw_d[H + h].rearrange(
                        "p (k c) -> p k c", k=KT))
                ps = pspool.tile([128, TLOC], F32, name="qk_ps")
                for k in range(KT):
                    nc.tensor.matmul(ps[:], wt[:, k, :], hT_sb[:, k, :],
                                     start=(k == 0), stop=(k == KT - 1))
                krot = spool.tile([128, TLOC], BF16, name="krot")
                rope_evict(ps, krot[:], spool)
                nc.sync.dma_start(out=kv_local[h], in_=krot[:])

        kt_cm = tc.tile_pool(name="ktpool", bufs=3)
        ktpool = kt_cm.__enter__()

        # V columns for attention rotate per 512-col block (2 resident:
        # heads 4fb..4fb+3 consume block fb while fb+1 streams in)
        vall_cm = tc.tile_pool(name="vall_pool", bufs=2)
        vap = vall_cm.__enter__()
        v_fbs = []

        # ---------------- phase 2b: V + allgather ----------------
        with ExitStack() as ph:
            spool = ph.enter_context(tc.tile_pool(name="v_s", bufs=3))
            vpspool = ph.enter_context(
                tc.tile_pool(name="v_psp", bufs=1, space="PSUM"))
            for fb in range(NFB):
                vps = [vpspool.tile([128, 512], F32, name=f"v_ps{it}",
                                    tag=f"v_ps{it}") for it in range(NT)]
                for k in range(KT):
                    vwt = wflow.tile([128, 512], BF16, name="vwt")
                    nc.sync.dma_start(out=vwt[:],
                                      in_=vw_d[k][:, fb * 512:(fb + 1) * 512])
                    for it in range(NT):
                        nc.tensor.matmul(vps[it][:],
                                         hT_sb[:, k, it * 128:(it + 1) * 128],
                                         vwt[:], start=(k == 0), stop=(k == KT - 1))
                for it in range(NT):
                    vsb = spool.tile([128, 512], BF16, name="vsb")
                    nc.scalar.copy(vsb[:], vps[it][:])
                    nc.sync.dma_start(out=kv_local[H + fb * NT + it],
                                      in_=vsb[:])
            if sim:
                for r in range(CORES_PER_B):
                    nc.gpsimd.dma_start(out=kv_full[r], in_=kv_local[:])
            else:
                nc.gpsimd.collective_compute(
                    "AllGather", ALU.bypass,
                    replica_groups=[[0, 1, 2, 3], [4, 5, 6, 7]],
                    ins=[kv_local.opt()], outs=[kv_full.opt()],
                )
            for fb in range(NFB):
                v_fb = vap.tile([128, NKT, 512], BF16, name="v_fb")
                for r in range(CORES_PER_B):
                    nc.gpsimd.dma_start(
                        out=v_fb[:, r * NT:(r + 1) * NT, :],
                        in_=kv_full[r, H + fb * NT:H + fb * NT + NT].rearrange(
                            "l p c -> p l c"))
                v_fbs.append(v_fb)

        # stage the first heads' K columns (gather overlaps the Q pass)
        kT_tiles = {}
        for h in range(2):
            kT_sb = ktpool.tile([128, T], BF16, name="kT_sb")
            for r in range(CORES_PER_B):
                nc.gpsimd.dma_start(
                    out=kT_sb[:, r * TLOC:(r + 1) * TLOC],
                    in_=kv_full[r, h])
            kT_tiles[h] = kT_sb

        # ---------------- phase 2c: Q heads + rope (KV gather overlaps) --
        with ExitStack() as ph:
            qwpool = ph.enter_context(tc.tile_pool(name="q_w", bufs=3))
            spool = ph.enter_context(tc.tile_pool(name="q_s", bufs=3))
            pspool = ph.enter_context(
                tc.tile_pool(name="q_ps", bufs=3, space="PSUM"))
            for h in range(H):
                wt = qwpool.tile([128, KT, 128], BF16, name="q_wt")
                nc.sync.dma_start(out=wt[:], in_=qkw_d[h].rearrange(
                    "p (k c) -> p k c", k=KT))
                ps = pspool.tile([128, TLOC], F32, name="qk_ps")
                for k in range(KT):
                    nc.tensor.matmul(ps[:], wt[:, k, :], hT_sb[:, k, :],
                                     start=(k == 0), stop=(k == KT - 1))
                rope_evict(ps, qrot_sb[:, h, :], spool)

        y_cm = tc.tile_pool(name="y_pool", bufs=1)
        y_pool = y_cm.__enter__()
        y_sb = y_pool.tile([128, H, TLOC], BF16)

        # ---------------- phase 3: attention (snake-folded causal) -------
        # kt block l=kt//4 covers local query cols [l*128:512); the first
        # 128 cols get the data-driven causal mask, the rest are always
        # fully allowed by construction of the snake fold.
        with ExitStack() as ph:
            apool = ph.enter_context(tc.tile_pool(name="att_pool", bufs=2))
            epool = ph.enter_context(tc.tile_pool(name="exp_pool", bufs=6))
            aps = ph.enter_context(tc.tile_pool(name="att_ps", bufs=2, space="PSUM"))
            sps_pool = ph.enter_context(
                tc.tile_pool(name="sps_pool", bufs=3, space="PSUM"))
            bps_pool = ph.enter_context(
                tc.tile_pool(name="bps_pool", bufs=1, space="PSUM"))

            for h in range(H):
                if h in kT_tiles:
                    kT_sb = kT_tiles.pop(h)
                else:
                    kT_sb = ktpool.tile([128, T], BF16, name="kT_sb")
                    for r in range(CORES_PER_B):
                        nc.gpsimd.dma_start(
                            out=kT_sb[:, r * TLOC:(r + 1) * TLOC],
                            in_=kv_full[r, h])
                yps = aps.tile([128, TLOC], F32, name="y_ps", tag="y_ps")
                sums = aps.tile([1, TLOC], F32, name="sums_ps", tag="sums_ps")
                for kt in range(NKT):
                    c0 = (kt // 4) * 128
                    w = TLOC - c0
                    kp = POS[kt]
                    sps = sps_pool.tile([128, TLOC], F32, name="s_ps",
                                        tag="s_ps")
                    nc.tensor.matmul(sps[:, :w], kT_sb[:, kp * 128:(kp + 1) * 128],
                                     qrot_sb[:, h, c0:TLOC], start=True,
                                     stop=True)
                    em = epool.tile([128, TLOC], BF16, name="em")
                    nc.scalar.activation(em[:, :w], sps[:, :w], AF.Exp,
                                         scale=SCALE)
                    nc.vector.tensor_tensor(em[:, 0:128], em[:, 0:128],
                                            trib_sb[:, kt, :], ALU.mult)
                    nc.tensor.matmul(yps[:, c0:TLOC],
                                     v_fbs[h // 4][:, kp,
                                                   (h % 4) * 128:
                                                   (h % 4 + 1) * 128],
                                     em[:, :w], start=(kt == 0),
                                     stop=(kt == NKT - 1),
                                     skip_group_check=True)
                    nc.tensor.matmul(sums[:, c0:TLOC], ones_col[:], em[:, :w],
                                     start=(kt == 0), stop=(kt == NKT - 1),
                                     skip_group_check=True)
                rec = apool.tile([1, TLOC], F32, name="rec_att")
                nc.vector.reciprocal(rec[:], sums[:])
                bps = bps_pool.tile([128, TLOC], F32, name="b_ps", tag="b_ps")
                nc.tensor.matmul(bps[:], ones_row[:], rec[:], start=True,
                                 stop=True)
                bsb = apool.tile([128, TLOC], F32, name="bsb")
                nc.vector.tensor_copy(bsb[:], bps[:])
                nc.vector.tensor_tensor(y_sb[:, h, :], yps[:], bsb[:], ALU.mult)

        # ------- phase 4: proj + residual -> x1 (DRAM spill) -------------
        with ExitStack() as ph:
            spool = ph.enter_context(tc.tile_pool(name="pj_s", bufs=4))
            pps = ph.enter_context(tc.tile_pool(name="pj_ps", bufs=2, space="PSUM"))
            for fb in range(NFB):
                pps_t = [pps.tile([128, 512], F32, name=f"p_ps{it}",
                                  tag=f"p_ps{it}") for it in range(NT)]
                for hd in range(H):
                    pwt = wflow.tile([128, 512], BF16, name="pwt")
                    nc.sync.dma_start(out=pwt[:],
                                      in_=pw_d[hd][:, fb * 512:(fb + 1) * 512])
                    for it in range(NT):
                        nc.tensor.matmul(pps_t[it][:],
                                         y_sb[:, hd, it * 128:(it + 1) * 128],
                                         pwt[:], start=(hd == 0),
                                         stop=(hd == H - 1))
                for it in range(NT):
                    xr = spool.tile([128, 512], F32, name="xr_p")
                    nc.sync.dma_start(
                        out=xr[:],
                        in_=x_d[it * 128:(it + 1) * 128,
                                fb * 512:(fb + 1) * 512])
                    x1t = spool.tile([128, 512], F32, name="x1t")
                    nc.vector.tensor_tensor(x1t[:], pps_t[it][:], xr[:],
                                            ALU.add)
                    nc.sync.dma_start(
                        out=x1_d[it][:, fb * 512:(fb + 1) * 512], in_=x1t[:])

        y_cm.__exit__(None, None, None)
        vall_cm.__exit__(None, None, None)
        kt_cm.__exit__(None, None, None)
        hT_cm.__exit__(None, None, None)
        qrot_cm.__exit__(None, None, None)

        # ---------------- phase 5: norm2 + h2^T ----------------
        h2T_cm = tc.tile_pool(name="h2T_pool", bufs=1)
        h2T_pool = h2T_cm.__enter__()
        h2T_sb = h2T_pool.tile([128, KT, TLOC], BF16)
        with ExitStack() as ph:
            x1p = ph.enter_context(tc.tile_pool(name="x1r_pool", bufs=1))
            x1rows = []
            for it in range(NT):
                x1r = x1p.tile([128, D], F32, name=f"x1r{it}", tag=f"x1r{it}")
                eng = nc.sync if it % 2 == 0 else nc.gpsimd
                eng.dma_start(out=x1r[:], in_=x1_d[it])
                x1rows.append(x1r)
            pool = ph.enter_context(tc.tile_pool(name="n2_pool", bufs=2))
            psum_pool = ph.enter_context(
                tc.tile_pool(name="n2_psum", bufs=4, space="PSUM"))
            rmsnorm_transpose(lambda it: x1rows[it][:], h2T_sb, pool, psum_pool)

        # ---------------- phase 6: gate/up ----------------
        gu_cm = tc.tile_pool(name="gu_pool", bufs=1)
        gu_pool = gu_cm.__enter__()
        gu_sb = gu_pool.tile([128, FT_FF, TLOC], BF16)
        with ExitStack() as ph:
            spool = ph.enter_context(tc.tile_pool(name="mlp_s", bufs=3))
            mps = ph.enter_context(tc.tile_pool(name="mlp_ps", bufs=4, space="PSUM"))
            for f in range(FT_FF):
                gwt = wflow.tile([128, KT, 128], BF16, name="gwt")
                nc.sync.dma_start(out=gwt[:], in_=gw_d[f].rearrange(
                    "p (k c) -> p k c", k=KT))
                gps = mps.tile([128, TLOC], F32, name="g_ps", tag="g_ps")
                for k in range(KT):
                    nc.tensor.matmul(gps[:], gwt[:, k, :], h2T_sb[:, k, :],
                                     start=(k == 0), stop=(k == KT - 1))
                gsil = spool.tile([128, TLOC], BF16, name="gsil")
                nc.scalar.activation(gsil[:], gps[:], AF.Silu)
                uwt = wflow.tile([128, KT, 128], BF16, name="uwt")
                nc.sync.dma_start(out=uwt[:], in_=uw_d[f].rearrange(
                    "p (k c) -> p k c", k=KT))
                ups = mps.tile([128, TLOC], F32, name="u_ps", tag="u_ps")
                for k in range(KT):
                    nc.tensor.matmul(ups[:], uwt[:, k, :], h2T_sb[:, k, :],
                                     start=(k == 0), stop=(k == KT - 1))
                nc.vector.tensor_tensor(gu_sb[:, f, :], ups[:], gsil[:],
                                        ALU.mult)

        # ---------------- phase 7: down + residual -> out ----------------
        with ExitStack() as ph:
            spool = ph.enter_context(tc.tile_pool(name="dn_s", bufs=8))
            dps = ph.enter_context(tc.tile_pool(name="dn_ps", bufs=1, space="PSUM"))
            for fbp in range(2):
                dps_t = [[dps.tile([128, 512], F32, name=f"d_ps{it}_{fbi}",
                                   tag=f"d_ps{it}_{fbi}") for fbi in range(2)]
                         for it in range(NT)]
                for k in range(FT_FF):
                    dwt = wflow.tile([128, 1024], BF16, name="dwt")
                    nc.sync.dma_start(
                        out=dwt[:],
                        in_=dw_d[k][:, fbp * 1024:(fbp + 1) * 1024])
                    for it in range(NT):
                        for fbi in range(2):
                            nc.tensor.matmul(
                                dps_t[it][fbi][:],
                                gu_sb[:, k, it * 128:(it + 1) * 128],
                                dwt[:, fbi * 512:(fbi + 1) * 512],
                                start=(k == 0), stop=(k == FT_FF - 1))
                for it in range(NT):
                    for fbi in range(2):
                        fb = fbp * 2 + fbi
                        xr = spool.tile([128, 512], F32, name="xr_d")
                        nc.gpsimd.dma_start(
                            out=xr[:],
                            in_=x1_d[it][:, fb * 512:(fb + 1) * 512])
                        osb = spool.tile([128, 512], F32, name="osb_d")
                        nc.vector.tensor_tensor(
                            osb[:], dps_t[it][fbi][:], xr[:], ALU.add)
                        nc.sync.dma_start(
                            out=out_d[it * 128:(it + 1) * 128,
                                      fb * 512:(fb + 1) * 512],
                            in_=osb[:])

        gu_cm.__exit__(None, None, None)
        h2T_cm.__exit__(None, None, None)
        wflow_cm.__exit__(None, None, None)


def core_token_idx(c):
    """Global token indices (within the batch row) owned by group-core c."""
    return np.concatenate([np.arange(g * 128, (g + 1) * 128)
                           for g in snake_tiles(c)])


def prepare_inputs(x, f_cos, f_sin, w_attn, w_proj, w_gate, w_up, w_down, g1, g2):
    """Host-side sharding + weight re-layout. Returns list of 8 input dicts."""
    x = np.asarray(x, dtype=np.float32)
    f_cos = np.asarray(f_cos, dtype=np.float32)
    f_sin = np.asarray(f_sin, dtype=np.float32)
    w_attn = np.asarray(w_attn, dtype=np.float32)
    g1 = np.asarray(g1, dtype=np.float32)
    g2 = np.asarray(g2, dtype=np.float32)

    perm = _rope_perm()
    wq = w_attn[0:D] * g1[None, :]
    wk = w_attn[D:2 * D] * g1[None, :]
    wv = w_attn[2 * D:3 * D] * g1[None, :]
    # permute rows within each head for q and k
    wq_p = wq.reshape(H, DK, D)[:, perm, :].reshape(H * DK, D)
    wk_p = wk.reshape(H, DK, D)[:, perm, :].reshape(H * DK, D)

    def lhsT_layout(w):  # w: [F, D] -> [F/128, 128(d within k-tile), D(k*128+c)]
        f = w.shape[0]
        # out[ft, p, k*128+c] = w[ft*128+c, k*128+p]
        a = w.reshape(f // 128, 128, KT, 128)       # [ft, c, k, p]
        a = a.transpose(0, 3, 2, 1).reshape(f // 128, 128, D)  # [ft, p, (k c)]
        return np.ascontiguousarray(a).astype(ml_dtypes.bfloat16)

    def rhsT_layout(w):  # w: [F, D_in] -> [D_in/128, 128(p), F] = w.T tiled
        d_in = w.shape[1]
        a = w.T.reshape(d_in // 128, 128, w.shape[0])  # [k, p, c]
        return np.ascontiguousarray(a).astype(ml_dtypes.bfloat16)

    qk_w = np.concatenate([lhsT_layout(wq_p), lhsT_layout(wk_p)], axis=0)
    v_w = rhsT_layout(wv)
    proj_w = rhsT_layout(np.asarray(w_proj, dtype=np.float32))
    gate_w = lhsT_layout(np.asarray(w_gate, dtype=np.float32) * g2[None, :])
    up_w = lhsT_layout(np.asarray(w_up, dtype=np.float32) * g2[None, :])
    down_w = rhsT_layout(np.asarray(w_down, dtype=np.float32))

    # cs1/cs2 in permuted-lane layout: [128, T]
    pair = np.zeros(DK, dtype=np.int64)
    sign = np.zeros(DK, dtype=np.float32)
    for p in range(DK):
        qd, j = p // 32, p % 32
        pair[p] = 16 * qd + (j if j < 16 else j - 16)
        sign[p] = -1.0 if j < 16 else 1.0
    cs1_full = f_cos.T[pair, :]                       # [128, T]
    cs2_full = f_sin.T[pair, :] * sign[:, None]       # [128, T]

    in_maps = []
    for core in range(N_CORES):
        b, c = core // CORES_PER_B, core % CORES_PER_B
        tok = core_token_idx(c)
        tiles = snake_tiles(c)
        # causal mask tiles: kt covers query tile l=kt//4 (this core's
        # global tile tiles[l]); allowed iff key_pos <= query_pos
        tri = np.zeros((NKT, 128, 128), dtype=np.float32)
        kk = np.arange(128)[:, None]
        qq = np.arange(128)[None, :]
        for kt in range(NKT):
            g = tiles[kt // 4]
            tri[kt] = (kt * 128 + kk) <= (g * 128 + qq)
        tri = np.ascontiguousarray(
            tri.transpose(1, 0, 2).reshape(128, NKT * 128))
        in_maps.append({
            "x": np.ascontiguousarray(x[b, tok, :]),
            "qk_w": qk_w, "v_w": v_w, "proj_w": proj_w,
            "gate_w": gate_w, "up_w": up_w, "down_w": down_w,
            "cs1": np.ascontiguousarray(cs1_full[:, tok]),
            "cs2": np.ascontiguousarray(cs2_full[:, tok]),
            "tri": tri.astype(ml_dtypes.bfloat16),
        })
    return in_maps


def assemble_output(results):
    out = np.zeros((B, T, D), dtype=np.float32)
    for core in range(N_CORES):
        b, c = core // CORES_PER_B, core % CORES_PER_B
        out[b, core_token_idx(c), :] = results[core]["out"]
    return out


_CACHE = {}
_LOCK = threading.Lock()


def get_program():
    with _LOCK:
        if "nc" not in _CACHE:
            _CACHE["nc"] = build_program()
        return _CACHE["nc"]


def kernel(**inputs):
    nc = get_program()
    in_maps = prepare_inputs(**inputs)
    res = run_bass_kernel_spmd(nc, in_maps, list(range(N_CORES)))
    return assemble_output(res.results)


def bench(inputs, iters=10):
    """Wall-clock the sharded executable with device-resident inputs.

    Returns the mean pipelined per-call time in ns (upper bound on HW exec
    time: it includes 1/iters of the axon dispatch round-trip)."""
    import jax
    from jax.sharding import Mesh, PartitionSpec, NamedSharding
    from jax.experimental.shard_map import shard_map
    from concourse import bass2jax, mybir as mb

    nc = get_program()
    in_maps = prepare_inputs(**inputs)
    bass2jax.install_neuronx_cc_hook()

    partition_name = (nc.partition_id_tensor.name
                      if nc.partition_id_tensor else None)
    in_names, out_names, out_avals, zero_outs = [], [], [], []
    for alloc in nc.m.functions[0].allocations:
        if not isinstance(alloc, mb.MemoryLocationSet):
            continue
        name = alloc.memorylocations[0].name
        if alloc.kind == "ExternalInput":
            if name != partition_name:
                in_names.append(name)
        elif alloc.kind == "ExternalOutput":
            shape = tuple(alloc.tensor_shape)
            dtype = mb.dt.np(alloc.dtype)
            out_names.append(name)
            out_avals.append(jax.core.ShapedArray(shape, dtype))
            zero_outs.append(np.zeros(shape, dtype))
    n_params = len(in_names)
    all_in_names = list(in_names) + list(out_names)
    if partition_name is not None:
        all_in_names.append(partition_name)
    donate = tuple(range(n_params, n_params + len(out_names)))

    def _body(*args):
        operands = list(args)
        if partition_name is not None:
            operands.append(bass2jax.partition_id_tensor())
        return tuple(bass2jax._bass_exec_p.bind(
            *operands,
            out_avals=tuple(out_avals),
            in_names=tuple(all_in_names),
            out_names=tuple(out_names),
            lowering_input_output_aliases=(),
            sim_require_finite=True,
            sim_require_nnan=True,
            nc=nc,
        ))

    devices = jax.devices()[:N_CORES]
    mesh = Mesh(np.asarray(devices), ("core",))
    in_specs = (PartitionSpec("core"),) * (n_params + len(out_names))
    out_specs = (PartitionSpec("core"),) * len(out_names)
    sharded = jax.jit(
        shard_map(_body, mesh=mesh, in_specs=in_specs, out_specs=out_specs,
                  check_rep=False),
        donate_argnums=donate, keep_unused=True)

    sh = NamedSharding(mesh, PartitionSpec("core"))
    concat_in = [
        jax.device_put(
            np.concatenate([np.asarray(in_maps[c][nm]) for c in range(N_CORES)],
                           axis=0), sh)
        for nm in in_names]
    jax.block_until_ready(concat_in)

    def make_zeros():
        return [jax.device_put(
            np.zeros((N_CORES * z.shape[0], *z.shape[1:]), z.dtype), sh)
            for z in zero_outs]

    # warmup (compile)
    outs = sharded(*concat_in, *make_zeros())
    jax.block_until_ready(outs)

    zs = [make_zeros() for _ in range(iters)]
    for z in zs:
        jax.block_until_ready(z)
    # async pipelined dispatch amortizes the ~100ms axon round-trip
    t0 = time.perf_counter()
    outs = [sharded(*concat_in, *zs[i]) for i in range(iters)]
    jax.block_until_ready(outs)
    dt = (time.perf_counter() - t0) / iters
    return dt * 1e9


# revision 18
# speedup vs baseline: 1.8924x; 1.0124x over previous
"""Trainium2 Bass kernel for a dense transformer block.

Reference computation (B=2, T=2048, D=2048, H=16, Dk=128, FF=8192, fp32):
    h   = rmsnorm(x, g1)
    qkv = h @ w_attn.T ; q,k = rope(q,k) ; y = causal_softmax(q k^T / sqrt(Dk)) v
    x1  = x + y @ w_proj.T
    h2  = rmsnorm(x1, g2)
    out = x1 + (silu(h2 @ w_gate.T) * (h2 @ w_up.T)) @ w_down.T

Distribution: data-parallel over tokens, 512 per core (cores 0-3: batch 0,
cores 4-7: batch 1). Token tiles are "snake"-folded across the 4-core group:
core c owns global 128-token tiles {c, 7-c, 8+c, 15-c}, so every core's
causal key footprint is identical (tiles 0..3 attend 4 key tiles, 4..7
attend 8, 8..11 attend 12, 12..15 attend 16 -> 62.5% of the dense score/AV
work, perfectly balanced). Causal masking within the padded footprint is
data-driven (per-core 0/1 mask tiles multiply the exp'd scores), which keeps
the SPMD program identical on all cores. K,V are computed locally and moved
in ONE merged AllGather per 4-core group (collective launches carry ~1 ms of
per-call runtime cost, so fewer is better); the Q pass and the first
attention heads overlap the gather transfer.

All weight matrices stream through one shared SBUF pool, so the DMA queue
naturally prefetches the next phase's weights while the current phase
computes. Matmuls run in bf16 with fp32 PSUM accumulation. Residuals and
normalization in fp32. RoPE is applied in the transposed [dk, t] layout via
a host-side permutation of the head dimension + DVE stream_shuffle.
"""

import os
import sys
import threading
import time

import numpy as np

for _p in ("/opt/trn_rl_repo", os.path.expanduser("~/.axon_site/_ro/trn_rl_repo")):
    if _p not in sys.path and os.path.isdir(_p):
        sys.path.append(_p)

import ml_dtypes  # noqa: E402

import concourse.bass as bass  # noqa: E402
import concourse.mybir as mybir  # noqa: E402
import concourse.tile as tile  # noqa: E402
from concourse import bacc  # noqa: E402
from concourse.bass_utils import run_bass_kernel_spmd  # noqa: E402
from concourse.masks import make_identity  # noqa: E402
from contextlib import ExitStack  # noqa: E402

F32 = mybir.dt.float32
BF16 = mybir.dt.bfloat16
AF = mybir.ActivationFunctionType
ALU = mybir.AluOpType

B, T, D = 2, 2048, 2048
H, DK, FF = 16, 128, 8192
EPS = 1e-6
N_CORES = 8
TLOC = T * B // N_CORES          # 512 tokens per core
CORES_PER_B = N_CORES // B       # 4
KT = D // 128                    # 16 d-tiles
NT = TLOC // 128                 # 4 t-tiles per core
NKT = T // 128                   # 16 key subtiles (full sequence)
FT_FF = FF // 128                # 64 ff tiles
NFB = D // 512                   # 4 v/proj 512-col blocks
SCALE = 1.0 / float(np.sqrt(DK))
SHUF_MASK = [(j + 16) % 32 for j in range(32)]


def snake_tiles(c):
    """Global 128-token tile indices owned by group-core c, local order."""
    return [c, 7 - c, 8 + c, 15 - c]


def _gmaps():
    """global tile g -> (owning group-core, local tile index)."""
    rmap, lmap = [0] * NKT, [0] * NKT
    for g in range(NKT):
        for r in range(CORES_PER_B):
            if g in snake_tiles(r):
                rmap[g], lmap[g] = r, snake_tiles(r).index(g)
    return rmap, lmap


RMAP, LMAP = _gmaps()
# core-major position of global tile g inside gathered K/V SBUF tiles
POS = [RMAP[g] * NT + LMAP[g] for g in range(NKT)]


def _rope_perm():
    """Within-head row permutation: pair i=(16*qd + j) real part -> partition
    32*qd + j, imag part -> partition 32*qd + 16 + j."""
    perm = np.zeros(DK, dtype=np.int64)
    for p in range(DK):
        qd, j = p // 32, p % 32
        i = 16 * qd + (j if j < 16 else j - 16)
        perm[p] = 2 * i + (0 if j < 16 else 1)
    return perm


def build_program(sim=False, repeat=1):
    nc = bacc.Bacc("TRN2", target_bir_lowering=False, debug=False,
                   num_devices=1 if sim else N_CORES)

    x_d = nc.declare_dram_parameter("x", [TLOC, D], F32, isOutput=False)
    qkw_d = nc.declare_dram_parameter("qk_w", [2 * H, 128, D], BF16, isOutput=False)
    vw_d = nc.declare_dram_parameter("v_w", [KT, 128, D], BF16, isOutput=False)
    pw_d = nc.declare_dram_parameter("proj_w", [H, 128, D], BF16, isOutput=False)
    gw_d = nc.declare_dram_parameter("gate_w", [FT_FF, 128, D], BF16, isOutput=False)
    uw_d = nc.declare_dram_parameter("up_w", [FT_FF, 128, D], BF16, isOutput=False)
    dw_d = nc.declare_dram_parameter("down_w", [FT_FF, 128, D], BF16, isOutput=False)
    cs1_d = nc.declare_dram_parameter("cs1", [128, TLOC], F32, isOutput=False)
    cs2_d = nc.declare_dram_parameter("cs2", [128, TLOC], F32, isOutput=False)
    tri_d = nc.declare_dram_parameter("tri", [128, NKT * 128], BF16,
                                      isOutput=False)
    out_d = nc.declare_dram_parameter("out", [TLOC, D], F32, isOutput=True)

    with ExitStack() as ctx:
        tc = ctx.enter_context(tile.TileContext(nc))
        for _rep in range(repeat):
            _emit_block(nc, tc, sim, x_d, qkw_d, vw_d, pw_d, gw_d, uw_d, dw_d,
                        cs1_d, cs2_d, tri_d, out_d)

    nc.compile()
    return nc


def _emit_block(nc, tc, sim, x_d, qkw_d, vw_d, pw_d, gw_d, uw_d, dw_d,
                cs1_d, cs2_d, tri_d, out_d):
    with ExitStack() as ctx:
        const = ctx.enter_context(tc.tile_pool(name="const", bufs=1))
        ident = const.tile([128, 128], BF16)
        make_identity(nc, ident)
        ones_col = const.tile([128, 1], BF16)
        nc.vector.memset(ones_col, 1.0)
        ones_row = const.tile([1, 128], F32)
        nc.vector.memset(ones_row, 1.0)
        cs1_sb = const.tile([128, TLOC], F32)
        nc.sync.dma_start(out=cs1_sb[:], in_=cs1_d[:, :])
        cs2_sb = const.tile([128, TLOC], F32)
        nc.sync.dma_start(out=cs2_sb[:], in_=cs2_d[:, :])
        trib_sb = const.tile([128, NKT, 128], BF16)
        nc.gpsimd.dma_start(out=trib_sb[:], in_=tri_d.rearrange(
            "p (n q) -> p n q", n=NKT))

        # shared streaming pool for ALL weight tiles: one rotation across
        # phases lets the DMA queue prefetch phase N+1's weights during
        # phase N's compute (slot = 4KB/partition).
        wflow_cm = tc.tile_pool(name="wflow", bufs=5)
        wflow = wflow_cm.__enter__()

        # DRAM scratch: K/V allgather buffers + x1 spill
        dram = ctx.enter_context(tc.tile_pool(name="dram", bufs=1, space="DRAM"))
        # merged K+V allgather payload: entries 0..H-1 = K heads,
        # H + fb*NT + l = V block (fb, local tile l). One collective per
        # call (collective launches carry ~1ms/call fixed runtime cost).
        kv_local = dram.tile([2 * H, 128, TLOC], BF16)
        kv_full = dram.tile([CORES_PER_B, 2 * H, 128, TLOC], BF16)
        x1_d = dram.tile([NT, 128, D], F32)

        def rmsnorm_transpose(loader, dst_sb, pool, psum_pool):
            """loader(it) -> [128, D] fp32 AP; writes dst_sb [128, KT, TLOC]
            bf16 = (rms-normalized rows) transposed. Gains folded in weights."""
            rstds = []
            for it in range(NT):
                sq_scr = pool.tile([128, D], BF16, name="sq_scr")
                ssq = pool.tile([128, 1], F32, name=f"ssq{it}", tag=f"ssq{it}",
                                bufs=1)
                nc.scalar.activation(sq_scr[:], loader(it), AF.Square,
                                     accum_out=ssq[:])
                mean = pool.tile([128, 1], F32, name="mean")
                nc.vector.tensor_scalar(mean[:], ssq[:], 1.0 / D, EPS,
                                        ALU.mult, ALU.add)
                rec = pool.tile([128, 1], F32, name="rec")
                nc.vector.reciprocal(rec[:], mean[:])
                rstd = pool.tile([128, 1], F32, name=f"rstd{it}",
                                 tag=f"rstd{it}", bufs=1)
                nc.scalar.activation(rstd[:], rec[:], AF.Sqrt)
                rstds.append(rstd)
            hrows = []
            for it in range(NT):
                hrow = pool.tile([128, D], BF16, name=f"hrow{it}",
                                 tag=f"hrow{it}", bufs=1)
                nc.vector.tensor_scalar(hrow[:], loader(it), rstds[it][:],
                                        None, ALU.mult)
                hrows.append(hrow)
            # k-outer so dst_sb[:, k, :] completes early for the consumers
            for k in range(KT):
                for it in range(NT):
                    tp = psum_pool.tile([128, 128], BF16, name="tp")
                    nc.tensor.transpose(tp[:], hrows[it][:, k * 128:(k + 1) * 128],
                                        ident[:])
                    nc.vector.tensor_copy(dst_sb[:, k, it * 128:(it + 1) * 128],
                                          tp[:])

        def rope_evict(ps, dst, pool):
            """ps: [128, TLOC] psum q/k head tile (permuted lanes) -> rotated"""
            sh = pool.tile([128, TLOC], F32, name="rp_sh")
            nc.vector.stream_shuffle(sh[:], ps[:], mask=SHUF_MASK)
            t1 = pool.tile([128, TLOC], F32, name="rp_t1")
            nc.vector.tensor_tensor(t1[:], ps[:], cs1_sb[:], ALU.mult)
            t2 = pool.tile([128, TLOC], F32, name="rp_t2")
            nc.vector.tensor_tensor(t2[:], sh[:], cs2_sb[:], ALU.mult)
            nc.vector.tensor_tensor(dst[:], t1[:], t2[:], ALU.add)

        # persistent pools, strict LIFO
        qrot_cm = tc.tile_pool(name="qrot_pool", bufs=1)
        qrot_pool = qrot_cm.__enter__()
        qrot_sb = qrot_pool.tile([128, H, TLOC], BF16)
        hT_cm = tc.tile_pool(name="hT_pool", bufs=1)
        hT_pool = hT_cm.__enter__()
        hT_sb = hT_pool.tile([128, KT, TLOC], BF16)

        # ---------------- phase 1: norm1 + h^T ----------------
        x_cm = tc.tile_pool(name="xpool", bufs=1)
        xpool = x_cm.__enter__()
        x_sb = xpool.tile([128, NT, D], F32)
        for it in range(NT):
            eng = nc.sync if it % 2 == 0 else nc.gpsimd
            eng.dma_start(out=x_sb[:, it, :],
                          in_=x_d[it * 128:(it + 1) * 128, :])
        # prefetch the first K-head weight tiles behind the x loads
        qk_wts = {}
        for h in range(2):
            wt = wflow.tile([128, KT, 128], BF16, name="qk_wt")
            nc.sync.dma_start(out=wt[:], in_=qkw_d[H + h].rearrange(
                "p (k c) -> p k c", k=KT))
            qk_wts[h] = wt
        with ExitStack() as ph:
            pool = ph.enter_context(tc.tile_pool(name="n1_pool", bufs=2))
            psum_pool = ph.enter_context(
                tc.tile_pool(name="n1_psum", bufs=4, space="PSUM"))
            rmsnorm_transpose(lambda it: x_sb[:, it, :], hT_sb, pool, psum_pool)
        x_cm.__exit__(None, None, None)

        # ---------------- phase 2a: K heads + allgather ----------------
        with ExitStack() as ph:
            spool = ph.enter_context(tc.tile_pool(name="k_s", bufs=3))
            pspool = ph.enter_context(
                tc.tile_pool(name="k_ps", bufs=3, space="PSUM"))
            for h in range(H):
                if h in qk_wts:
                    wt = qk_wts.pop(h)
                else:
                    wt = wflow.tile([128, KT, 128], BF16, name="qk_wt")
                    nc.sync.dma_start(out=wt[:], in_=qkw_d[H + h].rearrange(
                        "p (k c) -> p k c", k=KT))
                ps = pspool.tile([128, TLOC], F32, name="qk_ps")
                for k in range(KT):
                    nc.tensor.matmul(ps[:], wt[:, k, :], hT_sb[:, k, :],
                                     start=(k == 0), stop=(k == KT - 1))
                krot = spool.tile([128, TLOC], BF16, name="krot")
                rope_evict(ps, krot[:], spool)
                nc.sync.dma_start(out=kv_local[h], in_=krot[:])

        kt_cm = tc.tile_pool(name="ktpool", bufs=3)
        ktpool = kt_cm.__enter__()

        # V columns for attention rotate per 512-col block (2 resident:
        # heads 4fb..4fb+3 consume block fb while fb+1 streams in)
        vall_cm = tc.tile_pool(name="vall_pool", bufs=2)
        vap = vall_cm.__enter__()
        v_fbs = []

        # ---------------- phase 2b: V + allgather ----------------
        with ExitStack() as ph:
            spool = ph.enter_context(tc.tile_pool(name="v_s", bufs=3))
            vpspool = ph.enter_context(
                tc.tile_pool(name="v_psp", bufs=1, space="PSUM"))
            for fb in range(NFB):
                vps = [vpspool.tile([128, 512], F32, name=f"v_ps{it}",
                                    tag=f"v_ps{it}") for it in range(NT)]
                for k in range(KT):
                    vwt = wflow.tile([128, 512], BF16, name="vwt")
                    nc.sync.dma_start(out=vwt[:],
                                      in_=vw_d[k][:, fb * 512:(fb + 1) * 512])
                    for it in range(NT):
                        nc.tensor.matmul(vps[it][:],
                                         hT_sb[:, k, it * 128:(it + 1) * 128],
                                         vwt[:], start=(k == 0), stop=(k == KT - 1))
                for it in range(NT):
                    vsb = spool.tile([128, 512], BF16, name="vsb")
                    nc.scalar.copy(vsb[:], vps[it][:])
                    nc.sync.dma_start(out=kv_local[H + fb * NT + it],
                                      in_=vsb[:])
            if sim:
                for r in range(CORES_PER_B):
                    nc.gpsimd.dma_start(out=kv_full[r], in_=kv_local[:])
            else:
                nc.gpsimd.collective_compute(
                    "AllGather", ALU.bypass,
                    replica_groups=[[0, 1, 2, 3], [4, 5, 6, 7]],
                    ins=[kv_local.opt()], outs=[kv_full.opt()],
                )
            for fb in range(NFB):
                v_fb = vap.tile([128, NKT, 512], BF16, name="v_fb")
                for r in range(CORES_PER_B):
                    nc.gpsimd.dma_start(
                        out=v_fb[:, r * NT:(r + 1) * NT, :],
                        in_=kv_full[r, H + fb * NT:H + fb * NT + NT].rearrange(
                            "l p c -> p l c"))
                v_fbs.append(v_fb)

        # stage the first heads' K columns (gather overlaps the Q pass)
        kT_tiles = {}
        for h in range(2):
            kT_sb = ktpool.tile([128, T], BF16, name="kT_sb")
            for r in range(CORES_PER_B):
                nc.gpsimd.dma_start(
                    out=kT_sb[:, r * TLOC:(r + 1) * TLOC],
                    in_=kv_full[r, h])
            kT_tiles[h] = kT_sb

        # ---------------- phase 2c: Q heads + rope (KV gather overlaps) --
        with ExitStack() as ph:
            qwpool = ph.enter_context(tc.tile_pool(name="q_w", bufs=3))
            spool = ph.enter_context(tc.tile_pool(name="q_s", bufs=3))
            pspool = ph.enter_context(
                tc.tile_pool(name="q_ps", bufs=3, space="PSUM"))
            for h in range(H):
                wt = qwpool.tile([128, KT, 128], BF16, name="q_wt")
                nc.sync.dma_start(out=wt[:], in_=qkw_d[h].rearrange(
                    "p (k c) -> p k c", k=KT))
                ps = pspool.tile([128, TLOC], F32, name="qk_ps")
                for k in range(KT):
                    nc.tensor.matmul(ps[:], wt[:, k, :], hT_sb[:, k, :],
                                     start=(k == 0), stop=(k == KT - 1))
                rope_evict(ps, qrot_sb[:, h, :], spool)

        y_cm = tc.tile_pool(name="y_pool", bufs=1)
        y_pool = y_cm.__enter__()
        y_sb = y_pool.tile([128, H, TLOC], BF16)

        # ---------------- phase 3: attention (snake-folded causal) -------
        # kt block l=kt//4 covers local query cols [l*128:512); the first
        # 128 cols get the data-driven causal mask, the rest are always
        # fully allowed by construction of the snake fold.
        with ExitStack() as ph:
            apool = ph.enter_context(tc.tile_pool(name="att_pool", bufs=2))
            epool = ph.enter_context(tc.tile_pool(name="exp_pool", bufs=6))
            aps = ph.enter_context(tc.tile_pool(name="att_ps", bufs=2, space="PSUM"))
            sps_pool = ph.enter_context(
                tc.tile_pool(name="sps_pool", bufs=3, space="PSUM"))
            bps_pool = ph.enter_context(
                tc.tile_pool(name="bps_pool", bufs=1, space="PSUM"))

            for h in range(H):
                if h in kT_tiles:
                    kT_sb = kT_tiles.pop(h)
                else:
                    kT_sb = ktpool.tile([128, T], BF16, name="kT_sb")
                    for r in range(CORES_PER_B):
                        nc.gpsimd.dma_start(
                            out=kT_sb[:, r * TLOC:(r + 1) * TLOC],
                            in_=kv_full[r, h])
                yps = aps.tile([128, TLOC], F32, name="y_ps", tag="y_ps")
                sums = aps.tile([1, TLOC], F32, name="sums_ps", tag="sums_ps")
                for kt in range(NKT):
                    c0 = (kt // 4) * 128
                    w = TLOC - c0
                    kp = POS[kt]
                    sps = sps_pool.tile([128, TLOC], F32, name="s_ps",
                                        tag="s_ps")
                    nc.tensor.matmul(sps[:, :w], kT_sb[:, kp * 128:(kp + 1) * 128],
                                     qrot_sb[:, h, c0:TLOC], start=True,
                                     stop=True)
                    em = epool.tile([128, TLOC], BF16, name="em")
                    nc.scalar.activation(em[:, :w], sps[:, :w], AF.Exp,
                                         scale=SCALE)
                    nc.vector.tensor_tensor(em[:, 0:128], em[:, 0:128],
                                            trib_sb[:, kt, :], ALU.mult)
                    nc.tensor.matmul(yps[:, c0:TLOC],
                                     v_fbs[h // 4][:, kp,
                                                   (h % 4) * 128:
                                                   (h % 4 + 1) * 128],
                                     em[:, :w], start=(kt == 0),
                                     stop=(kt == NKT - 1),
                                     skip_group_check=True)
                    nc.tensor.matmul(sums[:, c0:TLOC], ones_col[:], em[:, :w],
                                     start=(kt == 0), stop=(kt == NKT - 1),
                                     skip_group_check=True)
                rec = apool.tile([1, TLOC], F32, name="rec_att")
                nc.vector.reciprocal(rec[:], sums[:])
                bps = bps_pool.tile([128, TLOC], F32, name="b_ps", tag="b_ps")
                nc.tensor.matmul(bps[:], ones_row[:], rec[:], start=True,
                                 stop=True)
                bsb = apool.tile([128, TLOC], F32, name="bsb")
                nc.vector.tensor_copy(bsb[:], bps[:])
                nc.vector.tensor_tensor(y_sb[:, h, :], yps[:], bsb[:], ALU.mult)

        # ------- phase 4: proj + residual -> x1 (DRAM spill) -------------
        with ExitStack() as ph:
            spool = ph.enter_context(tc.tile_pool(name="pj_s", bufs=4))
            pps = ph.enter_context(tc.tile_pool(name="pj_ps", bufs=2, space="PSUM"))
            for fb in range(NFB):
                pps_t = [pps.tile([128, 512], F32, name=f"p_ps{it}",
                                  tag=f"p_ps{it}") for it in range(NT)]
                for hd in range(H):
                    pwt = wflow.tile([128, 512], BF16, name="pwt")
                    nc.sync.dma_start(out=pwt[:],
                                      in_=pw_d[hd][:, fb * 512:(fb + 1) * 512])
                    for it in range(NT):
                        nc.tensor.matmul(pps_t[it][:],
                                         y_sb[:, hd, it * 128:(it + 1) * 128],
                                         pwt[:], start=(hd == 0),
                                         stop=(hd == H - 1))
                for it in range(NT):
                    xr = spool.tile([128, 512], F32, name="xr_p")
                    nc.sync.dma_start(
                        out=xr[:],
                        in_=x_d[it * 128:(it + 1) * 128,
                                fb * 512:(fb + 1) * 512])
                    x1t = spool.tile([128, 512], F32, name="x1t")
                    nc.vector.tensor_tensor(x1t[:], pps_t[it][:], xr[:],
                                            ALU.add)
                    nc.sync.dma_start(
                        out=x1_d[it][:, fb * 512:(fb + 1) * 512], in_=x1t[:])

        y_cm.__exit__(None, None, None)
        vall_cm.__exit__(None, None, None)
        kt_cm.__exit__(None, None, None)
        hT_cm.__exit__(None, None, None)
        qrot_cm.__exit__(None, None, None)

        # ---------------- phase 5: norm2 + h2^T ----------------
        h2T_cm = tc.tile_pool(name="h2T_pool", bufs=1)
        h2T_pool = h2T_cm.__enter__()
        h2T_sb = h2T_pool.tile([128, KT, TLOC], BF16)
        with ExitStack() as ph:
            x1p = ph.enter_context(tc.tile_pool(name="x1r_pool", bufs=1))
            x1rows = []
            for it in range(NT):
                x1r = x1p.tile([128, D], F32, name=f"x1r{it}", tag=f"x1r{it}")
                eng = nc.sync if it % 2 == 0 else nc.gpsimd
                eng.dma_start(out=x1r[:], in_=x1_d[it])
                x1rows.append(x1r)
            pool = ph.enter_context(tc.tile_pool(name="n2_pool", bufs=2))
            psum_pool = ph.enter_context(
                tc.tile_pool(name="n2_psum", bufs=4, space="PSUM"))
            rmsnorm_transpose(lambda it: x1rows[it][:], h2T_sb, pool, psum_pool)

        # ---------------- phase 6: gate/up ----------------
        gu_cm = tc.tile_pool(name="gu_pool", bufs=1)
        gu_pool = gu_cm.__enter__()
        gu_sb = gu_pool.tile([128, FT_FF, TLOC], BF16)
        with ExitStack() as ph:
            spool = ph.enter_context(tc.tile_pool(name="mlp_s", bufs=3))
            mps = ph.enter_context(tc.tile_pool(name="mlp_ps", bufs=4, space="PSUM"))
            for f in range(FT_FF):
                gwt = wflow.tile([128, KT, 128], BF16, name="gwt")
                nc.sync.dma_start(out=gwt[:], in_=gw_d[f].rearrange(
                    "p (k c) -> p k c", k=KT))
                gps = mps.tile([128, TLOC], F32, name="g_ps", tag="g_ps")
                for k in range(KT):
                    nc.tensor.matmul(gps[:], gwt[:, k, :], h2T_sb[:, k, :],
                                     start=(k == 0), stop=(k == KT - 1))
                gsil = spool.tile([128, TLOC], BF16, name="gsil")
                nc.scalar.activation(gsil[:], gps[:], AF.Silu)
                uwt = wflow.tile([128, KT, 128], BF16, name="uwt")
                nc.sync.dma_start(out=uwt[:], in_=uw_d[f].rearrange(
                    "p (k c) -> p k c", k=KT))
                ups = mps.tile([128, TLOC], F32, name="u_ps", tag="u_ps")
                for k in range(KT):
                    nc.tensor.matmul(ups[:], uwt[:, k, :], h2T_sb[:, k, :],
                                     start=(k == 0), stop=(k == KT - 1))
                nc.vector.tensor_tensor(gu_sb[:, f, :], ups[:], gsil[:],
                                        ALU.mult)

        # ---------------- phase 7: down + residual -> out ----------------
        with ExitStack() as ph:
            spool = ph.enter_context(tc.tile_pool(name="dn_s", bufs=8))
            dps = ph.enter_context(tc.tile_pool(name="dn_ps", bufs=1, space="PSUM"))
            for fbp in range(2):
                dps_t = [[dps.tile([128, 512], F32, name=f"d_ps{it}_{fbi}",
                                   tag=f"d_ps{it}_{fbi}") for fbi in range(2)]
                         for it in range(NT)]
                for k in range(FT_FF):
                    dwt = wflow.tile([128, 1024], BF16, name="dwt")
                    nc.sync.dma_start(
                        out=dwt[:],
                        in_=dw_d[k][:, fbp * 1024:(fbp + 1) * 1024])
                    for it in range(NT):
                        for fbi in range(2):
                            nc.tensor.matmul(
                                dps_t[it][fbi][:],
                                gu_sb[:, k, it * 128:(it + 1) * 128],
                                dwt[:, fbi * 512:(fbi + 1) * 512],
                                start=(k == 0), stop=(k == FT_FF - 1))
                for it in range(NT):
                    for fbi in range(2):
                        fb = fbp * 2 + fbi
                        xr = spool.tile([128, 512], F32, name="xr_d")
                        nc.gpsimd.dma_start(
                            out=xr[:],
                            in_=x1_d[it][:, fb * 512:(fb + 1) * 512])
                        osb = spool.tile([128, 512], F32, name="osb_d")
                        nc.vector.tensor_tensor(
                            osb[:], dps_t[it][fbi][:], xr[:], ALU.add)
                        nc.sync.dma_start(
                            out=out_d[it * 128:(it + 1) * 128,
                                      fb * 512:(fb + 1) * 512],
                            in_=osb[:])

        gu_cm.__exit__(None, None, None)
        h2T_cm.__exit__(None, None, None)
        wflow_cm.__exit__(None, None, None)


def core_token_idx(c):
    """Global token indices (within the batch row) owned by group-core c."""
    return np.concatenate([np.arange(g * 128, (g + 1) * 128)
                           for g in snake_tiles(c)])


def prepare_inputs(x, f_cos, f_sin, w_attn, w_proj, w_gate, w_up, w_down, g1, g2):
    """Host-side sharding + weight re-layout. Returns list of 8 input dicts."""
    x = np.asarray(x, dtype=np.float32)
    f_cos = np.asarray(f_cos, dtype=np.float32)
    f_sin = np.asarray(f_sin, dtype=np.float32)
    w_attn = np.asarray(w_attn, dtype=np.float32)
    g1 = np.asarray(g1, dtype=np.float32)
    g2 = np.asarray(g2, dtype=np.float32)

    perm = _rope_perm()
    wq = w_attn[0:D] * g1[None, :]
    wk = w_attn[D:2 * D] * g1[None, :]
    wv = w_attn[2 * D:3 * D] * g1[None, :]
    # permute rows within each head for q and k
    wq_p = wq.reshape(H, DK, D)[:, perm, :].reshape(H * DK, D)
    wk_p = wk.reshape(H, DK, D)[:, perm, :].reshape(H * DK, D)

    def lhsT_layout(w):  # w: [F, D] -> [F/128, 128(d within k-tile), D(k*128+c)]
        f = w.shape[0]
        # out[ft, p, k*128+c] = w[ft*128+c, k*128+p]
        a = w.reshape(f // 128, 128, KT, 128)       # [ft, c, k, p]
        a = a.transpose(0, 3, 2, 1).reshape(f // 128, 128, D)  # [ft, p, (k c)]
        return np.ascontiguousarray(a).astype(ml_dtypes.bfloat16)

    def rhsT_layout(w):  # w: [F, D_in] -> [D_in/128, 128(p), F] = w.T tiled
        d_in = w.shape[1]
        a = w.T.reshape(d_in // 128, 128, w.shape[0])  # [k, p, c]
        return np.ascontiguousarray(a).astype(ml_dtypes.bfloat16)

    qk_w = np.concatenate([lhsT_layout(wq_p), lhsT_layout(wk_p)], axis=0)
    v_w = rhsT_layout(wv)
    proj_w = rhsT_layout(np.asarray(w_proj, dtype=np.float32))
    gate_w = lhsT_layout(np.asarray(w_gate, dtype=np.float32) * g2[None, :])
    up_w = lhsT_layout(np.asarray(w_up, dtype=np.float32) * g2[None, :])
    down_w = rhsT_layout(np.asarray(w_down, dtype=np.float32))

    # cs1/cs2 in permuted-lane layout: [128, T]
    pair = np.zeros(DK, dtype=np.int64)
    sign = np.zeros(DK, dtype=np.float32)
    for p in range(DK):
        qd, j = p // 32, p % 32
        pair[p] = 16 * qd + (j if j < 16 else j - 16)
        sign[p] = -1.0 if j < 16 else 1.0
    cs1_full = f_cos.T[pair, :]                       # [128, T]
    cs2_full = f_sin.T[pair, :] * sign[:, None]       # [128, T]

    in_maps = []
    for core in range(N_CORES):
        b, c = core // CORES_PER_B, core % CORES_PER_B
        tok = core_token_idx(c)
        tiles = snake_tiles(c)
        # causal mask tiles: kt covers query tile l=kt//4 (this core's
        # global tile tiles[l]); allowed iff key_pos <= query_pos
        tri = np.zeros((NKT, 128, 128), dtype=np.float32)
        kk = np.arange(128)[:, None]
        qq = np.arange(128)[None, :]
        for kt in range(NKT):
            g = tiles[kt // 4]
            tri[kt] = (kt * 128 + kk) <= (g * 128 + qq)
        tri = np.ascontiguousarray(
            tri.transpose(1, 0, 2).reshape(128, NKT * 128))
        in_maps.append({
            "x": np.ascontiguousarray(x[b, tok, :]),
            "qk_w": qk_w, "v_w": v_w, "proj_w": proj_w,
            "gate_w": gate_w, "up_w": up_w, "down_w": down_w,
            "cs1": np.ascontiguousarray(cs1_full[:, tok]),
            "cs2": np.ascontiguousarray(cs2_full[:, tok]),
            "tri": tri.astype(ml_dtypes.bfloat16),
        })
    return in_maps


def assemble_output(results):
    out = np.zeros((B, T, D), dtype=np.float32)
    for core in range(N_CORES):
        b, c = core // CORES_PER_B, core % CORES_PER_B
        out[b, core_token_idx(c), :] = results[core]["out"]
    return out


_CACHE = {}
_LOCK = threading.Lock()


def get_program():
    with _LOCK:
        if "nc" not in _CACHE:
            _CACHE["nc"] = build_program()
        return _CACHE["nc"]


def kernel(**inputs):
    nc = get_program()
    in_maps = prepare_inputs(**inputs)
    res = run_bass_kernel_spmd(nc, in_maps, list(range(N_CORES)))
    return assemble_output(res.results)


def bench(inputs, iters=10):
    """Wall-clock the sharded executable with device-resident inputs.

    Returns the mean pipelined per-call time in ns (upper bound on HW exec
    time: it includes 1/iters of the axon dispatch round-trip)."""
    import jax
    from jax.sharding import Mesh, PartitionSpec, NamedSharding
    from jax.experimental.shard_map import shard_map
    from concourse import bass2jax, mybir as mb

    nc = get_program()
    in_maps = prepare_inputs(**inputs)
    bass2jax.install_neuronx_cc_hook()

    partition_name = (nc.partition_id_tensor.name
                      if nc.partition_id_tensor else None)
    in_names, out_names, out_avals, zero_outs = [], [], [], []
    for alloc in nc.m.functions[0].allocations:
        if not isinstance(alloc, mb.MemoryLocationSet):
            continue
        name = alloc.memorylocations[0].name
        if alloc.kind == "ExternalInput":
            if name != partition_name:
                in_names.append(name)
        elif alloc.kind == "ExternalOutput":
            shape = tuple(alloc.tensor_shape)
            dtype = mb.dt.np(alloc.dtype)
            out_names.append(name)
            out_avals.append(jax.core.ShapedArray(shape, dtype))
            zero_outs.append(np.zeros(shape, dtype))
    n_params = len(in_names)
    all_in_names = list(in_names) + list(out_names)
    if partition_name is not None:
        all_in_names.append(partition_name)
    donate = tuple(range(n_params, n_params + len(out_names)))

    def _body(*args):
        operands = list(args)
        if partition_name is not None:
            operands.append(bass2jax.partition_id_tensor())
        return tuple(bass2jax._bass_exec_p.bind(
            *operands,
            out_avals=tuple(out_avals),
            in_names=tuple(all_in_names),
            out_names=tuple(out_names),
            lowering_input_output_aliases=(),
            sim_require_finite=True,
            sim_require_nnan=True,
            nc=nc,
        ))

    devices = jax.devices()[:N_CORES]
    mesh = Mesh(np.asarray(devices), ("core",))
    in_specs = (PartitionSpec("core"),) * (n_params + len(out_names))
    out_specs = (PartitionSpec("core"),) * len(out_names)
    sharded = jax.jit(
        shard_map(_body, mesh=mesh, in_specs=in_specs, out_specs=out_specs,
                  check_rep=False),
        donate_argnums=donate, keep_unused=True)

    sh = NamedSharding(mesh, PartitionSpec("core"))
    concat_in = [
        jax.device_put(
            np.concatenate([np.asarray(in_maps[c][nm]) for c in range(N_CORES)],
                           axis=0), sh)
        for nm in in_names]
    jax.block_until_ready(concat_in)

    def make_zeros():
        return [jax.device_put(
            np.zeros((N_CORES * z.shape[0], *z.shape[1:]), z.dtype), sh)
            for z in zero_outs]

    # warmup (compile)
    outs = sharded(*concat_in, *make_zeros())
    jax.block_until_ready(outs)

    zs = [make_zeros() for _ in range(iters)]
    for z in zs:
        jax.block_until_ready(z)
    # async pipelined dispatch amortizes the ~100ms axon round-trip
    t0 = time.perf_counter()
    outs = [sharded(*concat_in, *zs[i]) for i in range(iters)]
    jax.block_until_ready(outs)
    dt = (time.perf_counter() - t0) / iters
    return dt * 1e9


# revision 19
# speedup vs baseline: 1.8964x; 1.0021x over previous
"""Trainium2 Bass kernel for a dense transformer block.

Reference computation (B=2, T=2048, D=2048, H=16, Dk=128, FF=8192, fp32):
    h   = rmsnorm(x, g1)
    qkv = h @ w_attn.T ; q,k = rope(q,k) ; y = causal_softmax(q k^T / sqrt(Dk)) v
    x1  = x + y @ w_proj.T
    h2  = rmsnorm(x1, g2)
    out = x1 + (silu(h2 @ w_gate.T) * (h2 @ w_up.T)) @ w_down.T

Distribution: data-parallel over tokens, 512 per core (cores 0-3: batch 0,
cores 4-7: batch 1). Token tiles are "snake"-folded across the 4-core group:
core c owns global 128-token tiles {c, 7-c, 8+c, 15-c}, so every core's
causal key footprint is identical (tiles 0..3 attend 4 key tiles, 4..7
attend 8, 8..11 attend 12, 12..15 attend 16 -> 62.5% of the dense score/AV
work, perfectly balanced). Causal masking within the padded footprint is
data-driven (per-core 0/1 mask tiles multiply the exp'd scores), which keeps
the SPMD program identical on all cores. K,V are computed locally and moved
in ONE merged AllGather per 4-core group (collective launches carry ~1 ms of
per-call runtime cost, so fewer is better); the Q pass and the first
attention heads overlap the gather transfer.

All weight matrices stream through one shared SBUF pool, so the DMA queue
naturally prefetches the next phase's weights while the current phase
computes. Matmuls run in bf16 with fp32 PSUM accumulation. Residuals and
normalization in fp32. RoPE is applied in the transposed [dk, t] layout via
a host-side permutation of the head dimension + DVE stream_shuffle.
"""

import os
import sys
import threading
import time

import numpy as np

for _p in ("/opt/trn_rl_repo", os.path.expanduser("~/.axon_site/_ro/trn_rl_repo")):
    if _p not in sys.path and os.path.isdir(_p):
        sys.path.append(_p)

import ml_dtypes  # noqa: E402

import concourse.bass as bass  # noqa: E402
import concourse.mybir as mybir  # noqa: E402
import concourse.tile as tile  # noqa: E402
from concourse import bacc  # noqa: E402
from concourse.bass_utils import run_bass_kernel_spmd  # noqa: E402
from concourse.masks import make_identity  # noqa: E402
from contextlib import ExitStack  # noqa: E402

F32 = mybir.dt.float32
BF16 = mybir.dt.bfloat16
AF = mybir.ActivationFunctionType
ALU = mybir.AluOpType

B, T, D = 2, 2048, 2048
H, DK, FF = 16, 128, 8192
EPS = 1e-6
N_CORES = 8
TLOC = T * B // N_CORES          # 512 tokens per core
CORES_PER_B = N_CORES // B       # 4
KT = D // 128                    # 16 d-tiles
NT = TLOC // 128                 # 4 t-tiles per core
NKT = T // 128                   # 16 key subtiles (full sequence)
FT_FF = FF // 128                # 64 ff tiles
NFB = D // 512                   # 4 v/proj 512-col blocks
SCALE = 1.0 / float(np.sqrt(DK))
SHUF_MASK = [(j + 16) % 32 for j in range(32)]


def snake_tiles(c):
    """Global 128-token tile indices owned by group-core c, local order."""
    return [c, 7 - c, 8 + c, 15 - c]


def _gmaps():
    """global tile g -> (owning group-core, local tile index)."""
    rmap, lmap = [0] * NKT, [0] * NKT
    for g in range(NKT):
        for r in range(CORES_PER_B):
            if g in snake_tiles(r):
                rmap[g], lmap[g] = r, snake_tiles(r).index(g)
    return rmap, lmap


RMAP, LMAP = _gmaps()
# core-major position of global tile g inside gathered K/V SBUF tiles
POS = [RMAP[g] * NT + LMAP[g] for g in range(NKT)]


def _rope_perm():
    """Within-head row permutation: pair i=(16*qd + j) real part -> partition
    32*qd + j, imag part -> partition 32*qd + 16 + j."""
    perm = np.zeros(DK, dtype=np.int64)
    for p in range(DK):
        qd, j = p // 32, p % 32
        i = 16 * qd + (j if j < 16 else j - 16)
        perm[p] = 2 * i + (0 if j < 16 else 1)
    return perm


def build_program(sim=False, repeat=1):
    nc = bacc.Bacc("TRN2", target_bir_lowering=False, debug=False,
                   num_devices=1 if sim else N_CORES)

    x_d = nc.declare_dram_parameter("x", [TLOC, D], F32, isOutput=False)
    qkw_d = nc.declare_dram_parameter("qk_w", [2 * H, 128, D], BF16, isOutput=False)
    vw_d = nc.declare_dram_parameter("v_w", [KT, 128, D], BF16, isOutput=False)
    pw_d = nc.declare_dram_parameter("proj_w", [H, 128, D], BF16, isOutput=False)
    gw_d = nc.declare_dram_parameter("gate_w", [FT_FF, 128, D], BF16, isOutput=False)
    uw_d = nc.declare_dram_parameter("up_w", [FT_FF, 128, D], BF16, isOutput=False)
    dw_d = nc.declare_dram_parameter("down_w", [FT_FF, 128, D], BF16, isOutput=False)
    cs1_d = nc.declare_dram_parameter("cs1", [128, TLOC], F32, isOutput=False)
    cs2_d = nc.declare_dram_parameter("cs2", [128, TLOC], F32, isOutput=False)
    tri_d = nc.declare_dram_parameter("tri", [128, NKT * 128], BF16,
                                      isOutput=False)
    out_d = nc.declare_dram_parameter("out", [TLOC, D], F32, isOutput=True)

    with ExitStack() as ctx:
        tc = ctx.enter_context(tile.TileContext(nc))
        for _rep in range(repeat):
            _emit_block(nc, tc, sim, x_d, qkw_d, vw_d, pw_d, gw_d, uw_d, dw_d,
                        cs1_d, cs2_d, tri_d, out_d)

    nc.compile()
    return nc


def _emit_block(nc, tc, sim, x_d, qkw_d, vw_d, pw_d, gw_d, uw_d, dw_d,
                cs1_d, cs2_d, tri_d, out_d):
    with ExitStack() as ctx:
        const = ctx.enter_context(tc.tile_pool(name="const", bufs=1))
        ident = const.tile([128, 128], BF16)
        make_identity(nc, ident)
        ones_col = const.tile([128, 1], BF16)
        nc.vector.memset(ones_col, 1.0)
        ones_row = const.tile([1, 128], F32)
        nc.vector.memset(ones_row, 1.0)
        cs1_sb = const.tile([128, TLOC], F32)
        nc.sync.dma_start(out=cs1_sb[:], in_=cs1_d[:, :])
        cs2_sb = const.tile([128, TLOC], F32)
        nc.sync.dma_start(out=cs2_sb[:], in_=cs2_d[:, :])
        trib_sb = const.tile([128, NKT, 128], BF16)
        nc.gpsimd.dma_start(out=trib_sb[:], in_=tri_d.rearrange(
            "p (n q) -> p n q", n=NKT))

        # shared streaming pool for ALL weight tiles: one rotation across
        # phases lets the DMA queue prefetch phase N+1's weights during
        # phase N's compute (slot = 4KB/partition).
        wflow_cm = tc.tile_pool(name="wflow", bufs=5)
        wflow = wflow_cm.__enter__()

        # DRAM scratch: K/V allgather buffers + x1 spill
        dram = ctx.enter_context(tc.tile_pool(name="dram", bufs=1, space="DRAM"))
        # merged K+V allgather payload: entries 0..H-1 = K heads,
        # H + fb*NT + l = V block (fb, local tile l). One collective per
        # call (collective launches carry ~1ms/call fixed runtime cost).
        kv_local = dram.tile([2 * H, 128, TLOC], BF16)
        kv_full = dram.tile([CORES_PER_B, 2 * H, 128, TLOC], BF16)
        x1_d = dram.tile([NT, 128, D], F32)

        def rmsnorm_transpose(loader, dst_sb, pool, psum_pool):
            """loader(it) -> [128, D] fp32 AP; writes dst_sb [128, KT, TLOC]
            bf16 = (rms-normalized rows) transposed. Gains folded in weights."""
            rstds = []
            for it in range(NT):
                sq_scr = pool.tile([128, D], BF16, name="sq_scr")
                ssq = pool.tile([128, 1], F32, name=f"ssq{it}", tag=f"ssq{it}",
                                bufs=1)
                nc.scalar.activation(sq_scr[:], loader(it), AF.Square,
                                     accum_out=ssq[:])
                mean = pool.tile([128, 1], F32, name="mean")
                nc.vector.tensor_scalar(mean[:], ssq[:], 1.0 / D, EPS,
                                        ALU.mult, ALU.add)
                rec = pool.tile([128, 1], F32, name="rec")
                nc.vector.reciprocal(rec[:], mean[:])
                rstd = pool.tile([128, 1], F32, name=f"rstd{it}",
                                 tag=f"rstd{it}", bufs=1)
                nc.scalar.activation(rstd[:], rec[:], AF.Sqrt)
                rstds.append(rstd)
            hrows = []
            for it in range(NT):
                hrow = pool.tile([128, D], BF16, name=f"hrow{it}",
                                 tag=f"hrow{it}", bufs=1)
                nc.vector.tensor_scalar(hrow[:], loader(it), rstds[it][:],
                                        None, ALU.mult)
                hrows.append(hrow)
            # k-outer so dst_sb[:, k, :] completes early for the consumers
            for k in range(KT):
                for it in range(NT):
                    tp = psum_pool.tile([128, 128], BF16, name="tp")
                    nc.tensor.transpose(tp[:], hrows[it][:, k * 128:(k + 1) * 128],
                                        ident[:])
                    nc.vector.tensor_copy(dst_sb[:, k, it * 128:(it + 1) * 128],
                                          tp[:])

        def rope_evict(ps, dst, pool):
            """ps: [128, TLOC] psum q/k head tile (permuted lanes) -> rotated"""
            sh = pool.tile([128, TLOC], F32, name="rp_sh")
            nc.vector.stream_shuffle(sh[:], ps[:], mask=SHUF_MASK)
            t1 = pool.tile([128, TLOC], F32, name="rp_t1")
            nc.vector.tensor_tensor(t1[:], ps[:], cs1_sb[:], ALU.mult)
            t2 = pool.tile([128, TLOC], F32, name="rp_t2")
            nc.vector.tensor_tensor(t2[:], sh[:], cs2_sb[:], ALU.mult)
            nc.vector.tensor_tensor(dst[:], t1[:], t2[:], ALU.add)

        # persistent pools, strict LIFO
        qrot_cm = tc.tile_pool(name="qrot_pool", bufs=1)
        qrot_pool = qrot_cm.__enter__()
        qrot_sb = qrot_pool.tile([128, H, TLOC], BF16)
        hT_cm = tc.tile_pool(name="hT_pool", bufs=1)
        hT_pool = hT_cm.__enter__()
        hT_sb = hT_pool.tile([128, KT, TLOC], BF16)

        # ---------------- phase 1: norm1 + h^T ----------------
        x_cm = tc.tile_pool(name="xpool", bufs=1)
        xpool = x_cm.__enter__()
        x_sb = xpool.tile([128, NT, D], F32)
        for it in range(NT):
            eng = nc.sync if it % 2 == 0 else nc.gpsimd
            eng.dma_start(out=x_sb[:, it, :],
                          in_=x_d[it * 128:(it + 1) * 128, :])
        # prefetch the first K-head weight tiles behind the x loads
        qk_wts = {}
        for h in range(2):
            wt = wflow.tile([128, KT, 128], BF16, name="qk_wt")
            nc.sync.dma_start(out=wt[:], in_=qkw_d[H + h].rearrange(
                "p (k c) -> p k c", k=KT))
            qk_wts[h] = wt
        with ExitStack() as ph:
            pool = ph.enter_context(tc.tile_pool(name="n1_pool", bufs=2))
            psum_pool = ph.enter_context(
                tc.tile_pool(name="n1_psum", bufs=4, space="PSUM"))
            rmsnorm_transpose(lambda it: x_sb[:, it, :], hT_sb, pool, psum_pool)
        x_cm.__exit__(None, None, None)

        # ---------------- phase 2a: K heads + allgather ----------------
        with ExitStack() as ph:
            spool = ph.enter_context(tc.tile_pool(name="k_s", bufs=3))
            pspool = ph.enter_context(
                tc.tile_pool(name="k_ps", bufs=3, space="PSUM"))
            for h in range(H):
                if h in qk_wts:
                    wt = qk_wts.pop(h)
                else:
                    wt = wflow.tile([128, KT, 128], BF16, name="qk_wt")
                    nc.sync.dma_start(out=wt[:], in_=qkw_d[H + h].rearrange(
                        "p (k c) -> p k c", k=KT))
                ps = pspool.tile([128, TLOC], F32, name="qk_ps")
                for k in range(KT):
                    nc.tensor.matmul(ps[:], wt[:, k, :], hT_sb[:, k, :],
                                     start=(k == 0), stop=(k == KT - 1))
                krot = spool.tile([128, TLOC], BF16, name="krot")
                rope_evict(ps, krot[:], spool)
                nc.gpsimd.dma_start(out=kv_local[h], in_=krot[:])

        kt_cm = tc.tile_pool(name="ktpool", bufs=3)
        ktpool = kt_cm.__enter__()

        # V columns for attention rotate per 512-col block (2 resident:
        # heads 4fb..4fb+3 consume block fb while fb+1 streams in)
        vall_cm = tc.tile_pool(name="vall_pool", bufs=2)
        vap = vall_cm.__enter__()
        v_fbs = []

        # ---------------- phase 2b: V + allgather ----------------
        with ExitStack() as ph:
            vwpool = ph.enter_context(tc.tile_pool(name="vw_pool", bufs=6))
            spool = ph.enter_context(tc.tile_pool(name="v_s", bufs=3))
            vpspool = ph.enter_context(
                tc.tile_pool(name="v_psp", bufs=1, space="PSUM"))
            for fb in range(NFB):
                vps = [vpspool.tile([128, 512], F32, name=f"v_ps{it}",
                                    tag=f"v_ps{it}") for it in range(NT)]
                for k in range(KT):
                    vwt = vwpool.tile([128, 512], BF16, name="vwt")
                    nc.sync.dma_start(out=vwt[:],
                                      in_=vw_d[k][:, fb * 512:(fb + 1) * 512])
                    for it in range(NT):
                        nc.tensor.matmul(vps[it][:],
                                         hT_sb[:, k, it * 128:(it + 1) * 128],
                                         vwt[:], start=(k == 0), stop=(k == KT - 1))
                for it in range(NT):
                    vsb = spool.tile([128, 512], BF16, name="vsb")
                    nc.scalar.copy(vsb[:], vps[it][:])
                    nc.gpsimd.dma_start(out=kv_local[H + fb * NT + it],
                                      in_=vsb[:])
            if sim:
                for r in range(CORES_PER_B):
                    nc.gpsimd.dma_start(out=kv_full[r], in_=kv_local[:])
            else:
                nc.gpsimd.collective_compute(
                    "AllGather", ALU.bypass,
                    replica_groups=[[0, 1, 2, 3], [4, 5, 6, 7]],
                    ins=[kv_local.opt()], outs=[kv_full.opt()],
                )
            for fb in range(NFB):
                v_fb = vap.tile([128, NKT, 512], BF16, name="v_fb")
                for r in range(CORES_PER_B):
                    nc.gpsimd.dma_start(
                        out=v_fb[:, r * NT:(r + 1) * NT, :],
                        in_=kv_full[r, H + fb * NT:H + fb * NT + NT].rearrange(
                            "l p c -> p l c"))
                v_fbs.append(v_fb)

        # stage the first heads' K columns (gather overlaps the Q pass)
        kT_tiles = {}
        for h in range(2):
            kT_sb = ktpool.tile([128, T], BF16, name="kT_sb")
            for r in range(CORES_PER_B):
                nc.gpsimd.dma_start(
                    out=kT_sb[:, r * TLOC:(r + 1) * TLOC],
                    in_=kv_full[r, h])
            kT_tiles[h] = kT_sb

        # ---------------- phase 2c: Q heads + rope (KV gather overlaps) --
        with ExitStack() as ph:
            qwpool = ph.enter_context(tc.tile_pool(name="q_w", bufs=3))
            spool = ph.enter_context(tc.tile_pool(name="q_s", bufs=3))
            pspool = ph.enter_context(
                tc.tile_pool(name="q_ps", bufs=3, space="PSUM"))
            for h in range(H):
                wt = qwpool.tile([128, KT, 128], BF16, name="q_wt")
                nc.sync.dma_start(out=wt[:], in_=qkw_d[h].rearrange(
                    "p (k c) -> p k c", k=KT))
                ps = pspool.tile([128, TLOC], F32, name="qk_ps")
                for k in range(KT):
                    nc.tensor.matmul(ps[:], wt[:, k, :], hT_sb[:, k, :],
                                     start=(k == 0), stop=(k == KT - 1))
                rope_evict(ps, qrot_sb[:, h, :], spool)

        y_cm = tc.tile_pool(name="y_pool", bufs=1)
        y_pool = y_cm.__enter__()
        y_sb = y_pool.tile([128, H, TLOC], BF16)

        # ---------------- phase 3: attention (snake-folded causal) -------
        # kt block l=kt//4 covers local query cols [l*128:512); the first
        # 128 cols get the data-driven causal mask, the rest are always
        # fully allowed by construction of the snake fold.
        with ExitStack() as ph:
            apool = ph.enter_context(tc.tile_pool(name="att_pool", bufs=2))
            epool = ph.enter_context(tc.tile_pool(name="exp_pool", bufs=6))
            aps = ph.enter_context(tc.tile_pool(name="att_ps", bufs=2, space="PSUM"))
            sps_pool = ph.enter_context(
                tc.tile_pool(name="sps_pool", bufs=3, space="PSUM"))
            bps_pool = ph.enter_context(
                tc.tile_pool(name="bps_pool", bufs=1, space="PSUM"))

            for h in range(H):
                if h in kT_tiles:
                    kT_sb = kT_tiles.pop(h)
                else:
                    kT_sb = ktpool.tile([128, T], BF16, name="kT_sb")
                    for r in range(CORES_PER_B):
                        nc.gpsimd.dma_start(
                            out=kT_sb[:, r * TLOC:(r + 1) * TLOC],
                            in_=kv_full[r, h])
                yps = aps.tile([128, TLOC], F32, name="y_ps", tag="y_ps")
                sums = aps.tile([1, TLOC], F32, name="sums_ps", tag="sums_ps")
                for kt in range(NKT):
                    c0 = (kt // 4) * 128
                    w = TLOC - c0
                    kp = POS[kt]
                    sps = sps_pool.tile([128, TLOC], F32, name="s_ps",
                                        tag="s_ps")
                    nc.tensor.matmul(sps[:, :w], kT_sb[:, kp * 128:(kp + 1) * 128],
                                     qrot_sb[:, h, c0:TLOC], start=True,
                                     stop=True)
                    em = epool.tile([128, TLOC], BF16, name="em")
                    nc.scalar.activation(em[:, :w], sps[:, :w], AF.Exp,
                                         scale=SCALE)
                    nc.vector.tensor_tensor(em[:, 0:128], em[:, 0:128],
                                            trib_sb[:, kt, :], ALU.mult)
                    nc.tensor.matmul(yps[:, c0:TLOC],
                                     v_fbs[h // 4][:, kp,
                                                   (h % 4) * 128:
                                                   (h % 4 + 1) * 128],
                                     em[:, :w], start=(kt == 0),
                                     stop=(kt == NKT - 1),
                                     skip_group_check=True)
                    nc.tensor.matmul(sums[:, c0:TLOC], ones_col[:], em[:, :w],
                                     start=(kt == 0), stop=(kt == NKT - 1),
                                     skip_group_check=True)
                rec = apool.tile([1, TLOC], F32, name="rec_att")
                nc.vector.reciprocal(rec[:], sums[:])
                bps = bps_pool.tile([128, TLOC], F32, name="b_ps", tag="b_ps")
                nc.tensor.matmul(bps[:], ones_row[:], rec[:], start=True,
                                 stop=True)
                bsb = apool.tile([128, TLOC], F32, name="bsb")
                nc.vector.tensor_copy(bsb[:], bps[:])
                nc.vector.tensor_tensor(y_sb[:, h, :], yps[:], bsb[:], ALU.mult)

        # ------- phase 4: proj + residual -> x1 (DRAM spill) -------------
        with ExitStack() as ph:
            spool = ph.enter_context(tc.tile_pool(name="pj_s", bufs=4))
            pps = ph.enter_context(tc.tile_pool(name="pj_ps", bufs=2, space="PSUM"))
            for fb in range(NFB):
                pps_t = [pps.tile([128, 512], F32, name=f"p_ps{it}",
                                  tag=f"p_ps{it}") for it in range(NT)]
                for hd in range(H):
                    pwt = wflow.tile([128, 512], BF16, name="pwt")
                    nc.sync.dma_start(out=pwt[:],
                                      in_=pw_d[hd][:, fb * 512:(fb + 1) * 512])
                    for it in range(NT):
                        nc.tensor.matmul(pps_t[it][:],
                                         y_sb[:, hd, it * 128:(it + 1) * 128],
                                         pwt[:], start=(hd == 0),
                                         stop=(hd == H - 1))
                for it in range(NT):
                    xr = spool.tile([128, 512], F32, name="xr_p")
                    nc.sync.dma_start(
                        out=xr[:],
                        in_=x_d[it * 128:(it + 1) * 128,
                                fb * 512:(fb + 1) * 512])
                    x1t = spool.tile([128, 512], F32, name="x1t")
                    nc.vector.tensor_tensor(x1t[:], pps_t[it][:], xr[:],
                                            ALU.add)
                    nc.sync.dma_start(
                        out=x1_d[it][:, fb * 512:(fb + 1) * 512], in_=x1t[:])

        y_cm.__exit__(None, None, None)
        vall_cm.__exit__(None, None, None)
        kt_cm.__exit__(None, None, None)
        hT_cm.__exit__(None, None, None)
        qrot_cm.__exit__(None, None, None)

        # ---------------- phase 5: norm2 + h2^T ----------------
        h2T_cm = tc.tile_pool(name="h2T_pool", bufs=1)
        h2T_pool = h2T_cm.__enter__()
        h2T_sb = h2T_pool.tile([128, KT, TLOC], BF16)
        with ExitStack() as ph:
            x1p = ph.enter_context(tc.tile_pool(name="x1r_pool", bufs=1))
            x1rows = []
            for it in range(NT):
                x1r = x1p.tile([128, D], F32, name=f"x1r{it}", tag=f"x1r{it}")
                eng = nc.sync if it % 2 == 0 else nc.gpsimd
                eng.dma_start(out=x1r[:], in_=x1_d[it])
                x1rows.append(x1r)
            pool = ph.enter_context(tc.tile_pool(name="n2_pool", bufs=2))
            psum_pool = ph.enter_context(
                tc.tile_pool(name="n2_psum", bufs=4, space="PSUM"))
            rmsnorm_transpose(lambda it: x1rows[it][:], h2T_sb, pool, psum_pool)

        # ---------------- phase 6: gate/up ----------------
        gu_cm = tc.tile_pool(name="gu_pool", bufs=1)
        gu_pool = gu_cm.__enter__()
        gu_sb = gu_pool.tile([128, FT_FF, TLOC], BF16)
        with ExitStack() as ph:
            spool = ph.enter_context(tc.tile_pool(name="mlp_s", bufs=3))
            mps = ph.enter_context(tc.tile_pool(name="mlp_ps", bufs=4, space="PSUM"))
            for f in range(FT_FF):
                gwt = wflow.tile([128, KT, 128], BF16, name="gwt")
                nc.sync.dma_start(out=gwt[:], in_=gw_d[f].rearrange(
                    "p (k c) -> p k c", k=KT))
                gps = mps.tile([128, TLOC], F32, name="g_ps", tag="g_ps")
                for k in range(KT):
                    nc.tensor.matmul(gps[:], gwt[:, k, :], h2T_sb[:, k, :],
                                     start=(k == 0), stop=(k == KT - 1))
                gsil = spool.tile([128, TLOC], BF16, name="gsil")
                nc.scalar.activation(gsil[:], gps[:], AF.Silu)
                uwt = wflow.tile([128, KT, 128], BF16, name="uwt")
                nc.sync.dma_start(out=uwt[:], in_=uw_d[f].rearrange(
                    "p (k c) -> p k c", k=KT))
                ups = mps.tile([128, TLOC], F32, name="u_ps", tag="u_ps")
                for k in range(KT):
                    nc.tensor.matmul(ups[:], uwt[:, k, :], h2T_sb[:, k, :],
                                     start=(k == 0), stop=(k == KT - 1))
                nc.vector.tensor_tensor(gu_sb[:, f, :], ups[:], gsil[:],
                                        ALU.mult)

        # ---------------- phase 7: down + residual -> out ----------------
        with ExitStack() as ph:
            spool = ph.enter_context(tc.tile_pool(name="dn_s", bufs=8))
            dps = ph.enter_context(tc.tile_pool(name="dn_ps", bufs=1, space="PSUM"))
            for fbp in range(2):
                dps_t = [[dps.tile([128, 512], F32, name=f"d_ps{it}_{fbi}",
                                   tag=f"d_ps{it}_{fbi}") for fbi in range(2)]
                         for it in range(NT)]
                for k in range(FT_FF):
                    dwt = wflow.tile([128, 1024], BF16, name="dwt")
                    nc.sync.dma_start(
                        out=dwt[:],
                        in_=dw_d[k][:, fbp * 1024:(fbp + 1) * 1024])
                    for it in range(NT):
                        for fbi in range(2):
                            nc.tensor.matmul(
                                dps_t[it][fbi][:],
                                gu_sb[:, k, it * 128:(it + 1) * 128],
                                dwt[:, fbi * 512:(fbi + 1) * 512],
                                start=(k == 0), stop=(k == FT_FF - 1))
                for it in range(NT):
                    for fbi in range(2):
                        fb = fbp * 2 + fbi
                        xr = spool.tile([128, 512], F32, name="xr_d")
                        nc.gpsimd.dma_start(
                            out=xr[:],
                            in_=x1_d[it][:, fb * 512:(fb + 1) * 512])
                        osb = spool.tile([128, 512], F32, name="osb_d")
                        nc.vector.tensor_tensor(
                            osb[:], dps_t[it][fbi][:], xr[:], ALU.add)
                        nc.sync.dma_start(
                            out=out_d[it * 128:(it + 1) * 128,
                                      fb * 512:(fb + 1) * 512],
                            in_=osb[:])

        gu_cm.__exit__(None, None, None)
        h2T_cm.__exit__(None, None, None)
        wflow_cm.__exit__(None, None, None)


def core_token_idx(c):
    """Global token indices (within the batch row) owned by group-core c."""
    return np.concatenate([np.arange(g * 128, (g + 1) * 128)
                           for g in snake_tiles(c)])


def prepare_inputs(x, f_cos, f_sin, w_attn, w_proj, w_gate, w_up, w_down, g1, g2):
    """Host-side sharding + weight re-layout. Returns list of 8 input dicts."""
    x = np.asarray(x, dtype=np.float32)
    f_cos = np.asarray(f_cos, dtype=np.float32)
    f_sin = np.asarray(f_sin, dtype=np.float32)
    w_attn = np.asarray(w_attn, dtype=np.float32)
    g1 = np.asarray(g1, dtype=np.float32)
    g2 = np.asarray(g2, dtype=np.float32)

    perm = _rope_perm()
    wq = w_attn[0:D] * g1[None, :]
    wk = w_attn[D:2 * D] * g1[None, :]
    wv = w_attn[2 * D:3 * D] * g1[None, :]
    # permute rows within each head for q and k
    wq_p = wq.reshape(H, DK, D)[:, perm, :].reshape(H * DK, D)
    wk_p = wk.reshape(H, DK, D)[:, perm, :].reshape(H * DK, D)

    def lhsT_layout(w):  # w: [F, D] -> [F/128, 128(d within k-tile), D(k*128+c)]
        f = w.shape[0]
        # out[ft, p, k*128+c] = w[ft*128+c, k*128+p]
        a = w.reshape(f // 128, 128, KT, 128)       # [ft, c, k, p]
        a = a.transpose(0, 3, 2, 1).reshape(f // 128, 128, D)  # [ft, p, (k c)]
        return np.ascontiguousarray(a).astype(ml_dtypes.bfloat16)

    def rhsT_layout(w):  # w: [F, D_in] -> [D_in/128, 128(p), F] = w.T tiled
        d_in = w.shape[1]
        a = w.T.reshape(d_in // 128, 128, w.shape[0])  # [k, p, c]
        return np.ascontiguousarray(a).astype(ml_dtypes.bfloat16)

    qk_w = np.concatenate([lhsT_layout(wq_p), lhsT_layout(wk_p)], axis=0)
    v_w = rhsT_layout(wv)
    proj_w = rhsT_layout(np.asarray(w_proj, dtype=np.float32))
    gate_w = lhsT_layout(np.asarray(w_gate, dtype=np.float32) * g2[None, :])
    up_w = lhsT_layout(np.asarray(w_up, dtype=np.float32) * g2[None, :])
    down_w = rhsT_layout(np.asarray(w_down, dtype=np.float32))

    # cs1/cs2 in permuted-lane layout: [128, T]
    pair = np.zeros(DK, dtype=np.int64)
    sign = np.zeros(DK, dtype=np.float32)
    for p in range(DK):
        qd, j = p // 32, p % 32
        pair[p] = 16 * qd + (j if j < 16 else j - 16)
        sign[p] = -1.0 if j < 16 else 1.0
    cs1_full = f_cos.T[pair, :]                       # [128, T]
    cs2_full = f_sin.T[pair, :] * sign[:, None]       # [128, T]

    in_maps = []
    for core in range(N_CORES):
        b, c = core // CORES_PER_B, core % CORES_PER_B
        tok = core_token_idx(c)
        tiles = snake_tiles(c)
        # causal mask tiles: kt covers query tile l=kt//4 (this core's
        # global tile tiles[l]); allowed iff key_pos <= query_pos
        tri = np.zeros((NKT, 128, 128), dtype=np.float32)
        kk = np.arange(128)[:, None]
        qq = np.arange(128)[None, :]
        for kt in range(NKT):
            g = tiles[kt // 4]
            tri[kt] = (kt * 128 + kk) <= (g * 128 + qq)
        tri = np.ascontiguousarray(
            tri.transpose(1, 0, 2).reshape(128, NKT * 128))
        in_maps.append({
            "x": np.ascontiguousarray(x[b, tok, :]),
            "qk_w": qk_w, "v_w": v_w, "proj_w": proj_w,
            "gate_w": gate_w, "up_w": up_w, "down_w": down_w,
            "cs1": np.ascontiguousarray(cs1_full[:, tok]),
            "cs2": np.ascontiguousarray(cs2_full[:, tok]),
            "tri": tri.astype(ml_dtypes.bfloat16),
        })
    return in_maps


def assemble_output(results):
    out = np.zeros((B, T, D), dtype=np.float32)
    for core in range(N_CORES):
        b, c = core // CORES_PER_B, core % CORES_PER_B
        out[b, core_token_idx(c), :] = results[core]["out"]
    return out


_CACHE = {}
_LOCK = threading.Lock()


def get_program():
    with _LOCK:
        if "nc" not in _CACHE:
            _CACHE["nc"] = build_program()
        return _CACHE["nc"]


def kernel(**inputs):
    nc = get_program()
    in_maps = prepare_inputs(**inputs)
    res = run_bass_kernel_spmd(nc, in_maps, list(range(N_CORES)))
    return assemble_output(res.results)


def bench(inputs, iters=10):
    """Wall-clock the sharded executable with device-resident inputs.

    Returns the mean pipelined per-call time in ns (upper bound on HW exec
    time: it includes 1/iters of the axon dispatch round-trip)."""
    import jax
    from jax.sharding import Mesh, PartitionSpec, NamedSharding
    from jax.experimental.shard_map import shard_map
    from concourse import bass2jax, mybir as mb

    nc = get_program()
    in_maps = prepare_inputs(**inputs)
    bass2jax.install_neuronx_cc_hook()

    partition_name = (nc.partition_id_tensor.name
                      if nc.partition_id_tensor else None)
    in_names, out_names, out_avals, zero_outs = [], [], [], []
    for alloc in nc.m.functions[0].allocations:
        if not isinstance(alloc, mb.MemoryLocationSet):
            continue
        name = alloc.memorylocations[0].name
        if alloc.kind == "ExternalInput":
            if name != partition_name:
                in_names.append(name)
        elif alloc.kind == "ExternalOutput":
            shape = tuple(alloc.tensor_shape)
            dtype = mb.dt.np(alloc.dtype)
            out_names.append(name)
            out_avals.append(jax.core.ShapedArray(shape, dtype))
            zero_outs.append(np.zeros(shape, dtype))
    n_params = len(in_names)
    all_in_names = list(in_names) + list(out_names)
    if partition_name is not None:
        all_in_names.append(partition_name)
    donate = tuple(range(n_params, n_params + len(out_names)))

    def _body(*args):
        operands = list(args)
        if partition_name is not None:
            operands.append(bass2jax.partition_id_tensor())
        return tuple(bass2jax._bass_exec_p.bind(
            *operands,
            out_avals=tuple(out_avals),
            in_names=tuple(all_in_names),
            out_names=tuple(out_names),
            lowering_input_output_aliases=(),
            sim_require_finite=True,
            sim_require_nnan=True,
            nc=nc,
        ))

    devices = jax.devices()[:N_CORES]
    mesh = Mesh(np.asarray(devices), ("core",))
    in_specs = (PartitionSpec("core"),) * (n_params + len(out_names))
    out_specs = (PartitionSpec("core"),) * len(out_names)
    sharded = jax.jit(
        shard_map(_body, mesh=mesh, in_specs=in_specs, out_specs=out_specs,
                  check_rep=False),
        donate_argnums=donate, keep_unused=True)

    sh = NamedSharding(mesh, PartitionSpec("core"))
    concat_in = [
        jax.device_put(
            np.concatenate([np.asarray(in_maps[c][nm]) for c in range(N_CORES)],
                           axis=0), sh)
        for nm in in_names]
    jax.block_until_ready(concat_in)

    def make_zeros():
        return [jax.device_put(
            np.zeros((N_CORES * z.shape[0], *z.shape[1:]), z.dtype), sh)
            for z in zero_outs]

    # warmup (compile)
    outs = sharded(*concat_in, *make_zeros())
    jax.block_until_ready(outs)

    zs = [make_zeros() for _ in range(iters)]
    for z in zs:
        jax.block_until_ready(z)
    # async pipelined dispatch amortizes the ~100ms axon round-trip
    t0 = time.perf_counter()
    outs = [sharded(*concat_in, *zs[i]) for i in range(iters)]
    jax.block_until_ready(outs)
    dt = (time.perf_counter() - t0) / iters
    return dt * 1e9
